# revision 1
# baseline (speedup 1.0000x reference)
"""2-layer GAT on 8 trn2 NeuronCores (Bass/Tile).

Strategy (matches the sharding hint): nodes are partitioned across the 8
cores (12500 each, padded to 12544 = 98*128), each core's nodes are sorted
by in-degree and tiled 128-per-tile.  Edges are assigned to the core owning
their destination.  Three SPMD launches:

  1. "build":  h1 = x @ W1 plus the attention projections, written as a
     per-node gather table T1 (fp16, 256B rows) -- each core builds its own
     node slice from its slice of x^T.
  2. "layer1": per-edge dma_gather of T1 rows (position-major, int16
     windowed), edge softmax via the factorization
        exp(leaky(s+a)) = A * max(exp(s), exp(0.2 s)*exp(-0.8 a))
     (the per-dst factor A cancels in the softmax normalization), segment
     sums via banded 0/1 S-matrices on the PE (PSUM band accumulation),
     producing the layer-2 table T2.
  3. "layer2": same machinery on T2, then out = agg @ W2 and log_softmax.

Between launches the host only concatenates / replicates device-computed
arrays (the halo exchange): T1/T2 slices -> full tables, per-dst r values ->
per-edge streams.  All model math runs on device.
"""

import numpy as np
import ml_dtypes

import concourse.bacc as bacc
import concourse.tile as tile
import concourse.mybir as mybir
from concourse import bass_utils

F32 = mybir.dt.float32
F16 = mybir.dt.float16
F8 = mybir.dt.float8e4
I16 = mybir.dt.int16
AF = mybir.ActivationFunctionType
ALU = mybir.AluOpType
AX = mybir.AxisListType

# problem constants (hardcoded per the task statement)
NCORES = 8
N = 100000
IN = 256
HID = 8
HEADS = 8
OUT = 16
NEG = 0.2
NPC = 12500            # real nodes per core
MPC = 12544            # padded nodes per core (98 * 128)
NT = MPC // 128        # 98 dst tiles per core
BATCH_EDGES = 16384    # shared edge budget per batch
NROWS = NCORES * MPC   # 100352 table rows
WINR = 32512           # gather window rows (int16-safe)
NWIN = (NROWS + WINR - 1) // WINR  # 4
EPS = 1e-16
SLAB1 = 7              # launch-1 chunks per slab (must divide NT)

_CACHE = {}
TRACE = False            # set True to capture HW profiles (exec_time_ns)
DBG = "full"             # debug: "gather" | "edge" | "matmul" | "full"
GSPLIT = 48              # max slabs (x128 idxs) per dma_gather call
SINGLE_PACKET = False


# --------------------------------------------------------------------------
# host-side graph preprocessing (pure index work)
# --------------------------------------------------------------------------

def _preprocess(edge_index):
    src = np.concatenate([np.asarray(edge_index[0]), np.arange(N)])
    dst = np.concatenate([np.asarray(edge_index[1]), np.arange(N)])
    deg = np.bincount(dst, minlength=N)

    # permutation: per core, nodes sorted by degree desc
    pos = np.empty(N, np.int64)
    perm_nodes = np.empty(NROWS, np.int64)   # table row -> node id (or -1)
    perm_nodes.fill(-1)
    for c in range(NCORES):
        ids = np.arange(c * NPC, (c + 1) * NPC)
        order = np.argsort(-deg[ids], kind="stable")
        pos[ids[order]] = c * MPC + np.arange(NPC)
        perm_nodes[c * MPC:c * MPC + NPC] = ids[order]

    srcpos = pos[src]
    dstpos = pos[dst]

    cores = []
    # per-(core, tile, window) counts; shared per-tile offsets across cores
    # keep the chunk->tile structure identical on every core (tight union
    # schedule).  Tiles are packed into variable-size batches by edge budget
    # so SBUF slab sizes stay bounded despite degree skew.
    counts = np.zeros((NCORES, NT, NWIN), np.int64)
    per_core = []
    for c in range(NCORES):
        m = (dst >= c * NPC) & (dst < (c + 1) * NPC)
        sp = srcpos[m]
        rank = dstpos[m] - c * MPC
        t = rank // 128
        w = sp // WINR
        per_core.append((sp, rank, t, w))
        np.add.at(counts[c], (t, w), 1)
    stc = counts.max(0)                              # [NT, NWIN]
    tile_load = stc.sum(1)
    bmap = np.zeros(NT, np.int64)
    acc = 0
    b = 0
    for t in range(NT):
        if acc and acc + tile_load[t] > BATCH_EDGES:
            b += 1
            acc = 0
        bmap[t] = b
        acc += tile_load[t]
    NBAT = int(bmap[-1]) + 1
    btiles = [list(np.where(bmap == bb)[0]) for bb in range(NBAT)]
    # shared tile offsets within each (batch, window)
    toff = np.zeros((NT, NWIN), np.int64)
    gsz = np.zeros((NBAT, NWIN), np.int64)
    for bb in range(NBAT):
        for w in range(NWIN):
            off = 0
            for t in btiles[bb]:
                toff[t, w] = off
                off += stc[t, w]
            gsz[bb, w] = off
    G = np.maximum((gsz + 127) // 128, 1)            # [NBAT, NWIN] slabs
    Q = G * 128
    qoff = np.zeros((NBAT, NWIN), np.int64)
    goff = np.zeros((NBAT, NWIN), np.int64)
    acc_q = 0
    for bb in range(NBAT):
        for w in range(NWIN):
            qoff[bb, w] = acc_q
            goff[bb, w] = acc_q // 128
            acc_q += Q[bb, w]
    TOTQ = acc_q
    TOTG = TOTQ // 128

    # per-core padded position arrays
    for c in range(NCORES):
        sp, rank, t, w = per_core[c]
        b = bmap[t]
        order = np.lexsort((rank, w, t))
        sp, rank, t, w, b = (sp[order], rank[order], t[order], w[order],
                             b[order])
        # within-(tile, window) index
        gid = t * NWIN + w
        gstart = np.searchsorted(gid, np.arange(NT * NWIN), side="left")
        within = np.arange(len(gid)) - gstart[gid]
        q = qoff[b, w] + toff[t, w] + within
        cores.append({"sp": sp, "rank": rank, "b": b, "w": w, "q": q})

    # union matmul schedule, merged per (b, t, w, j) with a band range.
    JMAX = TOTQ // 128 + 1
    keysets = []
    for c in range(NCORES):
        d = cores[c]
        j = (d["q"] - qoff[d["b"], d["w"]]) // 128
        t = d["rank"] // 128
        a = (d["rank"] % 128) // 32
        key = (t * NWIN + d["w"]) * JMAX + j
        keysets.append((key, a))
        d["j"] = j
        d["t"] = t
        d["key"] = key
    allk = np.concatenate([k for k, _ in keysets])
    alla = np.concatenate([a for _, a in keysets])
    ukeys, inv = np.unique(allk, return_inverse=True)
    TOTB = len(ukeys)
    amin = np.full(TOTB, 4, np.int64)
    amax = np.full(TOTB, -1, np.int64)
    np.minimum.at(amin, inv, alla)
    np.maximum.at(amax, inv, alla)
    # band -> (col base, width) in 32-partition units, PE-tile-aligned
    ecol = np.where(amin == amax, amin,
                    np.where((amin == 0) & (amax == 1), 0,
                             np.where((amin == 2) & (amax == 3), 2, 0)))
    ewid = np.where(amin == amax, 1,
                    np.where((amin == 0) & (amax == 1), 2,
                             np.where((amin == 2) & (amax == 3), 2, 4)))
    soff = np.concatenate([[0], np.cumsum(ewid)])   # block col offsets (32u)
    # decode (b, t, w, j)
    uj = ukeys % JMAX
    r1 = ukeys // JMAX
    uw = r1 % NWIN
    ut = r1 // NWIN
    ub = bmap[ut]
    sched = {"b": ub, "t": ut, "w": uw, "j": uj, "col": ecol, "wid": ewid,
             "soff": soff, "n": TOTB, "totw": int(soff[-1])}

    # per-core S fill data (entry id + in-chunk row + in-block col per edge)
    for c in range(NCORES):
        d = cores[c]
        ent = np.searchsorted(ukeys, d["key"])
        d["ent"] = ent
        d["k"] = d["q"] % 128
        d["scol"] = d["rank"] % 128 - ecol[ent] * 32

    meta = {"G": G, "Q": Q, "qoff": qoff, "goff": goff, "TOTQ": TOTQ,
            "TOTG": TOTG, "sched": sched, "pos": pos, "NBAT": NBAT,
            "btiles": btiles, "perm_nodes": perm_nodes, "cores": cores}
    return meta


def _build_idx_and_s(meta):
    """Per-core gather index arrays (int16 wrapped) and fp8 S blocks."""
    TOTQ = meta["TOTQ"]
    idx_all, s_all, streams = [], [], []
    for c in range(NCORES):
        d = meta["cores"][c]
        flat = np.zeros(TOTQ, np.int16)
        loc = d["sp"] - d["w"] * WINR
        flat[d["q"]] = loc.astype(np.int16)
        # wrap: idxw[p, j] = flat[j*16 + p%16]
        resh = flat.reshape(TOTQ // 16, 16).T          # [16, TOTQ/16]
        idxw = np.tile(resh, (8, 1)).copy()            # [128, TOTQ/16]
        idx_all.append(idxw)

        soff = meta["sched"]["soff"]
        totw = meta["sched"]["totw"]
        S = np.zeros((128, totw * 32), ml_dtypes.float8_e4m3)
        S[d["k"], soff[d["ent"]] * 32 + d["scol"]] = 1.0
        s_all.append(S)

        # per-position (p, g, rank) for the r streams
        streams.append((d["q"] % 128, d["q"] // 128, d["rank"]))
    return idx_all, s_all, streams


def _expand_stream(stream, r_core, width, totg):
    """r_core [MPC, width] f32 -> per-position [128, totg, width] f16."""
    p, g, rank = stream
    out = np.zeros((128, int(totg), width), np.float16)
    out[p, g, :] = r_core[rank, :width].astype(np.float16)
    return out


# --------------------------------------------------------------------------
# launch builders
# --------------------------------------------------------------------------

def _new_nc():
    return bacc.Bacc("TRN2", target_bir_lowering=False, debug=False,
                     enable_asserts=False, num_devices=NCORES)


def _build_launch1():
    nc = _new_nc()
    xs_d = nc.dram_tensor("xs", [IN, MPC], F16, kind="ExternalInput")
    wc_d = nc.dram_tensor("wc", [IN, 80], F16, kind="ExternalInput")
    t1_d = nc.dram_tensor("t1s", [MPC, 128], F16, kind="ExternalOutput")
    r1_d = nc.dram_tensor("r1", [MPC, 8], F32, kind="ExternalOutput")
    SLAB = SLAB1
    with tile.TileContext(nc) as tc:
        with tc.tile_pool(name="w", bufs=1) as wp, \
             tc.tile_pool(name="x", bufs=3) as xp, \
             tc.tile_pool(name="o", bufs=3) as op, \
             tc.tile_pool(name="ps", bufs=4, space="PSUM") as pp:
            wc_sb = wp.tile([128, 2, 80], F16)
            nc.sync.dma_start(wc_sb[:, 0, :], wc_d.ap()[0:128, :])
            nc.sync.dma_start(wc_sb[:, 1, :], wc_d.ap()[128:256, :])
            for s in range(NT // SLAB):
                cols = slice(s * SLAB * 128, (s + 1) * SLAB * 128)
                xt0 = xp.tile([128, SLAB * 128], F16, tag="xt0")
                xt1 = xp.tile([128, SLAB * 128], F16, tag="xt1")
                nc.sync.dma_start(xt0[:], xs_d.ap()[0:128, cols])
                nc.sync.dma_start(xt1[:], xs_d.ap()[128:256, cols])
                tout = op.tile([128, SLAB, 128], F16, tag="tout")
                rout = op.tile([128, SLAB, 8], F32, tag="rout")
                for i in range(SLAB):
                    ps = pp.tile([128, 80], F32)
                    nc.tensor.matmul(ps[:], lhsT=xt0[:, i * 128:(i + 1) * 128],
                                     rhs=wc_sb[:, 0, :], start=True, stop=False)
                    nc.tensor.matmul(ps[:], lhsT=xt1[:, i * 128:(i + 1) * 128],
                                     rhs=wc_sb[:, 1, :], start=False, stop=True)
                    nc.vector.tensor_copy(tout[:, i, 0:64], ps[:, 0:64])
                    nc.scalar.activation(out=tout[:, i, 64:72],
                                         in_=ps[:, 64:72], func=AF.Exp)
                    nc.scalar.activation(out=tout[:, i, 72:80],
                                         in_=ps[:, 64:72], func=AF.Exp, scale=0.2)
                    nc.vector.memset(tout[:, i, 80:128], 0.0)
                    nc.scalar.activation(out=rout[:, i, :], in_=ps[:, 72:80],
                                         func=AF.Exp, scale=-0.8)
                rows = slice(s * SLAB * 128, (s + 1) * SLAB * 128)
                nc.sync.dma_start(
                    t1_d.ap()[rows, :].rearrange("(i p) f -> p i f", p=128),
                    tout[:])
                nc.scalar.dma_start(
                    r1_d.ap()[rows, :].rearrange("(i p) f -> p i f", p=128),
                    rout[:])
    nc.compile()
    return nc


def _emit_msg_layer(nc, tc, meta, tab_d, idx_d, s_d, re_d, finalize,
                    rwidth, mwidth):
    """Shared structure of launches 2/3.

    rwidth: per-edge r width (8 for L1, 1 for L2); mwidth: matmul rhs width
    (72 for L1: 64 msg + 8 den; 65 for L2).  `finalize` supplies the
    per-edge elementwise ops and the per-dst-tile epilogue.
    """
    G, qoff, goff = meta["G"], meta["qoff"], meta["goff"]
    TOTQ = meta["TOTQ"]
    sched = meta["sched"]
    sb, st, sw, sj = (sched[k] for k in ("b", "t", "w", "j"))
    scol, swid, soff = sched["col"], sched["wid"], sched["soff"]
    TOTB = sched["n"]
    ent_by_t = {}
    for i in range(TOTB):
        ent_by_t.setdefault(int(st[i]), []).append(i)
    NBAT = meta["NBAT"]
    btiles = meta["btiles"]
    blo = np.searchsorted(sb, np.arange(NBAT))
    bhi = np.searchsorted(sb, np.arange(NBAT), side="right")
    # batch S-column ranges (32-unit blocks)
    slo = [int(soff[blo[b]]) for b in range(NBAT)]
    shi = [int(soff[bhi[b]]) for b in range(NBAT)]
    nw32max = max(1, max(shi[b] - slo[b] for b in range(NBAT)))
    qb_lo = [int(qoff[b, 0]) for b in range(NBAT)]
    qb_hi = [int(qoff[b, NWIN - 1] + G[b, NWIN - 1] * 128)
             for b in range(NBAT)]
    qbmax = max(qb_hi[b] - qb_lo[b] for b in range(NBAT))

    with tc.tile_pool(name="resident", bufs=1) as rp, \
         tc.tile_pool(name="gslab", bufs=2) as gp, \
         tc.tile_pool(name="mslab", bufs=1) as mp, \
         tc.tile_pool(name="work", bufs=3) as wkp, \
         tc.tile_pool(name="fin", bufs=3) as fp, \
         tc.tile_pool(name="psA", bufs=3, space="PSUM") as ppA, \
         tc.tile_pool(name="psB", bufs=2, space="PSUM") as ppB:
        pools = (rp, gp, mp, wkp, fp, ppA, ppB)
        zrow = rp.tile([1, 128], F16)
        nc.vector.memset(zrow[:], 0.0)
        cst_sb = finalize.load_consts(nc, rp)
        for b in range(NBAT):
            nw32 = max(shi[b] - slo[b], 1)
            ssb = mp.tile([128, nw32max, 32], F8, tag="s", bufs=2)
            if shi[b] > slo[b]:
                nc.sync.dma_start(
                    ssb[:, 0:nw32, :],
                    s_d.ap()[:, slo[b] * 32:shi[b] * 32]
                    .rearrange("p (n c) -> p n c", c=32))
            nq = qb_hi[b] - qb_lo[b]
            idx_sb = mp.tile([128, qbmax // 16], I16, tag="idx", bufs=2)
            nc.sync.dma_start(idx_sb[:, 0:nq // 16],
                              idx_d.ap()[:, qb_lo[b] // 16:qb_hi[b] // 16])
            slabs = {}
            for w in range(NWIN):
                g = int(G[b, w])
                q0 = int(qoff[b, w]) - qb_lo[b]
                g0 = int(goff[b, w])
                Gs = gp.tile([128, g, 128], F16, tag="G", bufs=4)
                win0 = w * WINR
                win1 = min(win0 + WINR, NROWS)
                for g1 in range(0, g, GSPLIT):
                    g2 = min(g1 + GSPLIT, g)
                    nn = (g2 - g1) * 128
                    nc.gpsimd.dma_gather(
                        out_ap=Gs[:, g1:g2, :], in_ap=tab_d.ap()[win0:win1, :],
                        idxs_ap=idx_sb[:, (q0 + g1 * 128) // 16:
                                       (q0 + g2 * 128) // 16],
                        num_idxs=nn, num_idxs_reg=nn, elem_size=128,
                        single_packet=SINGLE_PACKET)
                rs = gp.tile([128, g, rwidth], F16, tag="rs")
                nc.scalar.dma_start(
                    rs[:], re_d.ap()[:, g0 * rwidth:(g0 + g) * rwidth]
                    .rearrange("p (g r) -> p g r", r=rwidth))
                msg = mp.tile([128, g, mwidth], F16, tag="msg", bufs=6)
                el = wkp.tile([128, g, rwidth], F16, tag="el")
                if DBG != "gather":
                    finalize.edge_ops(nc, Gs, rs, el, msg)
                slabs[w] = msg
            # matmuls + finalize, tile-major within the batch
            if DBG in ("gather", "edge"):
                continue
            for t in btiles[b]:
                ents = ent_by_t.get(t, [])
                ps = ppA.tile([128, mwidth], F32, tag="ps")
                nc.tensor.matmul(ps[:], lhsT=zrow[:],
                                 rhs=zrow[:, 0:mwidth], start=True, stop=False,
                                 skip_group_check=True)
                for n, i in enumerate(ents):
                    w, j = int(sw[i]), int(sj[i])
                    col, wid = int(scol[i]), int(swid[i])
                    so = int(soff[i]) - slo[b]
                    nc.tensor.matmul(
                        ps[col * 32:(col + wid) * 32, :],
                        lhsT=ssb[:, so:so + wid, :]
                        .rearrange("p n c -> p (n c)"),
                        rhs=slabs[w][:, j, :],
                        start=False, stop=(n == len(ents) - 1),
                        tile_position=(0, col * 32),
                        skip_group_check=True)
                if DBG == "full":
                    finalize.tile_ops(nc, pools, t, ps, cst_sb)
        if DBG == "full" and hasattr(finalize, "finish"):
            finalize.finish(nc)


class _L1Final:
    """Layer-1 epilogue: softmax normalize, ELU, build T2 row + r2."""

    def __init__(self, nc, c2_d, t2_d, r2_d):
        self.c2_d, self.t2_d, self.r2_d = c2_d, t2_d, r2_d
        self.r2_sb = None

    def load_consts(self, nc, rp):
        c2 = rp.tile([128, 128], F16)
        nc.sync.dma_start(c2[:], self.c2_d.ap())
        self.r2_sb = rp.tile([128, NT], F32)
        return c2

    def finish(self, nc):
        nc.scalar.dma_start(
            self.r2_d.ap().rearrange("(t p) o -> p (t o)", p=128), self.r2_sb[:])

    def edge_ops(self, nc, Gs, rs, el, msg):
        g = Gs.shape[1]
        nc.vector.tensor_tensor(out=el[:], in0=Gs[:, :, 72:80], in1=rs[:],
                                op=ALU.mult)
        nc.vector.tensor_tensor(out=el[:], in0=Gs[:, :, 64:72], in1=el[:],
                                op=ALU.max)
        nc.vector.tensor_tensor(
            out=msg[:, :, 0:64].rearrange("p g (h c) -> p g h c", h=8),
            in0=Gs[:, :, 0:64].rearrange("p g (h c) -> p g h c", h=8),
            in1=el[:].to_broadcast([128, g, 8, 8]), op=ALU.mult)
        nc.vector.tensor_copy(msg[:, :, 64:72], el[:])

    def tile_ops(self, nc, pools, t, ps, c2):
        rp, gp, mp, wkp, fp, ppA, ppB = pools
        den = fp.tile([128, 8], F32, tag="den")
        nc.vector.tensor_scalar_add(den[:], ps[:, 64:72], EPS)
        rec = fp.tile([128, 8], F32, tag="rec")
        nc.vector.reciprocal(rec[:], den[:])
        y = fp.tile([128, 64], F32, tag="y")
        nc.vector.tensor_tensor(
            out=y[:].rearrange("p (h c) -> p h c", h=8),
            in0=ps[:, 0:64].rearrange("p (h c) -> p h c", h=8),
            in1=rec[:].to_broadcast([128, 8, 8]), op=ALU.mult)
        # ELU: g = max(y, exp(min(y,0)) - 1)
        yn = fp.tile([128, 64], F32, tag="yn")
        nc.vector.tensor_scalar_min(yn[:], y[:], 0.0)
        ey = fp.tile([128, 64], F32, tag="ey")
        nc.scalar.activation(out=ey[:], in_=yn[:], func=AF.Exp)
        nc.vector.tensor_scalar_add(ey[:], ey[:], -1.0)
        t2t = fp.tile([128, 128], F16, tag="t2t")
        nc.vector.tensor_tensor(out=t2t[:, 0:64], in0=y[:], in1=ey[:],
                                op=ALU.max)
        gw = fp.tile([128, 64], F32, tag="gw")
        nc.vector.tensor_tensor(out=gw[:], in0=t2t[:, 0:64], in1=c2[:, 0:64],
                                op=ALU.mult)
        as2 = fp.tile([128, 1], F32, tag="as2")
        nc.vector.tensor_reduce(out=as2[:], in_=gw[:], axis=AX.X, op=ALU.add)
        nc.scalar.activation(out=t2t[:, 64:65], in_=as2[:], func=AF.Exp)
        nc.scalar.activation(out=t2t[:, 65:66], in_=as2[:], func=AF.Exp,
                             scale=0.2)
        nc.vector.tensor_tensor(out=gw[:], in0=t2t[:, 0:64], in1=c2[:, 64:128],
                                op=ALU.mult)
        ad2 = fp.tile([128, 1], F32, tag="ad2")
        nc.vector.tensor_reduce(out=ad2[:], in_=gw[:], axis=AX.X, op=ALU.add)
        nc.scalar.activation(out=self.r2_sb[:, t:t + 1], in_=ad2[:],
                             func=AF.Exp, scale=-0.8)
        rows = slice(t * 128, (t + 1) * 128)
        nc.scalar.dma_start(self.t2_d.ap()[rows, :], t2t[:])


class _L2Final:
    """Layer-2 epilogue: normalize, @W2, log_softmax."""

    def __init__(self, nc, w2_d, id_d, o_d):
        self.w2_d, self.id_d, self.o_d = w2_d, id_d, o_d

    def load_consts(self, nc, rp):
        w2 = rp.tile([64, 16], F32)
        nc.sync.dma_start(w2[:], self.w2_d.ap())
        idm = rp.tile([128, 128], F32)
        nc.sync.dma_start(idm[:], self.id_d.ap())
        return (w2, idm)

    def edge_ops(self, nc, Gs, rs, el, msg):
        g = Gs.shape[1]
        nc.vector.tensor_tensor(out=el[:], in0=Gs[:, :, 65:66], in1=rs[:],
                                op=ALU.mult)
        nc.vector.tensor_tensor(out=el[:], in0=Gs[:, :, 64:65], in1=el[:],
                                op=ALU.max)
        nc.vector.tensor_tensor(
            out=msg[:, :, 0:64], in0=Gs[:, :, 0:64],
            in1=el[:].rearrange("p g o -> p (g o)").to_broadcast([128, g, 64]),
            op=ALU.mult)
        nc.vector.tensor_copy(msg[:, :, 64:65], el[:])

    def tile_ops(self, nc, pools, t, ps, consts):
        rp, gp, mp, wkp, fp, ppA, ppB = pools
        w2, idm = consts
        den = fp.tile([128, 1], F32, tag="den2")
        nc.vector.tensor_scalar_add(den[:], ps[:, 64:65], EPS)
        rec = fp.tile([128, 1], F32, tag="rec2")
        nc.vector.reciprocal(rec[:], den[:])
        agg = fp.tile([128, 64], F32, tag="agg")
        nc.vector.tensor_copy(agg[:], ps[:, 0:64])
        tp = ppB.tile([64, 128], F32, tag="tp")
        nc.tensor.transpose(tp[:], agg[:], idm[:])
        aggT = fp.tile([64, 128], F32, tag="aggT")
        nc.scalar.copy(aggT[:], tp[:])
        op = ppB.tile([128, 16], F32, tag="op")
        nc.tensor.matmul(op[:], lhsT=aggT[:], rhs=w2[:], start=True, stop=True)
        o1 = fp.tile([128, 16], F32, tag="o1")
        nc.vector.tensor_scalar_mul(o1[:], op[:], rec[:])
        mx = fp.tile([128, 1], F32, tag="mx")
        nc.vector.tensor_reduce(out=mx[:], in_=o1[:], axis=AX.X, op=ALU.max)
        nc.vector.tensor_scalar_sub(o1[:], o1[:], mx[:])
        es = fp.tile([128, 16], F16, tag="es")
        ssum = fp.tile([128, 1], F32, tag="ssum")
        nc.scalar.activation(out=es[:], in_=o1[:], func=AF.Exp,
                             accum_out=ssum[:])
        lns = fp.tile([128, 1], F32, tag="lns")
        nc.scalar.activation(out=lns[:], in_=ssum[:], func=AF.Ln)
        res = fp.tile([128, 16], F32, tag="res")
        nc.vector.tensor_scalar_sub(res[:], o1[:], lns[:])
        rows = slice(t * 128, (t + 1) * 128)
        nc.sync.dma_start(self.o_d.ap()[rows, :], res[:])


def _build_launch2(meta):
    nc = _new_nc()
    t1_d = nc.dram_tensor("t1", [NROWS, 128], F16, kind="ExternalInput")
    idx_d = nc.dram_tensor("idx", [128, meta["TOTQ"] // 16], I16,
                           kind="ExternalInput")
    s_d = nc.dram_tensor("sall", [128, meta["sched"]["totw"] * 32], F8,
                         kind="ExternalInput")
    re_d = nc.dram_tensor("re1", [128, meta["TOTG"] * 8], F16,
                          kind="ExternalInput")
    c2_d = nc.dram_tensor("c2", [128, 128], F16, kind="ExternalInput")
    t2_d = nc.dram_tensor("t2s", [MPC, 128], F16, kind="ExternalOutput")
    r2_d = nc.dram_tensor("r2", [MPC, 1], F32, kind="ExternalOutput")
    fin = _L1Final(nc, c2_d, t2_d, r2_d)
    with tile.TileContext(nc) as tc:
        _emit_msg_layer(nc, tc, meta, t1_d, idx_d, s_d, re_d, fin,
                        rwidth=8, mwidth=72)
    nc.compile()
    return nc


def _build_launch3(meta):
    nc = _new_nc()
    t2_d = nc.dram_tensor("t2", [NROWS, 128], F16, kind="ExternalInput")
    idx_d = nc.dram_tensor("idx", [128, meta["TOTQ"] // 16], I16,
                           kind="ExternalInput")
    s_d = nc.dram_tensor("sall", [128, meta["sched"]["totw"] * 32], F8,
                         kind="ExternalInput")
    re_d = nc.dram_tensor("re2", [128, meta["TOTG"] * 1], F16,
                          kind="ExternalInput")
    w2_d = nc.dram_tensor("w2", [64, 16], F32, kind="ExternalInput")
    id_d = nc.dram_tensor("idm", [128, 128], F32, kind="ExternalInput")
    o_d = nc.dram_tensor("o", [MPC, 16], F32, kind="ExternalOutput")
    fin = _L2Final(nc, w2_d, id_d, o_d)
    with tile.TileContext(nc) as tc:
        _emit_msg_layer(nc, tc, meta, t2_d, idx_d, s_d, re_d, fin,
                        rwidth=1, mwidth=65)
    nc.compile()
    return nc


# --------------------------------------------------------------------------
# the kernel
# --------------------------------------------------------------------------

def kernel(x, edge_index, W1, a_src1, a_dst1, b1, W2, a_src2, a_dst2, b2):
    x = np.asarray(x, np.float32)
    edge_index = np.asarray(edge_index)
    W1 = np.asarray(W1, np.float32)
    W2 = np.asarray(W2, np.float32)
    a_src1 = np.asarray(a_src1, np.float32)
    a_dst1 = np.asarray(a_dst1, np.float32)
    a_src2 = np.asarray(a_src2, np.float32)
    a_dst2 = np.asarray(a_dst2, np.float32)

    key = edge_index.tobytes()[:4096]
    if _CACHE.get("key") != key:
        meta = _preprocess(edge_index)
        idx_all, s_all, streams = _build_idx_and_s(meta)
        _CACHE.update(key=key, meta=meta, idx_all=idx_all, s_all=s_all,
                      streams=streams,
                      nc1=_build_launch1(), nc2=_build_launch2(meta),
                      nc3=_build_launch3(meta))
    meta = _CACHE["meta"]
    idx_all, s_all, streams = (_CACHE["idx_all"], _CACHE["s_all"],
                               _CACHE["streams"])

    # weight packing
    W1r = W1.reshape(IN, HEADS, HID)
    B1 = np.einsum("khc,hc->kh", W1r, a_src1)        # [256, 8]
    C1 = np.einsum("khc,hc->kh", W1r, a_dst1)
    wc = np.concatenate([W1, B1, C1], 1).astype(np.float16)   # [256, 80]
    w2a = W2 @ a_src2[0]                              # [64]
    w2d = W2 @ a_dst2[0]
    c2 = np.zeros((128, 128), np.float16)
    c2[:, 0:64] = w2a.astype(np.float16)[None, :]
    c2[:, 64:128] = w2d.astype(np.float16)[None, :]
    idm = np.eye(128, dtype=np.float32)

    # launch 1: build T1 slices
    perm = meta["perm_nodes"]
    xT = np.zeros((IN, NROWS), np.float16)
    real = perm >= 0
    xT[:, real] = x[perm[real]].astype(np.float16).T
    in1 = [{"xs": np.ascontiguousarray(xT[:, c * MPC:(c + 1) * MPC]),
            "wc": wc} for c in range(NCORES)]
    r1_res = bass_utils.run_bass_kernel_spmd(
        _CACHE["nc1"], in1, core_ids=list(range(NCORES)), trace=TRACE)
    T1 = np.concatenate([r1_res.results[c]["t1s"] for c in range(NCORES)], 0)

    # launch 2: layer-1 message passing
    in2 = []
    for c in range(NCORES):
        re1 = _expand_stream(streams[c], r1_res.results[c]["r1"], 8, meta["TOTG"])
        in2.append({"t1": T1, "idx": idx_all[c], "sall": s_all[c],
                    "re1": re1.reshape(128, -1), "c2": c2})
    r2_res = bass_utils.run_bass_kernel_spmd(
        _CACHE["nc2"], in2, core_ids=list(range(NCORES)), trace=TRACE)
    T2 = np.concatenate([r2_res.results[c]["t2s"] for c in range(NCORES)], 0)

    # launch 3: layer-2 + head
    in3 = []
    for c in range(NCORES):
        re2 = _expand_stream(streams[c], r2_res.results[c]["r2"], 1, meta["TOTG"])
        in3.append({"t2": T2, "idx": idx_all[c], "sall": s_all[c],
                    "re2": re2.reshape(128, -1), "w2": W2.astype(np.float32),
                    "idm": idm})
    r3_res = bass_utils.run_bass_kernel_spmd(
        _CACHE["nc3"], in3, core_ids=list(range(NCORES)), trace=TRACE)
    o_all = np.concatenate([r3_res.results[c]["o"] for c in range(NCORES)], 0)

    out = o_all[meta["pos"][np.arange(N)]].astype(np.float32)
    _CACHE["exec_ns"] = [r.exec_time_ns for r in (r1_res, r2_res, r3_res)]
    _CACHE["profiles"] = [r.profile_json for r in (r1_res, r2_res, r3_res)]
    _CACHE["traces"] = [r.instructions_and_trace
                        for r in (r1_res, r2_res, r3_res)]
    return out


def predict_ns():
    """Cost-model (TimelineSim) per-launch predictions for cached programs."""
    from concourse.timeline_sim import TimelineSim
    out = []
    for k in ("nc1", "nc2", "nc3"):
        out.append(TimelineSim(_CACHE[k]).simulate())
    return out



# revision 5
# speedup vs baseline: 1.5387x; 1.5387x over previous
"""2-layer GAT on 8 trn2 NeuronCores (Bass/Tile).

Node-partitioned (12500/core, padded 12544), edges assigned by destination,
per-edge dma_gather of source-node table rows, segment softmax via the
factorization  exp(leaky(s+a)) = A * max(exp(s), exp(0.2 s) * exp(-0.8 a))
(per-dst factor A cancels), segment sums via banded one-hot S matmuls on
the PE.  Three SPMD launches with host halo exchange between them:

  1. "build":  h1 = x @ W1 + attention projections -> per-node table T1
     rows of 96B: [h fp8e4 x64 | exp(s) fp16 x8 | exp(0.2 s) fp16 x8],
     256B row stride in DRAM; r1 = exp(-0.8 a) per node.
  2. "layer1": per-edge 96B gathers from T1 (cost-model: 8.5 ns/descriptor
     vs 22.8 at 256B), edge softmax, banded S matmuls -> per-node epilogue
     (batched: softmax-normalize, ELU, z = elu @ [W2|w2a|w2d] via PE
     transpose) -> T2 rows of 36B: [z fp16 x16 | exp(s2) | exp(0.2 s2)].
  3. "layer2": 36B gathers from T2, 17-wide messages [el*z | el], banded
     S matmuls, batched log_softmax epilogue (single Ln table load).

Folding W2 into the T2 table (z instead of the 64-wide hidden vector) cuts
layer-2 gather/message/matmul width 4x and removes the output-head matmul.
"""

import numpy as np
import ml_dtypes

import concourse.bacc as bacc
import concourse.tile as tile
import concourse.mybir as mybir
from concourse import bass_utils
from concourse.bass import ap_utils, exact_div, MemorySpace

F32 = mybir.dt.float32
F16 = mybir.dt.float16
F8 = mybir.dt.float8e4
I16 = mybir.dt.int16
AF = mybir.ActivationFunctionType
ALU = mybir.AluOpType
AX = mybir.AxisListType

# problem constants (hardcoded per the task statement)
NCORES = 8
N = 100000
IN = 256
HID = 8
HEADS = 8
OUT = 16
NEG = 0.2
NPC = 12500            # real nodes per core
MPC = 12544            # padded nodes per core (98 * 128)
NT = MPC // 128        # 98 dst tiles per core
BATCH_EDGES = 16384    # shared edge budget per batch
NROWS = NCORES * MPC   # 100352 table rows
WINR = 32512           # gather window rows (int16-safe)
NWIN = (NROWS + WINR - 1) // WINR  # 4
EPS = 1e-16
SLAB1 = 7              # launch-1 chunks per slab (must divide NT)
TB1 = 96               # T1 gathered bytes: 64 fp8 h + 16 fp16 exps
TW2 = 18               # T2 row width in fp16: 16 z + 2 exps

_CACHE = {}
TRACE = False
GSPLIT = 48            # max slabs (x128 idxs) per dma_gather call


# --------------------------------------------------------------------------
# raw gather: InstDMAGatherAnt without the elem%256B assert (the non-
# transpose ucode path supports any elem size; only the row STRIDE must be
# a multiple of 256B)
# --------------------------------------------------------------------------

def _dma_gather_raw(ns, out_ap, in_ap, idxs_ap, num_idxs, elem_size,
                    elem_step, queue_num=0):
    assert idxs_ap.dtype == mybir.dt.int16
    assert in_ap.dtype == out_ap.dtype
    assert in_ap.space == MemorySpace.DRAM
    assert ap_utils.ap_is_contiguous(in_ap.ap[1:])
    assert ap_utils.ap_is_contiguous(out_ap.ap[1:])
    assert ap_utils.ap_is_contiguous(idxs_ap.ap[1:])
    assert in_ap.ap[-1][1] == out_ap.ap[-1][1] == elem_size
    assert out_ap.ap[0][1] * out_ap.ap[1][1] == (num_idxs + 127) // 128 * 128
    assert in_ap.ap[0][0] == elem_step
    stride_bytes_256 = exact_div(elem_step * mybir.dt.size(in_ap.dtype), 256)
    assert 0 < stride_bytes_256 < 256
    _in_ap = ns.lower_ap_dma(in_ap, for_custom_bir_dma=True)
    return ns.add_instruction(
        mybir.InstDMAGatherAnt(
            name=ns.bass.get_next_instruction_name(),
            ins=[*_in_ap, ns.lower_ap(idxs_ap),
                 ns.lower_val_access(ns.to_reg(num_idxs))],
            outs=[ns.lower_ap(out_ap)],
            transpose=False, num_idxs=num_idxs, elem_size=elem_size,
            stride_bytes_256=stride_bytes_256, gen_mode=0,
            single_packet=False, queue_num=queue_num,
            sbuf_tokens_per_rank=0, sbuf_free_dim_per_rank=0,
            sbuf_free_dim_pad_per_rank=0, sbuf_byte_offset=0))


# --------------------------------------------------------------------------
# host-side graph preprocessing (pure index work, unchanged from baseline)
# --------------------------------------------------------------------------

def _preprocess(edge_index):
    src = np.concatenate([np.asarray(edge_index[0]), np.arange(N)])
    dst = np.concatenate([np.asarray(edge_index[1]), np.arange(N)])
    deg = np.bincount(dst, minlength=N)

    # permutation: per core, nodes sorted by degree desc
    pos = np.empty(N, np.int64)
    perm_nodes = np.empty(NROWS, np.int64)   # table row -> node id (or -1)
    perm_nodes.fill(-1)
    for c in range(NCORES):
        ids = np.arange(c * NPC, (c + 1) * NPC)
        order = np.argsort(-deg[ids], kind="stable")
        pos[ids[order]] = c * MPC + np.arange(NPC)
        perm_nodes[c * MPC:c * MPC + NPC] = ids[order]

    srcpos = pos[src]
    dstpos = pos[dst]

    cores = []
    counts = np.zeros((NCORES, NT, NWIN), np.int64)
    per_core = []
    for c in range(NCORES):
        m = (dst >= c * NPC) & (dst < (c + 1) * NPC)
        sp = srcpos[m]
        rank = dstpos[m] - c * MPC
        t = rank // 128
        w = sp // WINR
        per_core.append((sp, rank, t, w))
        np.add.at(counts[c], (t, w), 1)
    stc = counts.max(0)                              # [NT, NWIN]
    tile_load = stc.sum(1)
    bmap = np.zeros(NT, np.int64)
    acc = 0
    b = 0
    for t in range(NT):
        if acc and acc + tile_load[t] > BATCH_EDGES:
            b += 1
            acc = 0
        bmap[t] = b
        acc += tile_load[t]
    NBAT = int(bmap[-1]) + 1
    btiles = [list(np.where(bmap == bb)[0]) for bb in range(NBAT)]
    toff = np.zeros((NT, NWIN), np.int64)
    gsz = np.zeros((NBAT, NWIN), np.int64)
    for bb in range(NBAT):
        for w in range(NWIN):
            off = 0
            for t in btiles[bb]:
                toff[t, w] = off
                off += stc[t, w]
            gsz[bb, w] = off
    G = np.maximum((gsz + 127) // 128, 1)            # [NBAT, NWIN] slabs
    Q = G * 128
    qoff = np.zeros((NBAT, NWIN), np.int64)
    goff = np.zeros((NBAT, NWIN), np.int64)
    acc_q = 0
    for bb in range(NBAT):
        for w in range(NWIN):
            qoff[bb, w] = acc_q
            goff[bb, w] = acc_q // 128
            acc_q += Q[bb, w]
    TOTQ = acc_q
    TOTG = TOTQ // 128

    for c in range(NCORES):
        sp, rank, t, w = per_core[c]
        b = bmap[t]
        order = np.lexsort((rank, w, t))
        sp, rank, t, w, b = (sp[order], rank[order], t[order], w[order],
                             b[order])
        gid = t * NWIN + w
        gstart = np.searchsorted(gid, np.arange(NT * NWIN), side="left")
        within = np.arange(len(gid)) - gstart[gid]
        q = qoff[b, w] + toff[t, w] + within
        cores.append({"sp": sp, "rank": rank, "b": b, "w": w, "q": q})

    # union matmul schedule, merged per (b, t, w, j) with a band range.
    JMAX = TOTQ // 128 + 1
    keysets = []
    for c in range(NCORES):
        d = cores[c]
        j = (d["q"] - qoff[d["b"], d["w"]]) // 128
        t = d["rank"] // 128
        a = (d["rank"] % 128) // 32
        key = (t * NWIN + d["w"]) * JMAX + j
        keysets.append((key, a))
        d["j"] = j
        d["t"] = t
        d["key"] = key
    allk = np.concatenate([k for k, _ in keysets])
    alla = np.concatenate([a for _, a in keysets])
    ukeys, inv = np.unique(allk, return_inverse=True)
    TOTB = len(ukeys)
    amin = np.full(TOTB, 4, np.int64)
    amax = np.full(TOTB, -1, np.int64)
    np.minimum.at(amin, inv, alla)
    np.maximum.at(amax, inv, alla)
    ecol = np.where(amin == amax, amin,
                    np.where((amin == 0) & (amax == 1), 0,
                             np.where((amin == 2) & (amax == 3), 2, 0)))
    ewid = np.where(amin == amax, 1,
                    np.where((amin == 0) & (amax == 1), 2,
                             np.where((amin == 2) & (amax == 3), 2, 4)))
    soff = np.concatenate([[0], np.cumsum(ewid)])   # block col offsets (32u)
    uj = ukeys % JMAX
    r1 = ukeys // JMAX
    uw = r1 % NWIN
    ut = r1 // NWIN
    ub = bmap[ut]
    sched = {"b": ub, "t": ut, "w": uw, "j": uj, "col": ecol, "wid": ewid,
             "soff": soff, "n": TOTB, "totw": int(soff[-1])}

    for c in range(NCORES):
        d = cores[c]
        ent = np.searchsorted(ukeys, d["key"])
        d["ent"] = ent
        d["k"] = d["q"] % 128
        d["scol"] = d["rank"] % 128 - ecol[ent] * 32

    meta = {"G": G, "Q": Q, "qoff": qoff, "goff": goff, "TOTQ": TOTQ,
            "TOTG": TOTG, "sched": sched, "pos": pos, "NBAT": NBAT,
            "btiles": btiles, "perm_nodes": perm_nodes, "cores": cores}
    return meta


def _build_idx_and_s(meta):
    """Per-core gather index arrays (int16 wrapped) and fp8 S blocks."""
    TOTQ = meta["TOTQ"]
    idx_all, s_all, streams = [], [], []
    for c in range(NCORES):
        d = meta["cores"][c]
        flat = np.zeros(TOTQ, np.int16)
        loc = d["sp"] - d["w"] * WINR
        flat[d["q"]] = loc.astype(np.int16)
        resh = flat.reshape(TOTQ // 16, 16).T          # [16, TOTQ/16]
        idxw = np.tile(resh, (8, 1)).copy()            # [128, TOTQ/16]
        idx_all.append(idxw)

        soff = meta["sched"]["soff"]
        totw = meta["sched"]["totw"]
        S = np.zeros((128, totw * 32), ml_dtypes.float8_e4m3)
        S[d["k"], soff[d["ent"]] * 32 + d["scol"]] = 1.0
        s_all.append(S)

        streams.append((d["q"] % 128, d["q"] // 128, d["rank"]))
    return idx_all, s_all, streams


def _expand_stream(stream, r_core, width, totg):
    """r_core [MPC, width] f32 -> per-position [128, totg, width] f16."""
    p, g, rank = stream
    out = np.zeros((128, int(totg), width), np.float16)
    out[p, g, :] = r_core[rank, :width].astype(np.float16)
    return out


# --------------------------------------------------------------------------
# launch builders
# --------------------------------------------------------------------------

def _new_nc():
    return bacc.Bacc("TRN2", target_bir_lowering=False, debug=False,
                     enable_asserts=False, num_devices=NCORES)


def _build_launch1():
    nc = _new_nc()
    xs_d = nc.dram_tensor("xs", [IN, MPC], F16, kind="ExternalInput")
    wc_d = nc.dram_tensor("wc", [IN, 80], F16, kind="ExternalInput")
    t1_d = nc.dram_tensor("t1s", [MPC, TB1], F8, kind="ExternalOutput")
    r1_d = nc.dram_tensor("r1", [MPC, 8], F32, kind="ExternalOutput")
    SLAB = SLAB1
    with tile.TileContext(nc) as tc:
        with tc.tile_pool(name="w", bufs=1) as wp, \
             tc.tile_pool(name="x", bufs=3) as xp, \
             tc.tile_pool(name="o", bufs=3) as op, \
             tc.tile_pool(name="ps", bufs=4, space="PSUM") as pp:
            wc_sb = wp.tile([128, 2, 80], F16)
            nc.gpsimd.dma_start(wc_sb[:, 0, :], wc_d.ap()[0:128, :])
            nc.gpsimd.dma_start(wc_sb[:, 1, :], wc_d.ap()[128:256, :])
            for s in range(NT // SLAB):
                cols = slice(s * SLAB * 128, (s + 1) * SLAB * 128)
                xt0 = xp.tile([128, SLAB * 128], F16, tag="xt0")
                xt1 = xp.tile([128, SLAB * 128], F16, tag="xt1")
                nc.gpsimd.dma_start(xt0[:], xs_d.ap()[0:128, cols])
                nc.gpsimd.dma_start(xt1[:], xs_d.ap()[128:256, cols])
                tout = op.tile([128, SLAB, TB1], F8, tag="tout")
                ex = op.tile([128, SLAB, 16], F32, tag="ex")
                rout = op.tile([128, SLAB, 8], F32, tag="rout")
                for i in range(SLAB):
                    ps = pp.tile([128, 80], F32)
                    nc.tensor.matmul(ps[:], lhsT=xt0[:, i * 128:(i + 1) * 128],
                                     rhs=wc_sb[:, 0, :], start=True, stop=False)
                    nc.tensor.matmul(ps[:], lhsT=xt1[:, i * 128:(i + 1) * 128],
                                     rhs=wc_sb[:, 1, :], start=False, stop=True)
                    nc.vector.tensor_copy(tout[:, i, 0:64], ps[:, 0:64])
                    nc.vector.tensor_copy(ex[:, i, :], ps[:, 64:80])
                tv = tout[:, :, 64:96].bitcast(F16)      # [128, SLAB, 16]
                nc.scalar.activation(out=tv[:, :, 0:8], in_=ex[:, :, 0:8],
                                     func=AF.Exp)
                nc.scalar.activation(out=tv[:, :, 8:16], in_=ex[:, :, 0:8],
                                     func=AF.Exp, scale=0.2)
                nc.scalar.activation(out=rout[:], in_=ex[:, :, 8:16],
                                     func=AF.Exp, scale=-0.8)
                rows = slice(s * SLAB * 128, (s + 1) * SLAB * 128)
                nc.gpsimd.dma_start(
                    t1_d.ap()[rows, :].rearrange("(i p) f -> p i f", p=128),
                    tout[:])
                nc.gpsimd.dma_start(
                    r1_d.ap()[rows, :].rearrange("(i p) f -> p i f", p=128),
                    rout[:])
    nc.compile()
    return nc


def _batch_geometry(meta):
    G, qoff = meta["G"], meta["qoff"]
    sched = meta["sched"]
    soff = sched["soff"]
    NBAT = meta["NBAT"]
    sb = sched["b"]
    blo = np.searchsorted(sb, np.arange(NBAT))
    bhi = np.searchsorted(sb, np.arange(NBAT), side="right")
    slo = [int(soff[blo[b]]) for b in range(NBAT)]
    shi = [int(soff[bhi[b]]) for b in range(NBAT)]
    nw32max = max(1, max(shi[b] - slo[b] for b in range(NBAT)))
    qb_lo = [int(qoff[b, 0]) for b in range(NBAT)]
    qb_hi = [int(qoff[b, NWIN - 1] + G[b, NWIN - 1] * 128)
             for b in range(NBAT)]
    qbmax = max(qb_hi[b] - qb_lo[b] for b in range(NBAT))
    ent_by_t = {}
    for i in range(sched["n"]):
        ent_by_t.setdefault(int(sched["t"][i]), []).append(i)
    return blo, bhi, slo, shi, nw32max, qb_lo, qb_hi, qbmax, ent_by_t


def _emit_batches(nc, meta, pools, tab_ap, idx_d, s_d, re_d, elem, estep,
                  rwidth, mwidth, edge_ops, tile_out):
    """Shared batch loop: gathers, edge ops, banded S matmuls.

    edge_ops(Gs, rs, el, msg) fills msg [128, g, mwidth];
    tile_out(t, ps) consumes the per-tile PSUM accumulator."""
    G, qoff, goff = meta["G"], meta["qoff"], meta["goff"]
    sched = meta["sched"]
    sw, sj = sched["w"], sched["j"]
    scol, swid, soff = sched["col"], sched["wid"], sched["soff"]
    NBAT = meta["NBAT"]
    btiles = meta["btiles"]
    blo, bhi, slo, shi, nw32max, qb_lo, qb_hi, qbmax, ent_by_t = \
        _batch_geometry(meta)
    mp, gp, wkp, ppA, zrow = pools

    for b in range(NBAT):
        nw32 = max(shi[b] - slo[b], 1)
        ssb = mp.tile([128, nw32max, 32], F8, tag="s", bufs=2)
        if shi[b] > slo[b]:
            nc.gpsimd.dma_start(
                ssb[:, 0:nw32, :],
                s_d.ap()[:, slo[b] * 32:shi[b] * 32]
                .rearrange("p (n c) -> p n c", c=32))
        nq = qb_hi[b] - qb_lo[b]
        idx_sb = mp.tile([128, qbmax // 16], I16, tag="idx", bufs=2)
        nc.gpsimd.dma_start(idx_sb[:, 0:nq // 16],
                            idx_d.ap()[:, qb_lo[b] // 16:qb_hi[b] // 16])
        slabs = {}
        for w in range(NWIN):
            g = int(G[b, w])
            q0 = int(qoff[b, w]) - qb_lo[b]
            g0 = int(goff[b, w])
            Gs = gp.tile([128, g, elem], tab_ap.dtype, tag="G", bufs=4)
            win0 = w * WINR
            win1 = min(win0 + WINR, NROWS)
            for g1 in range(0, g, GSPLIT):
                g2 = min(g1 + GSPLIT, g)
                nn = (g2 - g1) * 128
                _dma_gather_raw(
                    nc.gpsimd, Gs[:, g1:g2, :],
                    tab_ap[win0:win1, 0:elem],
                    idx_sb[:, (q0 + g1 * 128) // 16:(q0 + g2 * 128) // 16],
                    nn, elem, estep)
            rs = gp.tile([128, g, rwidth], F16, tag="rs", bufs=3)
            nc.gpsimd.dma_start(
                rs[:], re_d.ap()[:, g0 * rwidth:(g0 + g) * rwidth]
                .rearrange("p (g r) -> p g r", r=rwidth))
            msg = wkp.tile([128, g, mwidth], F16, tag="msg", bufs=6)
            el = wkp.tile([128, g, rwidth], F16, tag="el", bufs=3)
            edge_ops(Gs, rs, el, msg)
            slabs[w] = msg
        for t in btiles[b]:
            ents = ent_by_t.get(t, [])
            ps = ppA.tile([128, mwidth], F32, tag="ps")
            nc.tensor.matmul(ps[:], lhsT=zrow[:], rhs=zrow[:, 0:mwidth],
                             start=True, stop=False, skip_group_check=True)
            for n, i in enumerate(ents):
                w, j = int(sw[i]), int(sj[i])
                col, wid = int(scol[i]), int(swid[i])
                so = int(soff[i]) - slo[b]
                nc.tensor.matmul(
                    ps[col * 32:(col + wid) * 32, :],
                    lhsT=ssb[:, so:so + wid, :]
                    .rearrange("p n c -> p (n c)"),
                    rhs=slabs[w][:, j, :],
                    start=False, stop=(n == len(ents) - 1),
                    tile_position=(0, col * 32),
                    skip_group_check=True)
            tile_out(t, ps)


def _build_launch2(meta):
    nc = _new_nc()
    t1_d = nc.dram_tensor("t1", [NROWS, 256], F8, kind="ExternalInput")
    idx_d = nc.dram_tensor("idx", [128, meta["TOTQ"] // 16], I16,
                           kind="ExternalInput")
    s_d = nc.dram_tensor("sall", [128, meta["sched"]["totw"] * 32], F8,
                         kind="ExternalInput")
    re_d = nc.dram_tensor("re1", [128, meta["TOTG"] * 8], F16,
                          kind="ExternalInput")
    w2_d = nc.dram_tensor("w2e", [64, 18], F16, kind="ExternalInput")
    id_d = nc.dram_tensor("idm", [128, 128], F16, kind="ExternalInput")
    t2_d = nc.dram_tensor("t2s", [MPC, TW2], F16, kind="ExternalOutput")
    r2_d = nc.dram_tensor("r2", [MPC, 1], F32, kind="ExternalOutput")

    with tile.TileContext(nc) as tc:
        with tc.tile_pool(name="res", bufs=1) as rp, \
             tc.tile_pool(name="m", bufs=1) as mp, \
             tc.tile_pool(name="g", bufs=1) as gp, \
             tc.tile_pool(name="wk", bufs=1) as wkp, \
             tc.tile_pool(name="ep", bufs=1) as ep, \
             tc.tile_pool(name="zi", bufs=3) as zp, \
             tc.tile_pool(name="psA", bufs=4, space="PSUM") as ppA, \
             tc.tile_pool(name="psB", bufs=2, space="PSUM") as ppB:
            zrow = rp.tile([1, 128], F16)
            nc.vector.memset(zrow[:], 0.0)
            w2_sb = rp.tile([64, 18], F16)
            nc.gpsimd.dma_start(w2_sb[:], w2_d.ap())
            idm = rp.tile([128, 128], F16)
            nc.gpsimd.dma_start(idm[:], id_d.ap())
            ybuf = rp.tile([128, NT, 72], F16)
            t2t = rp.tile([128, NT, TW2], F16)
            pbuf = rp.tile([128, NT, 2], F32)
            r2sb = rp.tile([128, NT], F32)

            def edge_ops(Gs, rs, el, msg):
                g = Gs.shape[1]
                es_v = Gs[:, :, 64:80].bitcast(F16)
                e02_v = Gs[:, :, 80:96].bitcast(F16)
                nc.vector.tensor_tensor(out=el[:], in0=e02_v, in1=rs[:],
                                        op=ALU.mult)
                nc.vector.tensor_tensor(out=el[:], in0=es_v, in1=el[:],
                                        op=ALU.max)
                nc.vector.tensor_tensor(
                    out=msg[:, :, 0:64].rearrange("p g (h c) -> p g h c", h=8),
                    in0=Gs[:, :, 0:64].rearrange("p g (h c) -> p g h c", h=8),
                    in1=el[:].to_broadcast([128, g, 8, 8]), op=ALU.mult)
                nc.vector.tensor_copy(msg[:, :, 64:72], el[:])

            def tile_out(t, ps):
                nc.scalar.copy(ybuf[:, t, :], ps[:])

            _emit_batches(nc, meta, (mp, gp, wkp, ppA, zrow), t1_d.ap(),
                          idx_d, s_d, re_d, TB1, 256, 8, 72,
                          edge_ops, tile_out)

            # ---- batched epilogue ----
            rec = ep.tile([128, NT, 8], F32)
            nc.vector.tensor_scalar_add(rec[:], ybuf[:, :, 64:72], EPS)
            nc.vector.reciprocal(rec[:], rec[:])
            y16 = ep.tile([128, NT, 64], F16)
            nc.vector.tensor_tensor(
                out=y16[:].rearrange("p t (h c) -> p t h c", h=8),
                in0=ybuf[:, :, 0:64].rearrange("p t (h c) -> p t h c", h=8),
                in1=rec[:].to_broadcast([128, NT, 8, 8]), op=ALU.mult)
            yn = ep.tile([128, NT, 64], F16)
            nc.vector.tensor_scalar_min(yn[:], y16[:], 0.0)
            ey = ep.tile([128, NT, 64], F16)
            nc.scalar.activation(out=ey[:], in_=yn[:], func=AF.Exp)
            nc.vector.tensor_scalar_add(ey[:], ey[:], -1.0)
            elu = ep.tile([128, NT, 64], F16)
            nc.vector.tensor_tensor(out=elu[:], in0=y16[:], in1=ey[:],
                                    op=ALU.max)
            # z = elu @ [W2 | w2a | w2d] per tile via PE transpose
            for t0 in range(0, NT, 8):
                nz = min(8, NT - t0)
                zacc = ppB.tile([128, 8, 32], F32, tag="zacc", bufs=2)
                for k in range(nz):
                    t = t0 + k
                    tp = ppB.tile([64, 128], F16, tag="tp", bufs=2)
                    nc.tensor.transpose(tp[:], elu[:, t, :], idm[:])
                    zin = zp.tile([64, 128], F16, tag="zin")
                    nc.scalar.copy(zin[:], tp[:])
                    nc.tensor.matmul(zacc[:, k, 0:18], lhsT=zin[:],
                                     rhs=w2_sb[:], start=True, stop=True)
                nc.vector.tensor_copy(t2t[:, t0:t0 + nz, 0:16],
                                      zacc[:, 0:nz, 0:16])
                nc.vector.tensor_copy(pbuf[:, t0:t0 + nz, :],
                                      zacc[:, 0:nz, 16:18])
            nc.scalar.activation(out=t2t[:, :, 16:17], in_=pbuf[:, :, 0:1],
                                 func=AF.Exp)
            nc.scalar.activation(out=t2t[:, :, 17:18], in_=pbuf[:, :, 0:1],
                                 func=AF.Exp, scale=0.2)
            nc.scalar.activation(out=r2sb[:], in_=pbuf[:, :, 1:2],
                                 func=AF.Exp, scale=-0.8)
            nc.gpsimd.dma_start(
                t2_d.ap().rearrange("(t p) f -> p t f", p=128), t2t[:])
            nc.gpsimd.dma_start(
                r2_d.ap().rearrange("(t p) o -> p (t o)", p=128), r2sb[:])
    nc.compile()
    return nc


def _build_launch3(meta):
    nc = _new_nc()
    t2_d = nc.dram_tensor("t2", [NROWS, 128], F16, kind="ExternalInput")
    idx_d = nc.dram_tensor("idx", [128, meta["TOTQ"] // 16], I16,
                           kind="ExternalInput")
    s_d = nc.dram_tensor("sall", [128, meta["sched"]["totw"] * 32], F8,
                         kind="ExternalInput")
    re_d = nc.dram_tensor("re2", [128, meta["TOTG"]], F16,
                          kind="ExternalInput")
    o_d = nc.dram_tensor("o", [MPC, 16], F32, kind="ExternalOutput")

    with tile.TileContext(nc) as tc:
        with tc.tile_pool(name="res", bufs=1) as rp, \
             tc.tile_pool(name="m", bufs=1) as mp, \
             tc.tile_pool(name="g", bufs=1) as gp, \
             tc.tile_pool(name="wk", bufs=1) as wkp, \
             tc.tile_pool(name="ep", bufs=1) as ep, \
             tc.tile_pool(name="psA", bufs=4, space="PSUM") as ppA:
            zrow = rp.tile([1, 128], F16)
            nc.vector.memset(zrow[:], 0.0)
            obuf = rp.tile([128, NT, 17], F32)

            def edge_ops(Gs, rs, el, msg):
                g = Gs.shape[1]
                nc.vector.tensor_tensor(out=el[:], in0=Gs[:, :, 17:18],
                                        in1=rs[:], op=ALU.mult)
                nc.vector.tensor_tensor(out=el[:], in0=Gs[:, :, 16:17],
                                        in1=el[:], op=ALU.max)
                nc.vector.tensor_tensor(
                    out=msg[:, :, 0:16], in0=Gs[:, :, 0:16],
                    in1=el[:].rearrange("p g o -> p (g o)")
                    .to_broadcast([128, g, 16]), op=ALU.mult)
                nc.vector.tensor_copy(msg[:, :, 16:17], el[:])

            def tile_out(t, ps):
                nc.scalar.copy(obuf[:, t, :], ps[:])

            _emit_batches(nc, meta, (mp, gp, wkp, ppA, zrow), t2_d.ap(),
                          idx_d, s_d, re_d, TW2, 128, 1, 17,
                          edge_ops, tile_out)

            # ---- batched log_softmax epilogue ----
            rec = ep.tile([128, NT, 1], F32)
            nc.vector.tensor_scalar_add(rec[:], obuf[:, :, 16:17], EPS)
            nc.vector.reciprocal(rec[:], rec[:])
            o1 = ep.tile([128, NT, 16], F32)
            nc.vector.tensor_tensor(
                out=o1[:], in0=obuf[:, :, 0:16],
                in1=rec[:].rearrange("p t o -> p (t o)")
                .to_broadcast([128, NT, 16]), op=ALU.mult)
            mx = ep.tile([128, NT, 1], F32)
            nc.vector.tensor_reduce(out=mx[:], in_=o1[:], axis=AX.X,
                                    op=ALU.max)
            nc.vector.tensor_tensor(
                out=o1[:], in0=o1[:],
                in1=mx[:].rearrange("p t o -> p (t o)")
                .to_broadcast([128, NT, 16]), op=ALU.subtract)
            es = ep.tile([128, NT, 16], F16)
            nc.scalar.activation(out=es[:], in_=o1[:], func=AF.Exp)
            ssum = ep.tile([128, NT, 1], F32)
            nc.vector.tensor_reduce(out=ssum[:], in_=es[:], axis=AX.X,
                                    op=ALU.add)
            lns = ep.tile([128, NT, 1], F32)
            nc.scalar.activation(out=lns[:], in_=ssum[:], func=AF.Ln)
            nc.vector.tensor_tensor(
                out=o1[:], in0=o1[:],
                in1=lns[:].rearrange("p t o -> p (t o)")
                .to_broadcast([128, NT, 16]), op=ALU.subtract)
            nc.gpsimd.dma_start(
                o_d.ap().rearrange("(t p) f -> p t f", p=128), o1[:])
    nc.compile()
    return nc


# --------------------------------------------------------------------------
# the kernel
# --------------------------------------------------------------------------

def kernel(x, edge_index, W1, a_src1, a_dst1, b1, W2, a_src2, a_dst2, b2):
    x = np.asarray(x, np.float32)
    edge_index = np.asarray(edge_index)
    W1 = np.asarray(W1, np.float32)
    W2 = np.asarray(W2, np.float32)
    a_src1 = np.asarray(a_src1, np.float32)
    a_dst1 = np.asarray(a_dst1, np.float32)
    a_src2 = np.asarray(a_src2, np.float32)
    a_dst2 = np.asarray(a_dst2, np.float32)

    key = edge_index.tobytes()[:4096]
    if _CACHE.get("key") != key:
        meta = _preprocess(edge_index)
        idx_all, s_all, streams = _build_idx_and_s(meta)
        _CACHE.update(key=key, meta=meta, idx_all=idx_all, s_all=s_all,
                      streams=streams,
                      nc1=_build_launch1(), nc2=_build_launch2(meta),
                      nc3=_build_launch3(meta))
    meta = _CACHE["meta"]
    idx_all, s_all, streams = (_CACHE["idx_all"], _CACHE["s_all"],
                               _CACHE["streams"])

    # weight packing
    W1r = W1.reshape(IN, HEADS, HID)
    B1 = np.einsum("khc,hc->kh", W1r, a_src1)        # [256, 8]
    C1 = np.einsum("khc,hc->kh", W1r, a_dst1)
    wc = np.concatenate([W1, B1, C1], 1).astype(np.float16)   # [256, 80]
    w2a = W2 @ a_src2[0]                              # [64]
    w2d = W2 @ a_dst2[0]
    w2e = np.concatenate([W2, w2a[:, None], w2d[:, None]],
                         1).astype(np.float16)        # [64, 18]
    idm = np.eye(128, dtype=np.float16)

    # launch 1: build T1 slices
    perm = meta["perm_nodes"]
    xT = np.zeros((IN, NROWS), np.float16)
    real = perm >= 0
    xT[:, real] = x[perm[real]].astype(np.float16).T
    in1 = [{"xs": np.ascontiguousarray(xT[:, c * MPC:(c + 1) * MPC]),
            "wc": wc} for c in range(NCORES)]
    r1_res = bass_utils.run_bass_kernel_spmd(
        _CACHE["nc1"], in1, core_ids=list(range(NCORES)), trace=TRACE)
    T1 = np.zeros((NROWS, 256), np.uint8)
    for c in range(NCORES):
        T1[c * MPC:(c + 1) * MPC, 0:TB1] = \
            np.asarray(r1_res.results[c]["t1s"]).view(np.uint8)
    T1 = T1.view(ml_dtypes.float8_e4m3)

    # launch 2: layer-1 message passing -> T2 slices
    in2 = []
    for c in range(NCORES):
        re1 = _expand_stream(streams[c], np.asarray(r1_res.results[c]["r1"]),
                             8, meta["TOTG"])
        in2.append({"t1": T1, "idx": idx_all[c], "sall": s_all[c],
                    "re1": re1.reshape(128, -1), "w2e": w2e, "idm": idm})
    r2_res = bass_utils.run_bass_kernel_spmd(
        _CACHE["nc2"], in2, core_ids=list(range(NCORES)), trace=TRACE)
    T2 = np.zeros((NROWS, 128), np.float16)
    for c in range(NCORES):
        T2[c * MPC:(c + 1) * MPC, 0:TW2] = \
            np.asarray(r2_res.results[c]["t2s"])

    # launch 3: layer-2 + log_softmax
    in3 = []
    for c in range(NCORES):
        re2 = _expand_stream(streams[c], np.asarray(r2_res.results[c]["r2"]),
                             1, meta["TOTG"])
        in3.append({"t2": T2, "idx": idx_all[c], "sall": s_all[c],
                    "re2": re2.reshape(128, -1)})
    r3_res = bass_utils.run_bass_kernel_spmd(
        _CACHE["nc3"], in3, core_ids=list(range(NCORES)), trace=TRACE)
    o_all = np.concatenate([np.asarray(r3_res.results[c]["o"])
                            for c in range(NCORES)], 0)

    out = o_all[meta["pos"][np.arange(N)]].astype(np.float32)
    _CACHE["exec_ns"] = [r.exec_time_ns for r in (r1_res, r2_res, r3_res)]
    return out


def predict_ns():
    """Cost-model (TimelineSim) per-launch predictions for cached programs."""
    from concourse.timeline_sim import TimelineSim
    out = []
    for k in ("nc1", "nc2", "nc3"):
        out.append(TimelineSim(_CACHE[k]).simulate())
    return out


# revision 14
# speedup vs baseline: 1.9057x; 1.2385x over previous
"""2-layer GAT on 8 trn2 NeuronCores (Bass/Tile).

Node-partitioned (12500/core, padded 12544), edges assigned by destination,
per-edge dma_gather of source-node table rows, segment softmax via the
factorization  exp(leaky(s+a)) = A * max(exp(s), exp(0.2 s) * exp(-0.8 a))
(per-dst factor A cancels), segment sums via banded one-hot S matmuls on
the PE.  Three SPMD launches with host halo exchange between them:

  1. "build":  h1 = x @ W1 + attention projections -> per-node table T1
     rows of 96B: [h fp8e4 x64 | exp(s) fp16 x8 | exp(0.2 s) fp16 x8],
     256B row stride in DRAM; r1 = exp(-0.8 a) per node.
  2. "layer1": per-edge 96B gathers from T1 (cost-model: 8.5 ns/descriptor
     vs 22.8 at 256B), edge softmax, banded S matmuls -> per-node epilogue
     (batched: softmax-normalize, ELU, z = elu @ [W2|w2a|w2d] via PE
     transpose) -> T2 rows of 36B: [z fp16 x16 | exp(s2) | exp(0.2 s2)].
  3. "layer2": 36B gathers from T2, 17-wide messages [el*z | el], banded
     S matmuls, batched log_softmax epilogue (single Ln table load).

Folding W2 into the T2 table (z instead of the 64-wide hidden vector) cuts
layer-2 gather/message/matmul width 4x and removes the output-head matmul.
"""

import numpy as np
import ml_dtypes

import concourse.bacc as bacc
import concourse.tile as tile
import concourse.mybir as mybir
from concourse import bass_utils
from concourse.bass import ap_utils, exact_div, MemorySpace

F32 = mybir.dt.float32
F16 = mybir.dt.float16
F8 = mybir.dt.float8e4
I16 = mybir.dt.int16
AF = mybir.ActivationFunctionType
ALU = mybir.AluOpType
AX = mybir.AxisListType

# problem constants (hardcoded per the task statement)
NCORES = 8
N = 100000
IN = 256
HID = 8
HEADS = 8
OUT = 16
NEG = 0.2
NPC = 12500            # real nodes per core
MPC = 12544            # padded nodes per core (98 * 128)
NT = MPC // 128        # 98 dst tiles per core
BATCH_EDGES = 24576    # shared edge budget per batch
NROWS = NCORES * MPC   # 100352 table rows
WINR = 32512           # gather window rows (int16-safe)
NWIN = (NROWS + WINR - 1) // WINR  # 4
EPS = 1e-16
SLAB1 = 7              # launch-1 chunks per slab (must divide NT)
TB1 = 96               # T1 gathered bytes: 64 fp8 h + 16 fp16 exps
TW2 = 18               # T2 row width in fp16: 16 z + 2 exps

_CACHE = {}
TRACE = False
GSPLIT = 96            # max slabs (x128 idxs) per dma_gather call
IDXR = 32              # idx tile partition replication (ucode reads <=32)


# --------------------------------------------------------------------------
# raw gather: InstDMAGatherAnt without the elem%256B assert (the non-
# transpose ucode path supports any elem size; only the row STRIDE must be
# a multiple of 256B)
# --------------------------------------------------------------------------

def _dma_gather_raw(ns, out_ap, in_ap, idxs_ap, num_idxs, elem_size,
                    elem_step, queue_num=0):
    assert idxs_ap.dtype == mybir.dt.int16
    assert in_ap.dtype == out_ap.dtype
    assert in_ap.space == MemorySpace.DRAM
    assert ap_utils.ap_is_contiguous(in_ap.ap[1:])
    assert ap_utils.ap_is_contiguous(out_ap.ap[1:])
    assert ap_utils.ap_is_contiguous(idxs_ap.ap[1:])
    assert in_ap.ap[-1][1] == out_ap.ap[-1][1] == elem_size
    assert out_ap.ap[0][1] * out_ap.ap[1][1] == (num_idxs + 127) // 128 * 128
    assert in_ap.ap[0][0] == elem_step
    stride_bytes_256 = exact_div(elem_step * mybir.dt.size(in_ap.dtype), 256)
    assert 0 < stride_bytes_256 < 256
    _in_ap = ns.lower_ap_dma(in_ap, for_custom_bir_dma=True)
    return ns.add_instruction(
        mybir.InstDMAGatherAnt(
            name=ns.bass.get_next_instruction_name(),
            ins=[*_in_ap, ns.lower_ap(idxs_ap),
                 ns.lower_val_access(ns.to_reg(num_idxs))],
            outs=[ns.lower_ap(out_ap)],
            transpose=False, num_idxs=num_idxs, elem_size=elem_size,
            stride_bytes_256=stride_bytes_256, gen_mode=0,
            single_packet=False, queue_num=queue_num,
            sbuf_tokens_per_rank=0, sbuf_free_dim_per_rank=0,
            sbuf_free_dim_pad_per_rank=0, sbuf_byte_offset=0))


# --------------------------------------------------------------------------
# host-side graph preprocessing (pure index work, unchanged from baseline)
# --------------------------------------------------------------------------

def _preprocess(edge_index):
    src = np.concatenate([np.asarray(edge_index[0]), np.arange(N)])
    dst = np.concatenate([np.asarray(edge_index[1]), np.arange(N)])
    deg = np.bincount(dst, minlength=N)

    # permutation: per core, nodes sorted by degree desc
    pos = np.empty(N, np.int64)
    perm_nodes = np.empty(NROWS, np.int64)   # table row -> node id (or -1)
    perm_nodes.fill(-1)
    for c in range(NCORES):
        ids = np.arange(c * NPC, (c + 1) * NPC)
        order = np.argsort(-deg[ids], kind="stable")
        pos[ids[order]] = c * MPC + np.arange(NPC)
        perm_nodes[c * MPC:c * MPC + NPC] = ids[order]

    srcpos = pos[src]
    dstpos = pos[dst]

    cores = []
    counts = np.zeros((NCORES, NT, NWIN), np.int64)
    per_core = []
    for c in range(NCORES):
        m = (dst >= c * NPC) & (dst < (c + 1) * NPC)
        sp = srcpos[m]
        rank = dstpos[m] - c * MPC
        t = rank // 128
        w = sp // WINR
        per_core.append((sp, rank, t, w))
        np.add.at(counts[c], (t, w), 1)
    stc = counts.max(0)                              # [NT, NWIN]
    tile_load = stc.sum(1)
    bmap = np.zeros(NT, np.int64)
    acc = 0
    b = 0
    for t in range(NT):
        if acc and acc + tile_load[t] > BATCH_EDGES:
            b += 1
            acc = 0
        bmap[t] = b
        acc += tile_load[t]
    NBAT = int(bmap[-1]) + 1
    btiles = [list(np.where(bmap == bb)[0]) for bb in range(NBAT)]
    toff = np.zeros((NT, NWIN), np.int64)
    gsz = np.zeros((NBAT, NWIN), np.int64)
    for bb in range(NBAT):
        for w in range(NWIN):
            off = 0
            for t in btiles[bb]:
                toff[t, w] = off
                off += stc[t, w]
            gsz[bb, w] = off
    G = np.maximum((gsz + 127) // 128, 1)            # [NBAT, NWIN] slabs
    Q = G * 128
    qoff = np.zeros((NBAT, NWIN), np.int64)
    goff = np.zeros((NBAT, NWIN), np.int64)
    acc_q = 0
    for bb in range(NBAT):
        for w in range(NWIN):
            qoff[bb, w] = acc_q
            goff[bb, w] = acc_q // 128
            acc_q += Q[bb, w]
    TOTQ = acc_q
    TOTG = TOTQ // 128

    for c in range(NCORES):
        sp, rank, t, w = per_core[c]
        b = bmap[t]
        order = np.lexsort((rank, w, t))
        sp, rank, t, w, b = (sp[order], rank[order], t[order], w[order],
                             b[order])
        gid = t * NWIN + w
        gstart = np.searchsorted(gid, np.arange(NT * NWIN), side="left")
        within = np.arange(len(gid)) - gstart[gid]
        q = qoff[b, w] + toff[t, w] + within
        cores.append({"sp": sp, "rank": rank, "b": b, "w": w, "q": q})

    # union matmul schedule, merged per (b, t, w, j) with a band range.
    JMAX = TOTQ // 128 + 1
    keysets = []
    for c in range(NCORES):
        d = cores[c]
        j = (d["q"] - qoff[d["b"], d["w"]]) // 128
        t = d["rank"] // 128
        a = (d["rank"] % 128) // 32
        key = (t * NWIN + d["w"]) * JMAX + j
        keysets.append((key, a))
        d["j"] = j
        d["t"] = t
        d["key"] = key
    allk = np.concatenate([k for k, _ in keysets])
    alla = np.concatenate([a for _, a in keysets])
    ukeys, inv = np.unique(allk, return_inverse=True)
    TOTB = len(ukeys)
    amin = np.full(TOTB, 4, np.int64)
    amax = np.full(TOTB, -1, np.int64)
    np.minimum.at(amin, inv, alla)
    np.maximum.at(amax, inv, alla)
    ecol = np.where(amin == amax, amin,
                    np.where((amin == 0) & (amax == 1), 0,
                             np.where((amin == 2) & (amax == 3), 2, 0)))
    ewid = np.where(amin == amax, 1,
                    np.where((amin == 0) & (amax == 1), 2,
                             np.where((amin == 2) & (amax == 3), 2, 4)))
    soff = np.concatenate([[0], np.cumsum(ewid)])   # block col offsets (32u)
    uj = ukeys % JMAX
    r1 = ukeys // JMAX
    uw = r1 % NWIN
    ut = r1 // NWIN
    ub = bmap[ut]
    sched = {"b": ub, "t": ut, "w": uw, "j": uj, "col": ecol, "wid": ewid,
             "soff": soff, "n": TOTB, "totw": int(soff[-1])}

    for c in range(NCORES):
        d = cores[c]
        ent = np.searchsorted(ukeys, d["key"])
        d["ent"] = ent
        d["k"] = d["q"] % 128
        d["scol"] = d["rank"] % 128 - ecol[ent] * 32

    meta = {"G": G, "Q": Q, "qoff": qoff, "goff": goff, "TOTQ": TOTQ,
            "TOTG": TOTG, "sched": sched, "pos": pos, "NBAT": NBAT,
            "btiles": btiles, "perm_nodes": perm_nodes, "cores": cores}
    return meta


def _build_idx_and_s(meta):
    """Per-core gather index arrays (int16 wrapped) and fp8 S blocks."""
    TOTQ = meta["TOTQ"]
    idx_all, s_all, streams = [], [], []
    for c in range(NCORES):
        d = meta["cores"][c]
        flat = np.zeros(TOTQ, np.int16)
        loc = d["sp"] - d["w"] * WINR
        flat[d["q"]] = loc.astype(np.int16)
        resh = flat.reshape(TOTQ // 16, 16).T          # [16, TOTQ/16]
        idxw = np.tile(resh, (IDXR // 16, 1)).copy()   # [IDXR, TOTQ/16]
        idx_all.append(idxw)

        soff = meta["sched"]["soff"]
        totw = meta["sched"]["totw"]
        S = np.zeros((128, totw * 32), ml_dtypes.float8_e4m3)
        S[d["k"], soff[d["ent"]] * 32 + d["scol"]] = 1.0
        s_all.append(S)

        streams.append((d["q"] % 128, d["q"] // 128, d["rank"]))
    return idx_all, s_all, streams


def _expand_stream(stream, r_core, width, totg):
    """r_core [MPC, width] f32 -> per-position [128, totg, width] f16."""
    p, g, rank = stream
    out = np.zeros((128, int(totg), width), np.float16)
    out[p, g, :] = r_core[rank, :width].astype(np.float16)
    return out


# --------------------------------------------------------------------------
# launch builders
# --------------------------------------------------------------------------

def _new_nc():
    return bacc.Bacc("TRN2", target_bir_lowering=False, debug=False,
                     enable_asserts=False, num_devices=NCORES)


def _build_launch1():
    nc = _new_nc()
    xs_d = nc.dram_tensor("xs", [IN, MPC], F16, kind="ExternalInput")
    wc_d = nc.dram_tensor("wc", [IN, 80], F16, kind="ExternalInput")
    t1_d = nc.dram_tensor("t1s", [MPC, TB1], F8, kind="ExternalOutput")
    r1_d = nc.dram_tensor("r1", [MPC, 8], F32, kind="ExternalOutput")
    SLAB = SLAB1
    with tile.TileContext(nc) as tc:
        with tc.tile_pool(name="w", bufs=1) as wp, \
             tc.tile_pool(name="x", bufs=3) as xp, \
             tc.tile_pool(name="o", bufs=3) as op, \
             tc.tile_pool(name="ps", bufs=4, space="PSUM") as pp:
            wc_sb = wp.tile([128, 2, 80], F16)
            nc.sync.dma_start(wc_sb[:, 0, :], wc_d.ap()[0:128, :])
            nc.sync.dma_start(wc_sb[:, 1, :], wc_d.ap()[128:256, :])
            for s in range(NT // SLAB):
                cols = slice(s * SLAB * 128, (s + 1) * SLAB * 128)
                xt0 = xp.tile([128, SLAB * 128], F16, tag="xt0")
                xt1 = xp.tile([128, SLAB * 128], F16, tag="xt1")
                nc.sync.dma_start(xt0[:], xs_d.ap()[0:128, cols])
                nc.sync.dma_start(xt1[:], xs_d.ap()[128:256, cols])
                tout = op.tile([128, SLAB, TB1], F8, tag="tout")
                ex = op.tile([128, SLAB, 16], F32, tag="ex")
                rout = op.tile([128, SLAB, 8], F32, tag="rout")
                for i in range(SLAB):
                    ps = pp.tile([128, 80], F32)
                    nc.tensor.matmul(ps[:], lhsT=xt0[:, i * 128:(i + 1) * 128],
                                     rhs=wc_sb[:, 0, :], start=True, stop=False)
                    nc.tensor.matmul(ps[:], lhsT=xt1[:, i * 128:(i + 1) * 128],
                                     rhs=wc_sb[:, 1, :], start=False, stop=True)
                    nc.vector.tensor_copy(tout[:, i, 0:64], ps[:, 0:64])
                    nc.vector.tensor_copy(ex[:, i, :], ps[:, 64:80])
                tv = tout[:, :, 64:96].bitcast(F16)      # [128, SLAB, 16]
                nc.scalar.activation(out=tv[:, :, 0:8], in_=ex[:, :, 0:8],
                                     func=AF.Exp)
                nc.scalar.activation(out=tv[:, :, 8:16], in_=ex[:, :, 0:8],
                                     func=AF.Exp, scale=0.2)
                nc.scalar.activation(out=rout[:], in_=ex[:, :, 8:16],
                                     func=AF.Exp, scale=-0.8)
                rows = slice(s * SLAB * 128, (s + 1) * SLAB * 128)
                nc.scalar.dma_start(
                    t1_d.ap()[rows, :].rearrange("(i p) f -> p i f", p=128),
                    tout[:])
                nc.scalar.dma_start(
                    r1_d.ap()[rows, :].rearrange("(i p) f -> p i f", p=128),
                    rout[:])
    nc.compile()
    return nc


def _batch_geometry(meta):
    G, qoff = meta["G"], meta["qoff"]
    sched = meta["sched"]
    soff = sched["soff"]
    NBAT = meta["NBAT"]
    sb = sched["b"]
    blo = np.searchsorted(sb, np.arange(NBAT))
    bhi = np.searchsorted(sb, np.arange(NBAT), side="right")
    slo = [int(soff[blo[b]]) for b in range(NBAT)]
    shi = [int(soff[bhi[b]]) for b in range(NBAT)]
    nw32max = max(1, max(shi[b] - slo[b] for b in range(NBAT)))
    qb_lo = [int(qoff[b, 0]) for b in range(NBAT)]
    qb_hi = [int(qoff[b, NWIN - 1] + G[b, NWIN - 1] * 128)
             for b in range(NBAT)]
    qbmax = max(qb_hi[b] - qb_lo[b] for b in range(NBAT))
    ent_by_t = {}
    for i in range(sched["n"]):
        ent_by_t.setdefault(int(sched["t"][i]), []).append(i)
    return blo, bhi, slo, shi, nw32max, qb_lo, qb_hi, qbmax, ent_by_t


def _emit_batches(nc, meta, pools, tab_ap, idx_d, s_d, re_d, elem, estep,
                  rwidth, mwidth, edge_ops, tile_out):
    """Shared batch loop: gathers, edge ops, banded S matmuls.

    edge_ops(Gs, rs, el, msg) fills msg [128, g, mwidth];
    tile_out(t, ps) consumes the per-tile PSUM accumulator."""
    G, qoff, goff = meta["G"], meta["qoff"], meta["goff"]
    sched = meta["sched"]
    sw, sj = sched["w"], sched["j"]
    scol, swid, soff = sched["col"], sched["wid"], sched["soff"]
    NBAT = meta["NBAT"]
    btiles = meta["btiles"]
    blo, bhi, slo, shi, nw32max, qb_lo, qb_hi, qbmax, ent_by_t = \
        _batch_geometry(meta)
    mp, gp, wkp, ppA, zrow = pools

    for b in range(NBAT):
        nw32 = max(shi[b] - slo[b], 1)
        ssb = mp.tile([128, nw32max, 32], F8, tag="s", bufs=2)
        if shi[b] > slo[b]:
            nc.sync.dma_start(
                ssb[:, 0:nw32, :],
                s_d.ap()[:, slo[b] * 32:shi[b] * 32]
                .rearrange("p (n c) -> p n c", c=32))
        nq = qb_hi[b] - qb_lo[b]
        idx_sb = mp.tile([IDXR, qbmax // 16], I16, tag="idx", bufs=2)
        nc.sync.dma_start(idx_sb[:, 0:nq // 16],
                          idx_d.ap()[:, qb_lo[b] // 16:qb_hi[b] // 16])
        slabs = {}
        for w in range(NWIN):
            g = int(G[b, w])
            q0 = int(qoff[b, w]) - qb_lo[b]
            g0 = int(goff[b, w])
            Gs = gp.tile([128, g, elem], tab_ap.dtype, tag="G", bufs=4)
            win0 = w * WINR
            win1 = min(win0 + WINR, NROWS)
            for g1 in range(0, g, GSPLIT):
                g2 = min(g1 + GSPLIT, g)
                nn = (g2 - g1) * 128
                _dma_gather_raw(
                    nc.gpsimd, Gs[:, g1:g2, :],
                    tab_ap[win0:win1, 0:elem],
                    idx_sb[:, (q0 + g1 * 128) // 16:(q0 + g2 * 128) // 16],
                    nn, elem, estep)
            rs = gp.tile([128, g, rwidth], F16, tag="rs", bufs=3)
            nc.sync.dma_start(
                rs[:], re_d.ap()[:, g0 * rwidth:(g0 + g) * rwidth]
                .rearrange("p (g r) -> p g r", r=rwidth))
            msg = wkp.tile([128, g, mwidth], F16, tag="msg", bufs=5)
            el = wkp.tile([128, g, rwidth], F16, tag="el", bufs=3)
            edge_ops(Gs, rs, el, msg)
            slabs[w] = msg
        for t in btiles[b]:
            ents = ent_by_t.get(t, [])
            ps = ppA.tile([128, mwidth], F32, tag="ps")
            nc.tensor.matmul(ps[:], lhsT=zrow[:], rhs=zrow[:, 0:mwidth],
                             start=True, stop=False, skip_group_check=True)
            for n, i in enumerate(ents):
                w, j = int(sw[i]), int(sj[i])
                col, wid = int(scol[i]), int(swid[i])
                so = int(soff[i]) - slo[b]
                nc.tensor.matmul(
                    ps[col * 32:(col + wid) * 32, :],
                    lhsT=ssb[:, so:so + wid, :]
                    .rearrange("p n c -> p (n c)"),
                    rhs=slabs[w][:, j, :],
                    start=False, stop=(n == len(ents) - 1),
                    tile_position=(0, col * 32),
                    skip_group_check=True)
            tile_out(t, ps)


def _build_launch2(meta):
    nc = _new_nc()
    t1_d = nc.dram_tensor("t1", [NROWS, 256], F8, kind="ExternalInput")
    idx_d = nc.dram_tensor("idx", [IDXR, meta["TOTQ"] // 16], I16,
                           kind="ExternalInput")
    s_d = nc.dram_tensor("sall", [128, meta["sched"]["totw"] * 32], F8,
                         kind="ExternalInput")
    re_d = nc.dram_tensor("re1", [128, meta["TOTG"] * 8], F16,
                          kind="ExternalInput")
    w2_d = nc.dram_tensor("w2e", [64, 18], F16, kind="ExternalInput")
    id_d = nc.dram_tensor("idm", [128, 128], F16, kind="ExternalInput")
    t2_d = nc.dram_tensor("t2s", [MPC, TW2], F16, kind="ExternalOutput")
    r2_d = nc.dram_tensor("r2", [MPC, 1], F32, kind="ExternalOutput")

    with tile.TileContext(nc) as tc:
        with tc.tile_pool(name="res", bufs=1) as rp, \
             tc.tile_pool(name="m", bufs=1) as mp, \
             tc.tile_pool(name="g", bufs=1) as gp, \
             tc.tile_pool(name="wk", bufs=1) as wkp, \
             tc.tile_pool(name="ep", bufs=1) as ep, \
             tc.tile_pool(name="zi", bufs=3) as zp, \
             tc.tile_pool(name="psA", bufs=4, space="PSUM") as ppA, \
             tc.tile_pool(name="psB", bufs=2, space="PSUM") as ppB:
            zrow = rp.tile([1, 128], F16)
            nc.vector.memset(zrow[:], 0.0)
            w2_sb = rp.tile([64, 18], F16)
            nc.sync.dma_start(w2_sb[:], w2_d.ap())
            idm = rp.tile([128, 128], F16)
            nc.sync.dma_start(idm[:], id_d.ap())
            ybuf = rp.tile([128, NT, 72], F16)
            t2t = rp.tile([128, NT, TW2], F16)
            pbuf = rp.tile([128, NT, 2], F32)
            r2sb = rp.tile([128, NT], F32)

            def edge_ops(Gs, rs, el, msg):
                g = Gs.shape[1]
                es_v = Gs[:, :, 64:80].bitcast(F16)
                e02_v = Gs[:, :, 80:96].bitcast(F16)
                nc.vector.tensor_tensor(out=el[:], in0=e02_v, in1=rs[:],
                                        op=ALU.mult)
                nc.vector.tensor_tensor(out=el[:], in0=es_v, in1=el[:],
                                        op=ALU.max)
                nc.vector.tensor_tensor(
                    out=msg[:, :, 0:64].rearrange("p g (h c) -> p g h c", h=8),
                    in0=Gs[:, :, 0:64].rearrange("p g (h c) -> p g h c", h=8),
                    in1=el[:].to_broadcast([128, g, 8, 8]), op=ALU.mult)
                nc.vector.tensor_copy(msg[:, :, 64:72], el[:])

            def tile_out(t, ps):
                nc.scalar.copy(ybuf[:, t, :], ps[:])

            _emit_batches(nc, meta, (mp, gp, wkp, ppA, zrow), t1_d.ap(),
                          idx_d, s_d, re_d, TB1, 256, 8, 72,
                          edge_ops, tile_out)

            # ---- batched epilogue ----
            rec = ep.tile([128, NT, 8], F32)
            nc.vector.tensor_scalar_add(rec[:], ybuf[:, :, 64:72], EPS)
            nc.vector.reciprocal(rec[:], rec[:])
            y16 = ep.tile([128, NT, 64], F16)
            nc.vector.tensor_tensor(
                out=y16[:].rearrange("p t (h c) -> p t h c", h=8),
                in0=ybuf[:, :, 0:64].rearrange("p t (h c) -> p t h c", h=8),
                in1=rec[:].to_broadcast([128, NT, 8, 8]), op=ALU.mult)
            yn = ep.tile([128, NT, 64], F16)
            nc.vector.tensor_scalar_min(yn[:], y16[:], 0.0)
            ey = ep.tile([128, NT, 64], F16)
            nc.scalar.activation(out=ey[:], in_=yn[:], func=AF.Exp)
            nc.vector.tensor_scalar_add(ey[:], ey[:], -1.0)
            elu = ep.tile([128, NT, 64], F16)
            nc.vector.tensor_tensor(out=elu[:], in0=y16[:], in1=ey[:],
                                    op=ALU.max)
            # z = elu @ [W2 | w2a | w2d] per tile via PE transpose
            for t0 in range(0, NT, 8):
                nz = min(8, NT - t0)
                zacc = ppB.tile([128, 8, 32], F32, tag="zacc", bufs=2)
                for k in range(nz):
                    t = t0 + k
                    tp = ppB.tile([64, 128], F16, tag="tp", bufs=2)
                    nc.tensor.transpose(tp[:], elu[:, t, :], idm[:])
                    zin = zp.tile([64, 128], F16, tag="zin")
                    nc.scalar.copy(zin[:], tp[:])
                    nc.tensor.matmul(zacc[:, k, 0:18], lhsT=zin[:],
                                     rhs=w2_sb[:], start=True, stop=True)
                nc.vector.tensor_copy(t2t[:, t0:t0 + nz, 0:16],
                                      zacc[:, 0:nz, 0:16])
                nc.vector.tensor_copy(pbuf[:, t0:t0 + nz, :],
                                      zacc[:, 0:nz, 16:18])
            nc.scalar.activation(out=t2t[:, :, 16:17], in_=pbuf[:, :, 0:1],
                                 func=AF.Exp)
            nc.scalar.activation(out=t2t[:, :, 17:18], in_=pbuf[:, :, 0:1],
                                 func=AF.Exp, scale=0.2)
            nc.scalar.activation(out=r2sb[:], in_=pbuf[:, :, 1:2],
                                 func=AF.Exp, scale=-0.8)
            nc.scalar.dma_start(
                t2_d.ap().rearrange("(t p) f -> p t f", p=128), t2t[:])
            nc.scalar.dma_start(
                r2_d.ap().rearrange("(t p) o -> p (t o)", p=128), r2sb[:])
    nc.compile()
    return nc


def _build_launch3(meta):
    nc = _new_nc()
    t2_d = nc.dram_tensor("t2", [NROWS, 128], F16, kind="ExternalInput")
    idx_d = nc.dram_tensor("idx", [IDXR, meta["TOTQ"] // 16], I16,
                           kind="ExternalInput")
    s_d = nc.dram_tensor("sall", [128, meta["sched"]["totw"] * 32], F8,
                         kind="ExternalInput")
    re_d = nc.dram_tensor("re2", [128, meta["TOTG"]], F16,
                          kind="ExternalInput")
    o_d = nc.dram_tensor("o", [MPC, 16], F32, kind="ExternalOutput")

    with tile.TileContext(nc) as tc:
        with tc.tile_pool(name="res", bufs=1) as rp, \
             tc.tile_pool(name="m", bufs=1) as mp, \
             tc.tile_pool(name="g", bufs=1) as gp, \
             tc.tile_pool(name="wk", bufs=1) as wkp, \
             tc.tile_pool(name="ep", bufs=1) as ep, \
             tc.tile_pool(name="psA", bufs=4, space="PSUM") as ppA:
            zrow = rp.tile([1, 128], F16)
            nc.vector.memset(zrow[:], 0.0)
            obuf = rp.tile([128, NT, 17], F32)

            def edge_ops(Gs, rs, el, msg):
                g = Gs.shape[1]
                nc.vector.tensor_tensor(out=el[:], in0=Gs[:, :, 17:18],
                                        in1=rs[:], op=ALU.mult)
                nc.vector.tensor_tensor(out=el[:], in0=Gs[:, :, 16:17],
                                        in1=el[:], op=ALU.max)
                nc.vector.tensor_tensor(
                    out=msg[:, :, 0:16], in0=Gs[:, :, 0:16],
                    in1=el[:].rearrange("p g o -> p (g o)")
                    .to_broadcast([128, g, 16]), op=ALU.mult)
                nc.vector.tensor_copy(msg[:, :, 16:17], el[:])

            def tile_out(t, ps):
                nc.scalar.copy(obuf[:, t, :], ps[:])

            _emit_batches(nc, meta, (mp, gp, wkp, ppA, zrow), t2_d.ap(),
                          idx_d, s_d, re_d, TW2, 128, 1, 17,
                          edge_ops, tile_out)

            # ---- batched log_softmax epilogue ----
            rec = ep.tile([128, NT, 1], F32)
            nc.vector.tensor_scalar_add(rec[:], obuf[:, :, 16:17], EPS)
            nc.vector.reciprocal(rec[:], rec[:])
            o1 = ep.tile([128, NT, 16], F32)
            nc.vector.tensor_tensor(
                out=o1[:], in0=obuf[:, :, 0:16],
                in1=rec[:].rearrange("p t o -> p (t o)")
                .to_broadcast([128, NT, 16]), op=ALU.mult)
            mx = ep.tile([128, NT, 1], F32)
            nc.vector.tensor_reduce(out=mx[:], in_=o1[:], axis=AX.X,
                                    op=ALU.max)
            nc.vector.tensor_tensor(
                out=o1[:], in0=o1[:],
                in1=mx[:].rearrange("p t o -> p (t o)")
                .to_broadcast([128, NT, 16]), op=ALU.subtract)
            es = ep.tile([128, NT, 16], F16)
            nc.scalar.activation(out=es[:], in_=o1[:], func=AF.Exp)
            ssum = ep.tile([128, NT, 1], F32)
            nc.vector.tensor_reduce(out=ssum[:], in_=es[:], axis=AX.X,
                                    op=ALU.add)
            lns = ep.tile([128, NT, 1], F32)
            nc.scalar.activation(out=lns[:], in_=ssum[:], func=AF.Ln)
            nc.vector.tensor_tensor(
                out=o1[:], in0=o1[:],
                in1=lns[:].rearrange("p t o -> p (t o)")
                .to_broadcast([128, NT, 16]), op=ALU.subtract)
            nc.scalar.dma_start(
                o_d.ap().rearrange("(t p) f -> p t f", p=128), o1[:])
    nc.compile()
    return nc


# --------------------------------------------------------------------------
# the kernel
# --------------------------------------------------------------------------

def kernel(x, edge_index, W1, a_src1, a_dst1, b1, W2, a_src2, a_dst2, b2):
    x = np.asarray(x, np.float32)
    edge_index = np.asarray(edge_index)
    W1 = np.asarray(W1, np.float32)
    W2 = np.asarray(W2, np.float32)
    a_src1 = np.asarray(a_src1, np.float32)
    a_dst1 = np.asarray(a_dst1, np.float32)
    a_src2 = np.asarray(a_src2, np.float32)
    a_dst2 = np.asarray(a_dst2, np.float32)

    key = edge_index.tobytes()[:4096]
    if _CACHE.get("key") != key:
        meta = _preprocess(edge_index)
        idx_all, s_all, streams = _build_idx_and_s(meta)
        _CACHE.update(key=key, meta=meta, idx_all=idx_all, s_all=s_all,
                      streams=streams,
                      nc1=_build_launch1(), nc2=_build_launch2(meta),
                      nc3=_build_launch3(meta))
    meta = _CACHE["meta"]
    idx_all, s_all, streams = (_CACHE["idx_all"], _CACHE["s_all"],
                               _CACHE["streams"])

    # weight packing
    W1r = W1.reshape(IN, HEADS, HID)
    B1 = np.einsum("khc,hc->kh", W1r, a_src1)        # [256, 8]
    C1 = np.einsum("khc,hc->kh", W1r, a_dst1)
    wc = np.concatenate([W1, B1, C1], 1).astype(np.float16)   # [256, 80]
    w2a = W2 @ a_src2[0]                              # [64]
    w2d = W2 @ a_dst2[0]
    w2e = np.concatenate([W2, w2a[:, None], w2d[:, None]],
                         1).astype(np.float16)        # [64, 18]
    idm = np.eye(128, dtype=np.float16)

    # launch 1: build T1 slices
    perm = meta["perm_nodes"]
    xT = np.zeros((IN, NROWS), np.float16)
    real = perm >= 0
    xT[:, real] = x[perm[real]].astype(np.float16).T
    in1 = [{"xs": np.ascontiguousarray(xT[:, c * MPC:(c + 1) * MPC]),
            "wc": wc} for c in range(NCORES)]
    r1_res = bass_utils.run_bass_kernel_spmd(
        _CACHE["nc1"], in1, core_ids=list(range(NCORES)), trace=TRACE)
    T1 = np.zeros((NROWS, 256), np.uint8)
    for c in range(NCORES):
        T1[c * MPC:(c + 1) * MPC, 0:TB1] = \
            np.asarray(r1_res.results[c]["t1s"]).view(np.uint8)
    T1 = T1.view(ml_dtypes.float8_e4m3)

    # launch 2: layer-1 message passing -> T2 slices
    in2 = []
    for c in range(NCORES):
        re1 = _expand_stream(streams[c], np.asarray(r1_res.results[c]["r1"]),
                             8, meta["TOTG"])
        in2.append({"t1": T1, "idx": idx_all[c], "sall": s_all[c],
                    "re1": re1.reshape(128, -1), "w2e": w2e, "idm": idm})
    r2_res = bass_utils.run_bass_kernel_spmd(
        _CACHE["nc2"], in2, core_ids=list(range(NCORES)), trace=TRACE)
    T2 = np.zeros((NROWS, 128), np.float16)
    for c in range(NCORES):
        T2[c * MPC:(c + 1) * MPC, 0:TW2] = \
            np.asarray(r2_res.results[c]["t2s"])

    # launch 3: layer-2 + log_softmax
    in3 = []
    for c in range(NCORES):
        re2 = _expand_stream(streams[c], np.asarray(r2_res.results[c]["r2"]),
                             1, meta["TOTG"])
        in3.append({"t2": T2, "idx": idx_all[c], "sall": s_all[c],
                    "re2": re2.reshape(128, -1)})
    r3_res = bass_utils.run_bass_kernel_spmd(
        _CACHE["nc3"], in3, core_ids=list(range(NCORES)), trace=TRACE)
    o_all = np.concatenate([np.asarray(r3_res.results[c]["o"])
                            for c in range(NCORES)], 0)

    out = o_all[meta["pos"][np.arange(N)]].astype(np.float32)
    _CACHE["exec_ns"] = [r.exec_time_ns for r in (r1_res, r2_res, r3_res)]
    return out


def predict_ns():
    """Cost-model (TimelineSim) per-launch predictions for cached programs."""
    from concourse.timeline_sim import TimelineSim
    out = []
    for k in ("nc1", "nc2", "nc3"):
        out.append(TimelineSim(_CACHE[k]).simulate())
    return out


# revision 20
# speedup vs baseline: 1.9529x; 1.0247x over previous
"""2-layer GAT on 8 trn2 NeuronCores (Bass/Tile).

Node-partitioned (12500/core, padded 12544), edges assigned by destination,
per-edge dma_gather of source-node table rows, segment softmax via the
factorization  exp(leaky(s+a)) = A * max(exp(s), exp(0.2 s) * exp(-0.8 a))
(per-dst factor A cancels), segment sums via banded one-hot S matmuls on
the PE.  Three SPMD launches with host halo exchange between them:

  1. "build":  h1 = x @ W1 + attention projections -> per-node table T1
     rows of 96B: [h fp8e4 x64 | exp(s) fp16 x8 | exp(0.2 s) fp16 x8],
     256B row stride in DRAM; r1 = exp(-0.8 a) per node.
  2. "layer1": per-edge 96B gathers from T1 (cost-model: 8.5 ns/descriptor
     vs 22.8 at 256B), edge softmax, banded S matmuls -> per-node epilogue
     (batched: softmax-normalize, ELU, z = elu @ [W2|w2a|w2d] via PE
     transpose) -> T2 rows of 36B: [z fp16 x16 | exp(s2) | exp(0.2 s2)].
  3. "layer2": 36B gathers from T2, 17-wide messages [el*z | el], banded
     S matmuls, batched log_softmax epilogue (single Ln table load).

Folding W2 into the T2 table (z instead of the 64-wide hidden vector) cuts
layer-2 gather/message/matmul width 4x and removes the output-head matmul.
"""

import numpy as np
import ml_dtypes

import concourse.bacc as bacc
import concourse.tile as tile
import concourse.mybir as mybir
from concourse import bass_utils
from concourse.bass import ap_utils, exact_div, MemorySpace

F32 = mybir.dt.float32
F16 = mybir.dt.float16
F8 = mybir.dt.float8e4
I16 = mybir.dt.int16
AF = mybir.ActivationFunctionType
ALU = mybir.AluOpType
AX = mybir.AxisListType

# problem constants (hardcoded per the task statement)
NCORES = 8
N = 100000
IN = 256
HID = 8
HEADS = 8
OUT = 16
NEG = 0.2
NPC = 12500            # real nodes per core
MPC = 12544            # padded nodes per core (98 * 128)
NT = MPC // 128        # 98 dst tiles per core
BATCH_EDGES = 24576    # shared edge budget per batch
NROWS = NCORES * MPC   # 100352 table rows
WINR = 32512           # gather window rows (int16-safe)
NWIN = (NROWS + WINR - 1) // WINR  # 4
EPS = 1e-16
SLAB1 = 14             # launch-1 chunks per slab (must divide NT)
TB1 = 96               # T1 gathered bytes: 64 fp8 h + 16 fp16 exps
TW2 = 18               # T2 row width in fp16: 16 z + 2 exps

_CACHE = {}
TRACE = False
GSPLIT = 96            # max slabs (x128 idxs) per dma_gather call
IDXR = 32              # idx tile partition replication (ucode reads <=32)


# --------------------------------------------------------------------------
# raw gather: InstDMAGatherAnt without the elem%256B assert (the non-
# transpose ucode path supports any elem size; only the row STRIDE must be
# a multiple of 256B)
# --------------------------------------------------------------------------

def _dma_gather_raw(ns, out_ap, in_ap, idxs_ap, num_idxs, elem_size,
                    elem_step, queue_num=0):
    assert idxs_ap.dtype == mybir.dt.int16
    assert in_ap.dtype == out_ap.dtype
    assert in_ap.space == MemorySpace.DRAM
    assert ap_utils.ap_is_contiguous(in_ap.ap[1:])
    assert ap_utils.ap_is_contiguous(out_ap.ap[1:])
    assert ap_utils.ap_is_contiguous(idxs_ap.ap[1:])
    assert in_ap.ap[-1][1] == out_ap.ap[-1][1] == elem_size
    assert out_ap.ap[0][1] * out_ap.ap[1][1] == (num_idxs + 127) // 128 * 128
    assert in_ap.ap[0][0] == elem_step
    stride_bytes_256 = exact_div(elem_step * mybir.dt.size(in_ap.dtype), 256)
    assert 0 < stride_bytes_256 < 256
    _in_ap = ns.lower_ap_dma(in_ap, for_custom_bir_dma=True)
    return ns.add_instruction(
        mybir.InstDMAGatherAnt(
            name=ns.bass.get_next_instruction_name(),
            ins=[*_in_ap, ns.lower_ap(idxs_ap),
                 ns.lower_val_access(ns.to_reg(num_idxs))],
            outs=[ns.lower_ap(out_ap)],
            transpose=False, num_idxs=num_idxs, elem_size=elem_size,
            stride_bytes_256=stride_bytes_256, gen_mode=0,
            single_packet=False, queue_num=queue_num,
            sbuf_tokens_per_rank=0, sbuf_free_dim_per_rank=0,
            sbuf_free_dim_pad_per_rank=0, sbuf_byte_offset=0))


# --------------------------------------------------------------------------
# host-side graph preprocessing (pure index work, unchanged from baseline)
# --------------------------------------------------------------------------

def _preprocess(edge_index):
    src = np.concatenate([np.asarray(edge_index[0]), np.arange(N)])
    dst = np.concatenate([np.asarray(edge_index[1]), np.arange(N)])
    deg = np.bincount(dst, minlength=N)

    # permutation: per core, nodes sorted by degree desc
    pos = np.empty(N, np.int64)
    perm_nodes = np.empty(NROWS, np.int64)   # table row -> node id (or -1)
    perm_nodes.fill(-1)
    for c in range(NCORES):
        ids = np.arange(c * NPC, (c + 1) * NPC)
        order = np.argsort(-deg[ids], kind="stable")
        pos[ids[order]] = c * MPC + np.arange(NPC)
        perm_nodes[c * MPC:c * MPC + NPC] = ids[order]

    srcpos = pos[src]
    dstpos = pos[dst]

    cores = []
    counts = np.zeros((NCORES, NT, NWIN), np.int64)
    per_core = []
    for c in range(NCORES):
        m = (dst >= c * NPC) & (dst < (c + 1) * NPC)
        sp = srcpos[m]
        rank = dstpos[m] - c * MPC
        t = rank // 128
        w = sp // WINR
        per_core.append((sp, rank, t, w))
        np.add.at(counts[c], (t, w), 1)
    stc = counts.max(0)                              # [NT, NWIN]
    tile_load = stc.sum(1)
    bmap = np.zeros(NT, np.int64)
    acc = 0
    b = 0
    for t in range(NT):
        if acc and acc + tile_load[t] > BATCH_EDGES:
            b += 1
            acc = 0
        bmap[t] = b
        acc += tile_load[t]
    NBAT = int(bmap[-1]) + 1
    btiles = [list(np.where(bmap == bb)[0]) for bb in range(NBAT)]
    toff = np.zeros((NT, NWIN), np.int64)
    gsz = np.zeros((NBAT, NWIN), np.int64)
    for bb in range(NBAT):
        for w in range(NWIN):
            off = 0
            for t in btiles[bb]:
                toff[t, w] = off
                off += stc[t, w]
            gsz[bb, w] = off
    G = np.maximum((gsz + 127) // 128, 1)            # [NBAT, NWIN] slabs
    Q = G * 128
    qoff = np.zeros((NBAT, NWIN), np.int64)
    goff = np.zeros((NBAT, NWIN), np.int64)
    acc_q = 0
    for bb in range(NBAT):
        for w in range(NWIN):
            qoff[bb, w] = acc_q
            goff[bb, w] = acc_q // 128
            acc_q += Q[bb, w]
    TOTQ = acc_q
    TOTG = TOTQ // 128

    for c in range(NCORES):
        sp, rank, t, w = per_core[c]
        b = bmap[t]
        order = np.lexsort((rank, w, t))
        sp, rank, t, w, b = (sp[order], rank[order], t[order], w[order],
                             b[order])
        gid = t * NWIN + w
        gstart = np.searchsorted(gid, np.arange(NT * NWIN), side="left")
        within = np.arange(len(gid)) - gstart[gid]
        q = qoff[b, w] + toff[t, w] + within
        cores.append({"sp": sp, "rank": rank, "b": b, "w": w, "q": q})

    # union matmul schedule, merged per (b, t, w, j) with a band range.
    JMAX = TOTQ // 128 + 1
    keysets = []
    for c in range(NCORES):
        d = cores[c]
        j = (d["q"] - qoff[d["b"], d["w"]]) // 128
        t = d["rank"] // 128
        a = (d["rank"] % 128) // 32
        key = (t * NWIN + d["w"]) * JMAX + j
        keysets.append((key, a))
        d["j"] = j
        d["t"] = t
        d["key"] = key
    allk = np.concatenate([k for k, _ in keysets])
    alla = np.concatenate([a for _, a in keysets])
    ukeys, inv = np.unique(allk, return_inverse=True)
    TOTB = len(ukeys)
    amin = np.full(TOTB, 4, np.int64)
    amax = np.full(TOTB, -1, np.int64)
    np.minimum.at(amin, inv, alla)
    np.maximum.at(amax, inv, alla)
    ecol = np.where(amin == amax, amin,
                    np.where((amin == 0) & (amax == 1), 0,
                             np.where((amin == 2) & (amax == 3), 2, 0)))
    ewid = np.where(amin == amax, 1,
                    np.where((amin == 0) & (amax == 1), 2,
                             np.where((amin == 2) & (amax == 3), 2, 4)))
    soff = np.concatenate([[0], np.cumsum(ewid)])   # block col offsets (32u)
    uj = ukeys % JMAX
    r1 = ukeys // JMAX
    uw = r1 % NWIN
    ut = r1 // NWIN
    ub = bmap[ut]
    sched = {"b": ub, "t": ut, "w": uw, "j": uj, "col": ecol, "wid": ewid,
             "soff": soff, "n": TOTB, "totw": int(soff[-1])}

    for c in range(NCORES):
        d = cores[c]
        ent = np.searchsorted(ukeys, d["key"])
        d["ent"] = ent
        d["k"] = d["q"] % 128
        d["scol"] = d["rank"] % 128 - ecol[ent] * 32

    meta = {"G": G, "Q": Q, "qoff": qoff, "goff": goff, "TOTQ": TOTQ,
            "TOTG": TOTG, "sched": sched, "pos": pos, "NBAT": NBAT,
            "btiles": btiles, "perm_nodes": perm_nodes, "cores": cores}
    return meta


def _build_idx_and_s(meta):
    """Per-core gather index arrays (int16 wrapped) and fp8 S blocks."""
    TOTQ = meta["TOTQ"]
    idx_all, s_all, streams = [], [], []
    for c in range(NCORES):
        d = meta["cores"][c]
        flat = np.zeros(TOTQ, np.int16)
        loc = d["sp"] - d["w"] * WINR
        flat[d["q"]] = loc.astype(np.int16)
        resh = flat.reshape(TOTQ // 16, 16).T          # [16, TOTQ/16]
        idxw = np.tile(resh, (IDXR // 16, 1)).copy()   # [IDXR, TOTQ/16]
        idx_all.append(idxw)

        soff = meta["sched"]["soff"]
        totw = meta["sched"]["totw"]
        S = np.zeros((128, totw * 32), ml_dtypes.float8_e4m3)
        S[d["k"], soff[d["ent"]] * 32 + d["scol"]] = 1.0
        s_all.append(S)

        streams.append((d["q"] % 128, d["q"] // 128, d["rank"]))
    return idx_all, s_all, streams


def _expand_stream(stream, r_core, width, totg):
    """r_core [MPC, width] f32 -> per-position [128, totg, width] f16."""
    p, g, rank = stream
    out = np.zeros((128, int(totg), width), np.float16)
    out[p, g, :] = r_core[rank, :width].astype(np.float16)
    return out


# --------------------------------------------------------------------------
# launch builders
# --------------------------------------------------------------------------

def _new_nc():
    return bacc.Bacc("TRN2", target_bir_lowering=False, debug=False,
                     enable_asserts=False, num_devices=NCORES)


def _build_launch1():
    nc = _new_nc()
    xs_d = nc.dram_tensor("xs", [IN, MPC], F16, kind="ExternalInput")
    wc_d = nc.dram_tensor("wc", [IN, 80], F16, kind="ExternalInput")
    t1_d = nc.dram_tensor("t1s", [MPC, TB1], F8, kind="ExternalOutput")
    r1_d = nc.dram_tensor("r1", [MPC, 8], F32, kind="ExternalOutput")
    SLAB = SLAB1
    with tile.TileContext(nc) as tc:
        with tc.tile_pool(name="w", bufs=1) as wp, \
             tc.tile_pool(name="x", bufs=3) as xp, \
             tc.tile_pool(name="o", bufs=3) as op, \
             tc.tile_pool(name="ps", bufs=4, space="PSUM") as pp:
            wc_sb = wp.tile([128, 2, 80], F16)
            nc.sync.dma_start(wc_sb[:, 0, :], wc_d.ap()[0:128, :])
            nc.sync.dma_start(wc_sb[:, 1, :], wc_d.ap()[128:256, :])
            for s in range(NT // SLAB):
                cols = slice(s * SLAB * 128, (s + 1) * SLAB * 128)
                xt0 = xp.tile([128, SLAB * 128], F16, tag="xt0")
                xt1 = xp.tile([128, SLAB * 128], F16, tag="xt1")
                nc.sync.dma_start(xt0[:], xs_d.ap()[0:128, cols])
                nc.sync.dma_start(xt1[:], xs_d.ap()[128:256, cols])
                tout = op.tile([128, SLAB, TB1], F8, tag="tout")
                ex = op.tile([128, SLAB, 16], F32, tag="ex")
                rout = op.tile([128, SLAB, 8], F32, tag="rout")
                for i in range(SLAB):
                    ps = pp.tile([128, 80], F32)
                    nc.tensor.matmul(ps[:], lhsT=xt0[:, i * 128:(i + 1) * 128],
                                     rhs=wc_sb[:, 0, :], start=True, stop=False)
                    nc.tensor.matmul(ps[:], lhsT=xt1[:, i * 128:(i + 1) * 128],
                                     rhs=wc_sb[:, 1, :], start=False, stop=True)
                    nc.vector.tensor_copy(tout[:, i, 0:64], ps[:, 0:64])
                    nc.scalar.copy(ex[:, i, :], ps[:, 64:80])
                tv = tout[:, :, 64:96].bitcast(F16)      # [128, SLAB, 16]
                nc.scalar.activation(out=tv[:, :, 0:8], in_=ex[:, :, 0:8],
                                     func=AF.Exp)
                nc.scalar.activation(out=tv[:, :, 8:16], in_=ex[:, :, 0:8],
                                     func=AF.Exp, scale=0.2)
                nc.scalar.activation(out=rout[:], in_=ex[:, :, 8:16],
                                     func=AF.Exp, scale=-0.8)
                rows = slice(s * SLAB * 128, (s + 1) * SLAB * 128)
                nc.scalar.dma_start(
                    t1_d.ap()[rows, :].rearrange("(i p) f -> p i f", p=128),
                    tout[:])
                nc.scalar.dma_start(
                    r1_d.ap()[rows, :].rearrange("(i p) f -> p i f", p=128),
                    rout[:])
    nc.compile()
    return nc


def _batch_geometry(meta):
    G, qoff = meta["G"], meta["qoff"]
    sched = meta["sched"]
    soff = sched["soff"]
    NBAT = meta["NBAT"]
    sb = sched["b"]
    blo = np.searchsorted(sb, np.arange(NBAT))
    bhi = np.searchsorted(sb, np.arange(NBAT), side="right")
    slo = [int(soff[blo[b]]) for b in range(NBAT)]
    shi = [int(soff[bhi[b]]) for b in range(NBAT)]
    nw32max = max(1, max(shi[b] - slo[b] for b in range(NBAT)))
    qb_lo = [int(qoff[b, 0]) for b in range(NBAT)]
    qb_hi = [int(qoff[b, NWIN - 1] + G[b, NWIN - 1] * 128)
             for b in range(NBAT)]
    qbmax = max(qb_hi[b] - qb_lo[b] for b in range(NBAT))
    ent_by_t = {}
    for i in range(sched["n"]):
        ent_by_t.setdefault(int(sched["t"][i]), []).append(i)
    return blo, bhi, slo, shi, nw32max, qb_lo, qb_hi, qbmax, ent_by_t


def _emit_batches(nc, meta, pools, tab_ap, idx_d, s_d, re_d, elem, estep,
                  rwidth, mwidth, edge_ops, tile_out, batch_out=None):
    """Shared batch loop: gathers, edge ops, banded S matmuls.

    edge_ops(Gs, rs, el, msg) fills msg [128, g, mwidth];
    tile_out(t, ps) consumes the per-tile PSUM accumulator;
    batch_out(t0, t1) runs after each batch's tiles [t0, t1) complete."""
    G, qoff, goff = meta["G"], meta["qoff"], meta["goff"]
    sched = meta["sched"]
    sw, sj = sched["w"], sched["j"]
    scol, swid, soff = sched["col"], sched["wid"], sched["soff"]
    NBAT = meta["NBAT"]
    btiles = meta["btiles"]
    blo, bhi, slo, shi, nw32max, qb_lo, qb_hi, qbmax, ent_by_t = \
        _batch_geometry(meta)
    mp, gp, wkp, ppA, zrow = pools

    for b in range(NBAT):
        nw32 = max(shi[b] - slo[b], 1)
        ssb = mp.tile([128, nw32max, 32], F8, tag="s", bufs=2)
        if shi[b] > slo[b]:
            nc.sync.dma_start(
                ssb[:, 0:nw32, :],
                s_d.ap()[:, slo[b] * 32:shi[b] * 32]
                .rearrange("p (n c) -> p n c", c=32))
        nq = qb_hi[b] - qb_lo[b]
        idx_sb = mp.tile([IDXR, qbmax // 16], I16, tag="idx", bufs=2)
        nc.sync.dma_start(idx_sb[:, 0:nq // 16],
                          idx_d.ap()[:, qb_lo[b] // 16:qb_hi[b] // 16])
        slabs = {}
        for w in range(NWIN):
            g = int(G[b, w])
            q0 = int(qoff[b, w]) - qb_lo[b]
            g0 = int(goff[b, w])
            Gs = gp.tile([128, g, elem], tab_ap.dtype, tag="G", bufs=4)
            win0 = w * WINR
            win1 = min(win0 + WINR, NROWS)
            for g1 in range(0, g, GSPLIT):
                g2 = min(g1 + GSPLIT, g)
                nn = (g2 - g1) * 128
                _dma_gather_raw(
                    nc.gpsimd, Gs[:, g1:g2, :],
                    tab_ap[win0:win1, 0:elem],
                    idx_sb[:, (q0 + g1 * 128) // 16:(q0 + g2 * 128) // 16],
                    nn, elem, estep)
            rs = gp.tile([128, g, rwidth], F16, tag="rs", bufs=3)
            nc.sync.dma_start(
                rs[:], re_d.ap()[:, g0 * rwidth:(g0 + g) * rwidth]
                .rearrange("p (g r) -> p g r", r=rwidth))
            msg = wkp.tile([128, g, mwidth], F16, tag="msg", bufs=5)
            el = wkp.tile([128, g, rwidth], F16, tag="el", bufs=3)
            edge_ops(Gs, rs, el, msg)
            slabs[w] = msg
        for t in btiles[b]:
            ents = ent_by_t.get(t, [])
            ps = ppA.tile([128, mwidth], F32, tag="ps")
            nc.tensor.matmul(ps[:], lhsT=zrow[:], rhs=zrow[:, 0:mwidth],
                             start=True, stop=False, skip_group_check=True)
            for n, i in enumerate(ents):
                w, j = int(sw[i]), int(sj[i])
                col, wid = int(scol[i]), int(swid[i])
                so = int(soff[i]) - slo[b]
                nc.tensor.matmul(
                    ps[col * 32:(col + wid) * 32, :],
                    lhsT=ssb[:, so:so + wid, :]
                    .rearrange("p n c -> p (n c)"),
                    rhs=slabs[w][:, j, :],
                    start=False, stop=(n == len(ents) - 1),
                    tile_position=(0, col * 32),
                    skip_group_check=True)
            tile_out(t, ps)
        if batch_out is not None:
            batch_out(btiles[b][0], btiles[b][-1] + 1)


def _build_launch2(meta):
    nc = _new_nc()
    t1_d = nc.dram_tensor("t1", [NROWS, 256], F8, kind="ExternalInput")
    idx_d = nc.dram_tensor("idx", [IDXR, meta["TOTQ"] // 16], I16,
                           kind="ExternalInput")
    s_d = nc.dram_tensor("sall", [128, meta["sched"]["totw"] * 32], F8,
                         kind="ExternalInput")
    re_d = nc.dram_tensor("re1", [128, meta["TOTG"] * 8], F16,
                          kind="ExternalInput")
    w2_d = nc.dram_tensor("w2e", [64, 18], F16, kind="ExternalInput")
    id_d = nc.dram_tensor("idm", [128, 128], F16, kind="ExternalInput")
    t2_d = nc.dram_tensor("t2s", [MPC, TW2], F16, kind="ExternalOutput")
    r2_d = nc.dram_tensor("r2", [MPC, 1], F32, kind="ExternalOutput")

    with tile.TileContext(nc) as tc:
        with tc.tile_pool(name="res", bufs=1) as rp, \
             tc.tile_pool(name="m", bufs=1) as mp, \
             tc.tile_pool(name="g", bufs=1) as gp, \
             tc.tile_pool(name="wk", bufs=1) as wkp, \
             tc.tile_pool(name="ep", bufs=1) as ep, \
             tc.tile_pool(name="zi", bufs=3) as zp, \
             tc.tile_pool(name="psA", bufs=4, space="PSUM") as ppA, \
             tc.tile_pool(name="psB", bufs=2, space="PSUM") as ppB:
            zrow = rp.tile([1, 128], F16)
            nc.vector.memset(zrow[:], 0.0)
            w2_sb = rp.tile([64, 18], F16)
            nc.sync.dma_start(w2_sb[:], w2_d.ap())
            idm = rp.tile([128, 128], F16)
            nc.sync.dma_start(idm[:], id_d.ap())
            ybuf = rp.tile([128, NT, 72], F16)
            t2t = rp.tile([128, NT, TW2], F16)
            pbuf = rp.tile([128, NT, 2], F32)
            r2sb = rp.tile([128, NT], F32)

            def edge_ops(Gs, rs, el, msg):
                g = Gs.shape[1]
                es_v = Gs[:, :, 64:80].bitcast(F16)
                e02_v = Gs[:, :, 80:96].bitcast(F16)
                nc.vector.tensor_tensor(out=el[:], in0=e02_v, in1=rs[:],
                                        op=ALU.mult)
                nc.vector.tensor_tensor(out=el[:], in0=es_v, in1=el[:],
                                        op=ALU.max)
                nc.vector.tensor_tensor(
                    out=msg[:, :, 0:64].rearrange("p g (h c) -> p g h c", h=8),
                    in0=Gs[:, :, 0:64].rearrange("p g (h c) -> p g h c", h=8),
                    in1=el[:].to_broadcast([128, g, 8, 8]), op=ALU.mult)
                nc.vector.tensor_copy(msg[:, :, 64:72], el[:])

            def tile_out(t, ps):
                nc.scalar.copy(ybuf[:, t, :], ps[:])

            def batch_out(t0, t1):
                nt = t1 - t0
                yb = ybuf[:, t0:t1, :]
                rec = ep.tile([128, NTB, 8], F32, tag="rec", bufs=2)
                nc.vector.tensor_scalar_add(rec[:, 0:nt, :],
                                            yb[:, :, 64:72], EPS)
                nc.vector.reciprocal(rec[:, 0:nt, :], rec[:, 0:nt, :])
                y16 = ep.tile([128, NTB, 64], F16, tag="y16", bufs=2)
                nc.vector.tensor_tensor(
                    out=y16[:, 0:nt, :]
                    .rearrange("p t (h c) -> p t h c", h=8),
                    in0=yb[:, :, 0:64].rearrange("p t (h c) -> p t h c", h=8),
                    in1=rec[:, 0:nt, :].to_broadcast([128, nt, 8, 8]),
                    op=ALU.mult)
                yn = ep.tile([128, NTB, 64], F16, tag="yn", bufs=2)
                nc.vector.tensor_scalar_min(yn[:, 0:nt, :], y16[:, 0:nt, :],
                                            0.0)
                nc.scalar.activation(out=yn[:, 0:nt, :], in_=yn[:, 0:nt, :],
                                     func=AF.Exp)
                nc.vector.tensor_scalar_add(yn[:, 0:nt, :], yn[:, 0:nt, :],
                                            -1.0)
                elu = ep.tile([128, NTB, 64], F16, tag="elu", bufs=2)
                nc.vector.tensor_tensor(out=elu[:, 0:nt, :],
                                        in0=y16[:, 0:nt, :],
                                        in1=yn[:, 0:nt, :], op=ALU.max)
                # z = elu @ [W2 | w2a | w2d] per tile via PE transpose
                for k0 in range(0, nt, 8):
                    nz = min(8, nt - k0)
                    zacc = ppB.tile([128, 8, 32], F32, tag="zacc", bufs=2)
                    for k in range(nz):
                        tp = ppB.tile([64, 128], F16, tag="tp", bufs=2)
                        nc.tensor.transpose(tp[:], elu[:, k0 + k, :], idm[:])
                        zin = zp.tile([64, 128], F16, tag="zin")
                        nc.scalar.copy(zin[:], tp[:])
                        nc.tensor.matmul(zacc[:, k, 0:18], lhsT=zin[:],
                                         rhs=w2_sb[:], start=True, stop=True)
                    tt = t0 + k0
                    nc.vector.tensor_copy(t2t[:, tt:tt + nz, 0:16],
                                          zacc[:, 0:nz, 0:16])
                    nc.vector.tensor_copy(pbuf[:, tt:tt + nz, :],
                                          zacc[:, 0:nz, 16:18])

            NTB = max(len(bt) for bt in meta["btiles"])
            _emit_batches(nc, meta, (mp, gp, wkp, ppA, zrow), t1_d.ap(),
                          idx_d, s_d, re_d, TB1, 256, 8, 72,
                          edge_ops, tile_out, batch_out)

            nc.scalar.activation(out=t2t[:, :, 16:17], in_=pbuf[:, :, 0:1],
                                 func=AF.Exp)
            nc.scalar.activation(out=t2t[:, :, 17:18], in_=pbuf[:, :, 0:1],
                                 func=AF.Exp, scale=0.2)
            nc.scalar.activation(out=r2sb[:], in_=pbuf[:, :, 1:2],
                                 func=AF.Exp, scale=-0.8)
            nc.scalar.dma_start(
                t2_d.ap().rearrange("(t p) f -> p t f", p=128), t2t[:])
            nc.scalar.dma_start(
                r2_d.ap().rearrange("(t p) o -> p (t o)", p=128), r2sb[:])
    nc.compile()
    return nc


def _build_launch3(meta):
    nc = _new_nc()
    t2_d = nc.dram_tensor("t2", [NROWS, 128], F16, kind="ExternalInput")
    idx_d = nc.dram_tensor("idx", [IDXR, meta["TOTQ"] // 16], I16,
                           kind="ExternalInput")
    s_d = nc.dram_tensor("sall", [128, meta["sched"]["totw"] * 32], F8,
                         kind="ExternalInput")
    re_d = nc.dram_tensor("re2", [128, meta["TOTG"]], F16,
                          kind="ExternalInput")
    o_d = nc.dram_tensor("o", [MPC, 16], F32, kind="ExternalOutput")

    with tile.TileContext(nc) as tc:
        with tc.tile_pool(name="res", bufs=1) as rp, \
             tc.tile_pool(name="m", bufs=1) as mp, \
             tc.tile_pool(name="g", bufs=1) as gp, \
             tc.tile_pool(name="wk", bufs=1) as wkp, \
             tc.tile_pool(name="ep", bufs=1) as ep, \
             tc.tile_pool(name="psA", bufs=4, space="PSUM") as ppA:
            zrow = rp.tile([1, 128], F16)
            nc.vector.memset(zrow[:], 0.0)
            obuf = rp.tile([128, NT, 17], F32)

            def edge_ops(Gs, rs, el, msg):
                g = Gs.shape[1]
                nc.vector.tensor_tensor(out=el[:], in0=Gs[:, :, 17:18],
                                        in1=rs[:], op=ALU.mult)
                nc.vector.tensor_tensor(out=el[:], in0=Gs[:, :, 16:17],
                                        in1=el[:], op=ALU.max)
                nc.vector.tensor_tensor(
                    out=msg[:, :, 0:16], in0=Gs[:, :, 0:16],
                    in1=el[:].rearrange("p g o -> p (g o)")
                    .to_broadcast([128, g, 16]), op=ALU.mult)
                nc.vector.tensor_copy(msg[:, :, 16:17], el[:])

            o16 = rp.tile([128, NT, 16], F32)
            ssum = rp.tile([128, NT, 1], F32)

            def tile_out(t, ps):
                nc.scalar.copy(obuf[:, t, :], ps[:])

            def batch_out(t0, t1):
                nt = t1 - t0
                ob = obuf[:, t0:t1, :]
                o1 = o16[:, t0:t1, :]
                rec = ep.tile([128, NTB, 1], F32, tag="rec", bufs=2)
                nc.vector.tensor_scalar_add(rec[:, 0:nt, :],
                                            ob[:, :, 16:17], EPS)
                nc.vector.reciprocal(rec[:, 0:nt, :], rec[:, 0:nt, :])
                nc.vector.tensor_tensor(
                    out=o1[:], in0=ob[:, :, 0:16],
                    in1=rec[:, 0:nt, :].rearrange("p t o -> p (t o)")
                    .to_broadcast([128, nt, 16]), op=ALU.mult)
                mx = ep.tile([128, NTB, 1], F32, tag="mx", bufs=2)
                nc.vector.tensor_reduce(out=mx[:, 0:nt, :], in_=o1[:],
                                        axis=AX.X, op=ALU.max)
                nc.vector.tensor_tensor(
                    out=o1[:], in0=o1[:],
                    in1=mx[:, 0:nt, :].rearrange("p t o -> p (t o)")
                    .to_broadcast([128, nt, 16]), op=ALU.subtract)
                es = ep.tile([128, NTB, 16], F16, tag="es", bufs=2)
                nc.scalar.activation(out=es[:, 0:nt, :], in_=o1[:],
                                     func=AF.Exp)
                nc.vector.tensor_reduce(out=ssum[:, t0:t1, :],
                                        in_=es[:, 0:nt, :], axis=AX.X,
                                        op=ALU.add)

            NTB = max(len(bt) for bt in meta["btiles"])
            _emit_batches(nc, meta, (mp, gp, wkp, ppA, zrow), t2_d.ap(),
                          idx_d, s_d, re_d, TW2, 128, 1, 17,
                          edge_ops, tile_out, batch_out)

            lns = ep.tile([128, NT, 1], F32, tag="lns")
            nc.scalar.activation(out=lns[:], in_=ssum[:], func=AF.Ln)
            nc.vector.tensor_tensor(
                out=o16[:], in0=o16[:],
                in1=lns[:].rearrange("p t o -> p (t o)")
                .to_broadcast([128, NT, 16]), op=ALU.subtract)
            nc.scalar.dma_start(
                o_d.ap().rearrange("(t p) f -> p t f", p=128), o16[:])
    nc.compile()
    return nc


# --------------------------------------------------------------------------
# the kernel
# --------------------------------------------------------------------------

def kernel(x, edge_index, W1, a_src1, a_dst1, b1, W2, a_src2, a_dst2, b2):
    x = np.asarray(x, np.float32)
    edge_index = np.asarray(edge_index)
    W1 = np.asarray(W1, np.float32)
    W2 = np.asarray(W2, np.float32)
    a_src1 = np.asarray(a_src1, np.float32)
    a_dst1 = np.asarray(a_dst1, np.float32)
    a_src2 = np.asarray(a_src2, np.float32)
    a_dst2 = np.asarray(a_dst2, np.float32)

    key = edge_index.tobytes()[:4096]
    if _CACHE.get("key") != key:
        meta = _preprocess(edge_index)
        idx_all, s_all, streams = _build_idx_and_s(meta)
        _CACHE.update(key=key, meta=meta, idx_all=idx_all, s_all=s_all,
                      streams=streams,
                      nc1=_build_launch1(), nc2=_build_launch2(meta),
                      nc3=_build_launch3(meta))
    meta = _CACHE["meta"]
    idx_all, s_all, streams = (_CACHE["idx_all"], _CACHE["s_all"],
                               _CACHE["streams"])

    # weight packing
    W1r = W1.reshape(IN, HEADS, HID)
    B1 = np.einsum("khc,hc->kh", W1r, a_src1)        # [256, 8]
    C1 = np.einsum("khc,hc->kh", W1r, a_dst1)
    wc = np.concatenate([W1, B1, C1], 1).astype(np.float16)   # [256, 80]
    w2a = W2 @ a_src2[0]                              # [64]
    w2d = W2 @ a_dst2[0]
    w2e = np.concatenate([W2, w2a[:, None], w2d[:, None]],
                         1).astype(np.float16)        # [64, 18]
    idm = np.eye(128, dtype=np.float16)

    # launch 1: build T1 slices
    perm = meta["perm_nodes"]
    xT = np.zeros((IN, NROWS), np.float16)
    real = perm >= 0
    xT[:, real] = x[perm[real]].astype(np.float16).T
    in1 = [{"xs": np.ascontiguousarray(xT[:, c * MPC:(c + 1) * MPC]),
            "wc": wc} for c in range(NCORES)]
    r1_res = bass_utils.run_bass_kernel_spmd(
        _CACHE["nc1"], in1, core_ids=list(range(NCORES)), trace=TRACE)
    T1 = np.zeros((NROWS, 256), np.uint8)
    for c in range(NCORES):
        T1[c * MPC:(c + 1) * MPC, 0:TB1] = \
            np.asarray(r1_res.results[c]["t1s"]).view(np.uint8)
    T1 = T1.view(ml_dtypes.float8_e4m3)

    # launch 2: layer-1 message passing -> T2 slices
    in2 = []
    for c in range(NCORES):
        re1 = _expand_stream(streams[c], np.asarray(r1_res.results[c]["r1"]),
                             8, meta["TOTG"])
        in2.append({"t1": T1, "idx": idx_all[c], "sall": s_all[c],
                    "re1": re1.reshape(128, -1), "w2e": w2e, "idm": idm})
    r2_res = bass_utils.run_bass_kernel_spmd(
        _CACHE["nc2"], in2, core_ids=list(range(NCORES)), trace=TRACE)
    T2 = np.zeros((NROWS, 128), np.float16)
    for c in range(NCORES):
        T2[c * MPC:(c + 1) * MPC, 0:TW2] = \
            np.asarray(r2_res.results[c]["t2s"])

    # launch 3: layer-2 + log_softmax
    in3 = []
    for c in range(NCORES):
        re2 = _expand_stream(streams[c], np.asarray(r2_res.results[c]["r2"]),
                             1, meta["TOTG"])
        in3.append({"t2": T2, "idx": idx_all[c], "sall": s_all[c],
                    "re2": re2.reshape(128, -1)})
    r3_res = bass_utils.run_bass_kernel_spmd(
        _CACHE["nc3"], in3, core_ids=list(range(NCORES)), trace=TRACE)
    o_all = np.concatenate([np.asarray(r3_res.results[c]["o"])
                            for c in range(NCORES)], 0)

    out = o_all[meta["pos"][np.arange(N)]].astype(np.float32)
    _CACHE["exec_ns"] = [r.exec_time_ns for r in (r1_res, r2_res, r3_res)]
    return out


def predict_ns():
    """Cost-model (TimelineSim) per-launch predictions for cached programs."""
    from concourse.timeline_sim import TimelineSim
    out = []
    for k in ("nc1", "nc2", "nc3"):
        out.append(TimelineSim(_CACHE[k]).simulate())
    return out


# revision 21
# speedup vs baseline: 2.0200x; 1.0344x over previous
"""2-layer GAT on 8 trn2 NeuronCores (Bass/Tile).

Node-partitioned (12500/core, padded 12544), edges assigned by destination,
per-edge dma_gather of source-node table rows, segment softmax via the
factorization  exp(leaky(s+a)) = A * max(exp(s), exp(0.2 s) * exp(-0.8 a))
(per-dst factor A cancels), segment sums via banded one-hot S matmuls on
the PE.  Three SPMD launches with host halo exchange between them:

  1. "build":  h1 = x @ W1 + attention projections -> per-node table T1
     rows of 96B: [h fp8e4 x64 | exp(s) fp16 x8 | exp(0.2 s) fp16 x8],
     256B row stride in DRAM; r1 = exp(-0.8 a) per node.
  2. "layer1": per-edge 96B gathers from T1 (cost-model: 8.5 ns/descriptor
     vs 22.8 at 256B), edge softmax, banded S matmuls -> per-node epilogue
     (batched: softmax-normalize, ELU, z = elu @ [W2|w2a|w2d] via PE
     transpose) -> T2 rows of 36B: [z fp16 x16 | exp(s2) | exp(0.2 s2)].
  3. "layer2": 36B gathers from T2, 17-wide messages [el*z | el], banded
     S matmuls, batched log_softmax epilogue (single Ln table load).

Folding W2 into the T2 table (z instead of the 64-wide hidden vector) cuts
layer-2 gather/message/matmul width 4x and removes the output-head matmul.
"""

import numpy as np
import ml_dtypes

import concourse.bacc as bacc
import concourse.tile as tile
import concourse.mybir as mybir
from concourse import bass_utils
from concourse.bass import ap_utils, exact_div, MemorySpace

F32 = mybir.dt.float32
F16 = mybir.dt.float16
F8 = mybir.dt.float8e4
I16 = mybir.dt.int16
AF = mybir.ActivationFunctionType
ALU = mybir.AluOpType
AX = mybir.AxisListType

# problem constants (hardcoded per the task statement)
NCORES = 8
N = 100000
IN = 256
HID = 8
HEADS = 8
OUT = 16
NEG = 0.2
NPC = 12500            # real nodes per core
MPC = 12544            # padded nodes per core (98 * 128)
NT = MPC // 128        # 98 dst tiles per core
BATCH_EDGES = 24576    # shared edge budget per batch
NROWS = NCORES * MPC   # 100352 table rows
WINR = 32512           # gather window rows (int16-safe)
NWIN = (NROWS + WINR - 1) // WINR  # 4
EPS = 1e-16
SLAB1 = 14             # launch-1 chunks per slab (must divide NT)
TB1 = 96               # T1 gathered bytes: 64 fp8 h + 16 fp16 exps
TW2 = 18               # T2 row width in fp16: 16 z + 2 exps

_CACHE = {}
TRACE = False
GSPLIT = 96            # max slabs (x128 idxs) per dma_gather call
IDXR = 32              # idx tile partition replication (ucode reads <=32)


# --------------------------------------------------------------------------
# raw gather: InstDMAGatherAnt without the elem%256B assert (the non-
# transpose ucode path supports any elem size; only the row STRIDE must be
# a multiple of 256B)
# --------------------------------------------------------------------------

def _dma_gather_raw(ns, out_ap, in_ap, idxs_ap, num_idxs, elem_size,
                    elem_step, queue_num=0):
    assert idxs_ap.dtype == mybir.dt.int16
    assert in_ap.dtype == out_ap.dtype
    assert in_ap.space == MemorySpace.DRAM
    assert ap_utils.ap_is_contiguous(in_ap.ap[1:])
    assert ap_utils.ap_is_contiguous(out_ap.ap[1:])
    assert ap_utils.ap_is_contiguous(idxs_ap.ap[1:])
    assert in_ap.ap[-1][1] == out_ap.ap[-1][1] == elem_size
    assert out_ap.ap[0][1] * out_ap.ap[1][1] == (num_idxs + 127) // 128 * 128
    assert in_ap.ap[0][0] == elem_step
    stride_bytes_256 = exact_div(elem_step * mybir.dt.size(in_ap.dtype), 256)
    assert 0 < stride_bytes_256 < 256
    _in_ap = ns.lower_ap_dma(in_ap, for_custom_bir_dma=True)
    return ns.add_instruction(
        mybir.InstDMAGatherAnt(
            name=ns.bass.get_next_instruction_name(),
            ins=[*_in_ap, ns.lower_ap(idxs_ap),
                 ns.lower_val_access(ns.to_reg(num_idxs))],
            outs=[ns.lower_ap(out_ap)],
            transpose=False, num_idxs=num_idxs, elem_size=elem_size,
            stride_bytes_256=stride_bytes_256, gen_mode=0,
            single_packet=False, queue_num=queue_num,
            sbuf_tokens_per_rank=0, sbuf_free_dim_per_rank=0,
            sbuf_free_dim_pad_per_rank=0, sbuf_byte_offset=0))


# --------------------------------------------------------------------------
# host-side graph preprocessing (pure index work, unchanged from baseline)
# --------------------------------------------------------------------------

def _preprocess(edge_index):
    src = np.concatenate([np.asarray(edge_index[0]), np.arange(N)])
    dst = np.concatenate([np.asarray(edge_index[1]), np.arange(N)])
    deg = np.bincount(dst, minlength=N)

    # permutation: per core, nodes sorted by degree desc
    pos = np.empty(N, np.int64)
    perm_nodes = np.empty(NROWS, np.int64)   # table row -> node id (or -1)
    perm_nodes.fill(-1)
    for c in range(NCORES):
        ids = np.arange(c * NPC, (c + 1) * NPC)
        order = np.argsort(-deg[ids], kind="stable")
        pos[ids[order]] = c * MPC + np.arange(NPC)
        perm_nodes[c * MPC:c * MPC + NPC] = ids[order]

    srcpos = pos[src]
    dstpos = pos[dst]

    cores = []
    counts = np.zeros((NCORES, NT, NWIN), np.int64)
    per_core = []
    for c in range(NCORES):
        m = (dst >= c * NPC) & (dst < (c + 1) * NPC)
        sp = srcpos[m]
        rank = dstpos[m] - c * MPC
        t = rank // 128
        w = sp // WINR
        per_core.append((sp, rank, t, w))
        np.add.at(counts[c], (t, w), 1)
    stc = counts.max(0)                              # [NT, NWIN]
    tile_load = stc.sum(1)
    bmap = np.zeros(NT, np.int64)
    acc = 0
    b = 0
    for t in range(NT):
        if acc and acc + tile_load[t] > BATCH_EDGES:
            b += 1
            acc = 0
        bmap[t] = b
        acc += tile_load[t]
    NBAT = int(bmap[-1]) + 1
    btiles = [list(np.where(bmap == bb)[0]) for bb in range(NBAT)]
    toff = np.zeros((NT, NWIN), np.int64)
    gsz = np.zeros((NBAT, NWIN), np.int64)
    for bb in range(NBAT):
        for w in range(NWIN):
            off = 0
            for t in btiles[bb]:
                toff[t, w] = off
                off += stc[t, w]
            gsz[bb, w] = off
    G = np.maximum((gsz + 127) // 128, 1)            # [NBAT, NWIN] slabs
    Q = G * 128
    qoff = np.zeros((NBAT, NWIN), np.int64)
    goff = np.zeros((NBAT, NWIN), np.int64)
    acc_q = 0
    for bb in range(NBAT):
        for w in range(NWIN):
            qoff[bb, w] = acc_q
            goff[bb, w] = acc_q // 128
            acc_q += Q[bb, w]
    TOTQ = acc_q
    TOTG = TOTQ // 128

    for c in range(NCORES):
        sp, rank, t, w = per_core[c]
        b = bmap[t]
        order = np.lexsort((rank, w, t))
        sp, rank, t, w, b = (sp[order], rank[order], t[order], w[order],
                             b[order])
        gid = t * NWIN + w
        gstart = np.searchsorted(gid, np.arange(NT * NWIN), side="left")
        within = np.arange(len(gid)) - gstart[gid]
        q = qoff[b, w] + toff[t, w] + within
        cores.append({"sp": sp, "rank": rank, "b": b, "w": w, "q": q})

    # union matmul schedule, merged per (b, t, w, j) with a band range.
    JMAX = TOTQ // 128 + 1
    keysets = []
    for c in range(NCORES):
        d = cores[c]
        j = (d["q"] - qoff[d["b"], d["w"]]) // 128
        t = d["rank"] // 128
        a = (d["rank"] % 128) // 32
        key = (t * NWIN + d["w"]) * JMAX + j
        keysets.append((key, a))
        d["j"] = j
        d["t"] = t
        d["key"] = key
    allk = np.concatenate([k for k, _ in keysets])
    alla = np.concatenate([a for _, a in keysets])
    ukeys, inv = np.unique(allk, return_inverse=True)
    TOTB = len(ukeys)
    amin = np.full(TOTB, 4, np.int64)
    amax = np.full(TOTB, -1, np.int64)
    np.minimum.at(amin, inv, alla)
    np.maximum.at(amax, inv, alla)
    ecol = np.where(amin == amax, amin,
                    np.where((amin == 0) & (amax == 1), 0,
                             np.where((amin == 2) & (amax == 3), 2, 0)))
    ewid = np.where(amin == amax, 1,
                    np.where((amin == 0) & (amax == 1), 2,
                             np.where((amin == 2) & (amax == 3), 2, 4)))
    soff = np.concatenate([[0], np.cumsum(ewid)])   # block col offsets (32u)
    uj = ukeys % JMAX
    r1 = ukeys // JMAX
    uw = r1 % NWIN
    ut = r1 // NWIN
    ub = bmap[ut]
    sched = {"b": ub, "t": ut, "w": uw, "j": uj, "col": ecol, "wid": ewid,
             "soff": soff, "n": TOTB, "totw": int(soff[-1])}

    for c in range(NCORES):
        d = cores[c]
        ent = np.searchsorted(ukeys, d["key"])
        d["ent"] = ent
        d["k"] = d["q"] % 128
        d["scol"] = d["rank"] % 128 - ecol[ent] * 32

    meta = {"G": G, "Q": Q, "qoff": qoff, "goff": goff, "TOTQ": TOTQ,
            "TOTG": TOTG, "sched": sched, "pos": pos, "NBAT": NBAT,
            "btiles": btiles, "perm_nodes": perm_nodes, "cores": cores}
    return meta


def _build_idx_and_s(meta):
    """Per-core gather index arrays (int16 wrapped) and fp8 S blocks."""
    TOTQ = meta["TOTQ"]
    idx_all, s_all, streams = [], [], []
    for c in range(NCORES):
        d = meta["cores"][c]
        flat = np.zeros(TOTQ, np.int16)
        loc = d["sp"] - d["w"] * WINR
        flat[d["q"]] = loc.astype(np.int16)
        resh = flat.reshape(TOTQ // 16, 16).T          # [16, TOTQ/16]
        idxw = np.tile(resh, (IDXR // 16, 1)).copy()   # [IDXR, TOTQ/16]
        idx_all.append(idxw)

        soff = meta["sched"]["soff"]
        totw = meta["sched"]["totw"]
        S = np.zeros((128, totw * 32), ml_dtypes.float8_e4m3)
        S[d["k"], soff[d["ent"]] * 32 + d["scol"]] = 1.0
        s_all.append(S)

        streams.append((d["q"] % 128, d["q"] // 128, d["rank"]))
    return idx_all, s_all, streams


def _expand_stream(stream, r_core, width, totg):
    """r_core [MPC, width] f32 -> per-position [128, totg, width] f16."""
    p, g, rank = stream
    out = np.zeros((128, int(totg), width), np.float16)
    out[p, g, :] = r_core[rank, :width].astype(np.float16)
    return out


# --------------------------------------------------------------------------
# launch builders
# --------------------------------------------------------------------------

def _new_nc():
    return bacc.Bacc("TRN2", target_bir_lowering=False, debug=False,
                     enable_asserts=False, num_devices=NCORES)


def _build_launch1():
    nc = _new_nc()
    xs_d = nc.dram_tensor("xs", [IN, MPC], F16, kind="ExternalInput")
    wc_d = nc.dram_tensor("wc", [IN, 80], F16, kind="ExternalInput")
    t1_d = nc.dram_tensor("t1s", [MPC, TB1], F8, kind="ExternalOutput")
    r1_d = nc.dram_tensor("r1", [MPC, 8], F32, kind="ExternalOutput")
    SLAB = SLAB1
    with tile.TileContext(nc) as tc:
        with tc.tile_pool(name="w", bufs=1) as wp, \
             tc.tile_pool(name="x", bufs=3) as xp, \
             tc.tile_pool(name="o", bufs=3) as op, \
             tc.tile_pool(name="ps", bufs=4, space="PSUM") as pp:
            wc_sb = wp.tile([128, 2, 80], F16)
            nc.sync.dma_start(wc_sb[:, 0, :], wc_d.ap()[0:128, :])
            nc.sync.dma_start(wc_sb[:, 1, :], wc_d.ap()[128:256, :])
            for s in range(NT // SLAB):
                cols = slice(s * SLAB * 128, (s + 1) * SLAB * 128)
                xt0 = xp.tile([128, SLAB * 128], F16, tag="xt0")
                xt1 = xp.tile([128, SLAB * 128], F16, tag="xt1")
                nc.sync.dma_start(xt0[:], xs_d.ap()[0:128, cols])
                nc.sync.dma_start(xt1[:], xs_d.ap()[128:256, cols])
                tout = op.tile([128, SLAB, TB1], F8, tag="tout")
                ex = op.tile([128, SLAB, 16], F32, tag="ex")
                rout = op.tile([128, SLAB, 8], F32, tag="rout")
                for i in range(SLAB):
                    ps = pp.tile([128, 80], F32)
                    nc.tensor.matmul(ps[:], lhsT=xt0[:, i * 128:(i + 1) * 128],
                                     rhs=wc_sb[:, 0, :], start=True, stop=False)
                    nc.tensor.matmul(ps[:], lhsT=xt1[:, i * 128:(i + 1) * 128],
                                     rhs=wc_sb[:, 1, :], start=False, stop=True)
                    nc.vector.tensor_copy(tout[:, i, 0:64], ps[:, 0:64])
                    nc.vector.tensor_copy(ex[:, i, :], ps[:, 64:80])
                tv = tout[:, :, 64:96].bitcast(F16)      # [128, SLAB, 16]
                nc.scalar.activation(out=tv[:, :, 0:8], in_=ex[:, :, 0:8],
                                     func=AF.Exp)
                nc.scalar.activation(out=tv[:, :, 8:16], in_=ex[:, :, 0:8],
                                     func=AF.Exp, scale=0.2)
                nc.scalar.activation(out=rout[:], in_=ex[:, :, 8:16],
                                     func=AF.Exp, scale=-0.8)
                rows = slice(s * SLAB * 128, (s + 1) * SLAB * 128)
                nc.scalar.dma_start(
                    t1_d.ap()[rows, :].rearrange("(i p) f -> p i f", p=128),
                    tout[:])
                nc.scalar.dma_start(
                    r1_d.ap()[rows, :].rearrange("(i p) f -> p i f", p=128),
                    rout[:])
    nc.compile()
    return nc


def _batch_geometry(meta):
    G, qoff = meta["G"], meta["qoff"]
    sched = meta["sched"]
    soff = sched["soff"]
    NBAT = meta["NBAT"]
    sb = sched["b"]
    blo = np.searchsorted(sb, np.arange(NBAT))
    bhi = np.searchsorted(sb, np.arange(NBAT), side="right")
    slo = [int(soff[blo[b]]) for b in range(NBAT)]
    shi = [int(soff[bhi[b]]) for b in range(NBAT)]
    nw32max = max(1, max(shi[b] - slo[b] for b in range(NBAT)))
    qb_lo = [int(qoff[b, 0]) for b in range(NBAT)]
    qb_hi = [int(qoff[b, NWIN - 1] + G[b, NWIN - 1] * 128)
             for b in range(NBAT)]
    qbmax = max(qb_hi[b] - qb_lo[b] for b in range(NBAT))
    ent_by_t = {}
    for i in range(sched["n"]):
        ent_by_t.setdefault(int(sched["t"][i]), []).append(i)
    return blo, bhi, slo, shi, nw32max, qb_lo, qb_hi, qbmax, ent_by_t


def _emit_batches(nc, meta, pools, tab_ap, idx_d, s_d, re_d, elem, estep,
                  rwidth, mwidth, edge_ops, tile_out, batch_out=None):
    """Shared batch loop: gathers, edge ops, banded S matmuls.

    edge_ops(Gs, rs, el, msg) fills msg [128, g, mwidth];
    tile_out(t, ps) consumes the per-tile PSUM accumulator;
    batch_out(t0, t1) runs after each batch's tiles [t0, t1) complete."""
    G, qoff, goff = meta["G"], meta["qoff"], meta["goff"]
    sched = meta["sched"]
    sw, sj = sched["w"], sched["j"]
    scol, swid, soff = sched["col"], sched["wid"], sched["soff"]
    NBAT = meta["NBAT"]
    btiles = meta["btiles"]
    blo, bhi, slo, shi, nw32max, qb_lo, qb_hi, qbmax, ent_by_t = \
        _batch_geometry(meta)
    mp, gp, wkp, ppA, zrow = pools

    for b in range(NBAT):
        nw32 = max(shi[b] - slo[b], 1)
        ssb = mp.tile([128, nw32max, 32], F8, tag="s", bufs=2)
        if shi[b] > slo[b]:
            nc.sync.dma_start(
                ssb[:, 0:nw32, :],
                s_d.ap()[:, slo[b] * 32:shi[b] * 32]
                .rearrange("p (n c) -> p n c", c=32))
        nq = qb_hi[b] - qb_lo[b]
        idx_sb = mp.tile([IDXR, qbmax // 16], I16, tag="idx", bufs=2)
        nc.sync.dma_start(idx_sb[:, 0:nq // 16],
                          idx_d.ap()[:, qb_lo[b] // 16:qb_hi[b] // 16])
        slabs = {}
        for w in range(NWIN):
            g = int(G[b, w])
            q0 = int(qoff[b, w]) - qb_lo[b]
            g0 = int(goff[b, w])
            Gs = gp.tile([128, g, elem], tab_ap.dtype, tag="G", bufs=4)
            win0 = w * WINR
            win1 = min(win0 + WINR, NROWS)
            for g1 in range(0, g, GSPLIT):
                g2 = min(g1 + GSPLIT, g)
                nn = (g2 - g1) * 128
                _dma_gather_raw(
                    nc.gpsimd, Gs[:, g1:g2, :],
                    tab_ap[win0:win1, 0:elem],
                    idx_sb[:, (q0 + g1 * 128) // 16:(q0 + g2 * 128) // 16],
                    nn, elem, estep)
            rs = gp.tile([128, g, rwidth], F16, tag="rs", bufs=3)
            nc.sync.dma_start(
                rs[:], re_d.ap()[:, g0 * rwidth:(g0 + g) * rwidth]
                .rearrange("p (g r) -> p g r", r=rwidth))
            msg = wkp.tile([128, g, mwidth], F16, tag="msg", bufs=5)
            el = wkp.tile([128, g, rwidth], F16, tag="el", bufs=3)
            edge_ops(Gs, rs, el, msg)
            slabs[w] = msg
        for t in btiles[b]:
            ents = ent_by_t.get(t, [])
            ps = ppA.tile([128, mwidth], F32, tag="ps")
            nc.tensor.matmul(ps[:], lhsT=zrow[:], rhs=zrow[:, 0:mwidth],
                             start=True, stop=False, skip_group_check=True)
            for n, i in enumerate(ents):
                w, j = int(sw[i]), int(sj[i])
                col, wid = int(scol[i]), int(swid[i])
                so = int(soff[i]) - slo[b]
                nc.tensor.matmul(
                    ps[col * 32:(col + wid) * 32, :],
                    lhsT=ssb[:, so:so + wid, :]
                    .rearrange("p n c -> p (n c)"),
                    rhs=slabs[w][:, j, :],
                    start=False, stop=(n == len(ents) - 1),
                    tile_position=(0, col * 32),
                    skip_group_check=True)
            tile_out(t, ps)
        if batch_out is not None:
            batch_out(btiles[b][0], btiles[b][-1] + 1)


def _build_launch2(meta):
    nc = _new_nc()
    t1_d = nc.dram_tensor("t1", [NROWS, 256], F8, kind="ExternalInput")
    idx_d = nc.dram_tensor("idx", [IDXR, meta["TOTQ"] // 16], I16,
                           kind="ExternalInput")
    s_d = nc.dram_tensor("sall", [128, meta["sched"]["totw"] * 32], F8,
                         kind="ExternalInput")
    re_d = nc.dram_tensor("re1", [128, meta["TOTG"] * 8], F16,
                          kind="ExternalInput")
    w2_d = nc.dram_tensor("w2e", [64, 18], F16, kind="ExternalInput")
    id_d = nc.dram_tensor("idm", [128, 128], F16, kind="ExternalInput")
    t2_d = nc.dram_tensor("t2s", [MPC, TW2], F16, kind="ExternalOutput")
    r2_d = nc.dram_tensor("r2", [MPC, 1], F32, kind="ExternalOutput")

    with tile.TileContext(nc) as tc:
        with tc.tile_pool(name="res", bufs=1) as rp, \
             tc.tile_pool(name="m", bufs=1) as mp, \
             tc.tile_pool(name="g", bufs=1) as gp, \
             tc.tile_pool(name="wk", bufs=1) as wkp, \
             tc.tile_pool(name="ep", bufs=1) as ep, \
             tc.tile_pool(name="zi", bufs=3) as zp, \
             tc.tile_pool(name="psA", bufs=4, space="PSUM") as ppA, \
             tc.tile_pool(name="psB", bufs=2, space="PSUM") as ppB:
            zrow = rp.tile([1, 128], F16)
            nc.vector.memset(zrow[:], 0.0)
            w2_sb = rp.tile([64, 18], F16)
            nc.sync.dma_start(w2_sb[:], w2_d.ap())
            idm = rp.tile([128, 128], F16)
            nc.sync.dma_start(idm[:], id_d.ap())
            ybuf = rp.tile([128, NT, 72], F16)
            t2t = rp.tile([128, NT, TW2], F16)
            pbuf = rp.tile([128, NT, 2], F32)
            r2sb = rp.tile([128, NT], F32)

            def edge_ops(Gs, rs, el, msg):
                g = Gs.shape[1]
                es_v = Gs[:, :, 64:80].bitcast(F16)
                e02_v = Gs[:, :, 80:96].bitcast(F16)
                nc.vector.tensor_tensor(out=el[:], in0=e02_v, in1=rs[:],
                                        op=ALU.mult)
                nc.vector.tensor_tensor(out=el[:], in0=es_v, in1=el[:],
                                        op=ALU.max)
                nc.vector.tensor_tensor(
                    out=msg[:, :, 0:64].rearrange("p g (h c) -> p g h c", h=8),
                    in0=Gs[:, :, 0:64].rearrange("p g (h c) -> p g h c", h=8),
                    in1=el[:].to_broadcast([128, g, 8, 8]), op=ALU.mult)
                nc.vector.tensor_copy(msg[:, :, 64:72], el[:])

            def tile_out(t, ps):
                nc.scalar.copy(ybuf[:, t, :], ps[:])

            def batch_out(t0, t1):
                nt = t1 - t0
                yb = ybuf[:, t0:t1, :]
                rec = ep.tile([128, NTB, 8], F32, tag="rec", bufs=2)
                nc.vector.tensor_scalar_add(rec[:, 0:nt, :],
                                            yb[:, :, 64:72], EPS)
                nc.vector.reciprocal(rec[:, 0:nt, :], rec[:, 0:nt, :])
                y16 = ep.tile([128, NTB, 64], F16, tag="y16", bufs=2)
                nc.vector.tensor_tensor(
                    out=y16[:, 0:nt, :]
                    .rearrange("p t (h c) -> p t h c", h=8),
                    in0=yb[:, :, 0:64].rearrange("p t (h c) -> p t h c", h=8),
                    in1=rec[:, 0:nt, :].to_broadcast([128, nt, 8, 8]),
                    op=ALU.mult)
                yn = ep.tile([128, NTB, 64], F16, tag="yn", bufs=2)
                nc.vector.tensor_scalar_min(yn[:, 0:nt, :], y16[:, 0:nt, :],
                                            0.0)
                nc.scalar.activation(out=yn[:, 0:nt, :], in_=yn[:, 0:nt, :],
                                     func=AF.Exp)
                nc.vector.tensor_scalar_add(yn[:, 0:nt, :], yn[:, 0:nt, :],
                                            -1.0)
                elu = ep.tile([128, NTB, 64], F16, tag="elu", bufs=2)
                nc.vector.tensor_tensor(out=elu[:, 0:nt, :],
                                        in0=y16[:, 0:nt, :],
                                        in1=yn[:, 0:nt, :], op=ALU.max)
                # z = elu @ [W2 | w2a | w2d] per tile via PE transpose
                for k0 in range(0, nt, 8):
                    nz = min(8, nt - k0)
                    zacc = ppB.tile([128, 8, 32], F32, tag="zacc", bufs=2)
                    for k in range(nz):
                        tp = ppB.tile([64, 128], F16, tag="tp", bufs=2)
                        nc.tensor.transpose(tp[:], elu[:, k0 + k, :], idm[:])
                        zin = zp.tile([64, 128], F16, tag="zin")
                        nc.scalar.copy(zin[:], tp[:])
                        nc.tensor.matmul(zacc[:, k, 0:18], lhsT=zin[:],
                                         rhs=w2_sb[:], start=True, stop=True)
                    tt = t0 + k0
                    nc.vector.tensor_copy(t2t[:, tt:tt + nz, 0:16],
                                          zacc[:, 0:nz, 0:16])
                    nc.vector.tensor_copy(pbuf[:, tt:tt + nz, :],
                                          zacc[:, 0:nz, 16:18])

            NTB = max(len(bt) for bt in meta["btiles"])
            _emit_batches(nc, meta, (mp, gp, wkp, ppA, zrow), t1_d.ap(),
                          idx_d, s_d, re_d, TB1, 256, 8, 72,
                          edge_ops, tile_out, batch_out)

            nc.scalar.activation(out=t2t[:, :, 16:17], in_=pbuf[:, :, 0:1],
                                 func=AF.Exp)
            nc.scalar.activation(out=t2t[:, :, 17:18], in_=pbuf[:, :, 0:1],
                                 func=AF.Exp, scale=0.2)
            nc.scalar.activation(out=r2sb[:], in_=pbuf[:, :, 1:2],
                                 func=AF.Exp, scale=-0.8)
            nc.scalar.dma_start(
                t2_d.ap().rearrange("(t p) f -> p t f", p=128), t2t[:])
            nc.scalar.dma_start(
                r2_d.ap().rearrange("(t p) o -> p (t o)", p=128), r2sb[:])
    nc.compile()
    return nc


def _build_launch3(meta):
    nc = _new_nc()
    t2_d = nc.dram_tensor("t2", [NROWS, 128], F16, kind="ExternalInput")
    idx_d = nc.dram_tensor("idx", [IDXR, meta["TOTQ"] // 16], I16,
                           kind="ExternalInput")
    s_d = nc.dram_tensor("sall", [128, meta["sched"]["totw"] * 32], F8,
                         kind="ExternalInput")
    re_d = nc.dram_tensor("re2", [128, meta["TOTG"]], F16,
                          kind="ExternalInput")
    o_d = nc.dram_tensor("o", [MPC, 16], F32, kind="ExternalOutput")

    with tile.TileContext(nc) as tc:
        with tc.tile_pool(name="res", bufs=1) as rp, \
             tc.tile_pool(name="m", bufs=1) as mp, \
             tc.tile_pool(name="g", bufs=1) as gp, \
             tc.tile_pool(name="wk", bufs=1) as wkp, \
             tc.tile_pool(name="ep", bufs=1) as ep, \
             tc.tile_pool(name="psA", bufs=4, space="PSUM") as ppA:
            zrow = rp.tile([1, 128], F16)
            nc.vector.memset(zrow[:], 0.0)
            obuf = rp.tile([128, NT, 17], F32)

            def edge_ops(Gs, rs, el, msg):
                g = Gs.shape[1]
                nc.vector.tensor_tensor(out=el[:], in0=Gs[:, :, 17:18],
                                        in1=rs[:], op=ALU.mult)
                nc.vector.tensor_tensor(out=el[:], in0=Gs[:, :, 16:17],
                                        in1=el[:], op=ALU.max)
                nc.vector.tensor_tensor(
                    out=msg[:, :, 0:16], in0=Gs[:, :, 0:16],
                    in1=el[:].rearrange("p g o -> p (g o)")
                    .to_broadcast([128, g, 16]), op=ALU.mult)
                nc.vector.tensor_copy(msg[:, :, 16:17], el[:])

            o16 = rp.tile([128, NT, 16], F32)
            ssum = rp.tile([128, NT, 1], F32)

            def tile_out(t, ps):
                nc.scalar.copy(obuf[:, t, :], ps[:])

            def batch_out(t0, t1):
                nt = t1 - t0
                ob = obuf[:, t0:t1, :]
                o1 = o16[:, t0:t1, :]
                rec = ep.tile([128, NTB, 1], F32, tag="rec", bufs=2)
                nc.vector.tensor_scalar_add(rec[:, 0:nt, :],
                                            ob[:, :, 16:17], EPS)
                nc.vector.reciprocal(rec[:, 0:nt, :], rec[:, 0:nt, :])
                nc.vector.tensor_tensor(
                    out=o1[:], in0=ob[:, :, 0:16],
                    in1=rec[:, 0:nt, :].rearrange("p t o -> p (t o)")
                    .to_broadcast([128, nt, 16]), op=ALU.mult)
                mx = ep.tile([128, NTB, 1], F32, tag="mx", bufs=2)
                nc.vector.tensor_reduce(out=mx[:, 0:nt, :], in_=o1[:],
                                        axis=AX.X, op=ALU.max)
                nc.vector.tensor_tensor(
                    out=o1[:], in0=o1[:],
                    in1=mx[:, 0:nt, :].rearrange("p t o -> p (t o)")
                    .to_broadcast([128, nt, 16]), op=ALU.subtract)
                es = ep.tile([128, NTB, 16], F16, tag="es", bufs=2)
                nc.scalar.activation(out=es[:, 0:nt, :], in_=o1[:],
                                     func=AF.Exp)
                nc.vector.tensor_reduce(out=ssum[:, t0:t1, :],
                                        in_=es[:, 0:nt, :], axis=AX.X,
                                        op=ALU.add)

            NTB = max(len(bt) for bt in meta["btiles"])
            _emit_batches(nc, meta, (mp, gp, wkp, ppA, zrow), t2_d.ap(),
                          idx_d, s_d, re_d, TW2, 128, 1, 17,
                          edge_ops, tile_out, batch_out)

            lns = ep.tile([128, NT, 1], F32, tag="lns")
            nc.scalar.activation(out=lns[:], in_=ssum[:], func=AF.Ln)
            nc.vector.tensor_tensor(
                out=o16[:], in0=o16[:],
                in1=lns[:].rearrange("p t o -> p (t o)")
                .to_broadcast([128, NT, 16]), op=ALU.subtract)
            nc.scalar.dma_start(
                o_d.ap().rearrange("(t p) f -> p t f", p=128), o16[:])
    nc.compile()
    return nc


# --------------------------------------------------------------------------
# the kernel
# --------------------------------------------------------------------------

def kernel(x, edge_index, W1, a_src1, a_dst1, b1, W2, a_src2, a_dst2, b2):
    x = np.asarray(x, np.float32)
    edge_index = np.asarray(edge_index)
    W1 = np.asarray(W1, np.float32)
    W2 = np.asarray(W2, np.float32)
    a_src1 = np.asarray(a_src1, np.float32)
    a_dst1 = np.asarray(a_dst1, np.float32)
    a_src2 = np.asarray(a_src2, np.float32)
    a_dst2 = np.asarray(a_dst2, np.float32)

    key = edge_index.tobytes()[:4096]
    if _CACHE.get("key") != key:
        meta = _preprocess(edge_index)
        idx_all, s_all, streams = _build_idx_and_s(meta)
        _CACHE.update(key=key, meta=meta, idx_all=idx_all, s_all=s_all,
                      streams=streams,
                      nc1=_build_launch1(), nc2=_build_launch2(meta),
                      nc3=_build_launch3(meta))
    meta = _CACHE["meta"]
    idx_all, s_all, streams = (_CACHE["idx_all"], _CACHE["s_all"],
                               _CACHE["streams"])

    # weight packing
    W1r = W1.reshape(IN, HEADS, HID)
    B1 = np.einsum("khc,hc->kh", W1r, a_src1)        # [256, 8]
    C1 = np.einsum("khc,hc->kh", W1r, a_dst1)
    wc = np.concatenate([W1, B1, C1], 1).astype(np.float16)   # [256, 80]
    w2a = W2 @ a_src2[0]                              # [64]
    w2d = W2 @ a_dst2[0]
    w2e = np.concatenate([W2, w2a[:, None], w2d[:, None]],
                         1).astype(np.float16)        # [64, 18]
    idm = np.eye(128, dtype=np.float16)

    # launch 1: build T1 slices
    perm = meta["perm_nodes"]
    xT = np.zeros((IN, NROWS), np.float16)
    real = perm >= 0
    xT[:, real] = x[perm[real]].astype(np.float16).T
    in1 = [{"xs": np.ascontiguousarray(xT[:, c * MPC:(c + 1) * MPC]),
            "wc": wc} for c in range(NCORES)]
    r1_res = bass_utils.run_bass_kernel_spmd(
        _CACHE["nc1"], in1, core_ids=list(range(NCORES)), trace=TRACE)
    T1 = np.zeros((NROWS, 256), np.uint8)
    for c in range(NCORES):
        T1[c * MPC:(c + 1) * MPC, 0:TB1] = \
            np.asarray(r1_res.results[c]["t1s"]).view(np.uint8)
    T1 = T1.view(ml_dtypes.float8_e4m3)

    # launch 2: layer-1 message passing -> T2 slices
    in2 = []
    for c in range(NCORES):
        re1 = _expand_stream(streams[c], np.asarray(r1_res.results[c]["r1"]),
                             8, meta["TOTG"])
        in2.append({"t1": T1, "idx": idx_all[c], "sall": s_all[c],
                    "re1": re1.reshape(128, -1), "w2e": w2e, "idm": idm})
    r2_res = bass_utils.run_bass_kernel_spmd(
        _CACHE["nc2"], in2, core_ids=list(range(NCORES)), trace=TRACE)
    T2 = np.zeros((NROWS, 128), np.float16)
    for c in range(NCORES):
        T2[c * MPC:(c + 1) * MPC, 0:TW2] = \
            np.asarray(r2_res.results[c]["t2s"])

    # launch 3: layer-2 + log_softmax
    in3 = []
    for c in range(NCORES):
        re2 = _expand_stream(streams[c], np.asarray(r2_res.results[c]["r2"]),
                             1, meta["TOTG"])
        in3.append({"t2": T2, "idx": idx_all[c], "sall": s_all[c],
                    "re2": re2.reshape(128, -1)})
    r3_res = bass_utils.run_bass_kernel_spmd(
        _CACHE["nc3"], in3, core_ids=list(range(NCORES)), trace=TRACE)
    o_all = np.concatenate([np.asarray(r3_res.results[c]["o"])
                            for c in range(NCORES)], 0)

    out = o_all[meta["pos"][np.arange(N)]].astype(np.float32)
    _CACHE["exec_ns"] = [r.exec_time_ns for r in (r1_res, r2_res, r3_res)]
    return out


def predict_ns():
    """Cost-model (TimelineSim) per-launch predictions for cached programs."""
    from concourse.timeline_sim import TimelineSim
    out = []
    for k in ("nc1", "nc2", "nc3"):
        out.append(TimelineSim(_CACHE[k]).simulate())
    return out


# revision 26
# speedup vs baseline: 2.1684x; 1.0735x over previous
"""2-layer GAT on 8 trn2 NeuronCores (Bass/Tile).

Node-partitioned (12500/core, padded 12544), edges assigned by destination,
per-edge dma_gather of source-node table rows, segment softmax via the
factorization  exp(leaky(s+a)) = A * max(exp(s), exp(0.2 s) * exp(-0.8 a))
(per-dst factor A cancels), segment sums via banded one-hot S matmuls on
the PE.  Three SPMD launches with host halo exchange between them:

  1. "build":  h1 = x @ W1 + attention projections -> per-node table T1
     rows of 96B: [h fp8e4 x64 | exp(s) fp16 x8 | exp(0.2 s) fp16 x8],
     256B row stride in DRAM; r1 = exp(-0.8 a) per node.
  2. "layer1": per-edge 96B gathers from T1 (cost-model: 8.5 ns/descriptor
     vs 22.8 at 256B), edge softmax, banded S matmuls -> per-node epilogue
     (batched: softmax-normalize, ELU, z = elu @ [W2|w2a|w2d] via PE
     transpose) -> T2 rows of 36B: [z fp16 x16 | exp(s2) | exp(0.2 s2)].
  3. "layer2": 36B gathers from T2, 17-wide messages [el*z | el], banded
     S matmuls, batched log_softmax epilogue (single Ln table load).

Folding W2 into the T2 table (z instead of the 64-wide hidden vector) cuts
layer-2 gather/message/matmul width 4x and removes the output-head matmul.
"""

import numpy as np
import ml_dtypes

import concourse.bacc as bacc
import concourse.tile as tile
import concourse.mybir as mybir
from concourse import bass_utils
from concourse.bass import ap_utils, exact_div, MemorySpace

F32 = mybir.dt.float32
F16 = mybir.dt.float16
F8 = mybir.dt.float8e4
I16 = mybir.dt.int16
AF = mybir.ActivationFunctionType
ALU = mybir.AluOpType
AX = mybir.AxisListType

# problem constants (hardcoded per the task statement)
NCORES = 8
N = 100000
IN = 256
HID = 8
HEADS = 8
OUT = 16
NEG = 0.2
NPC = 12500            # real nodes per core
MPC = 12544            # padded nodes per core (98 * 128)
NT = MPC // 128        # 98 dst tiles per core
BATCH_EDGES = 24576    # shared edge budget per batch
NROWS = NCORES * MPC   # 100352 table rows
WINR = 32512           # gather window rows (int16-safe)
NWIN = (NROWS + WINR - 1) // WINR  # 4
EPS = 1e-16
SLAB1 = 14             # launch-1 chunks per slab (must divide NT)
TB1 = 96               # T1 gathered bytes: 64 fp8 h + 16 fp16 exps
TW2 = 18               # T2 row width in fp16: 16 z + 2 exps

_CACHE = {}
TRACE = False
GSPLIT = 96            # max slabs (x128 idxs) per dma_gather call
IDXR = 32              # idx tile partition replication (ucode reads <=32)


# --------------------------------------------------------------------------
# raw gather: InstDMAGatherAnt without the elem%256B assert (the non-
# transpose ucode path supports any elem size; only the row STRIDE must be
# a multiple of 256B)
# --------------------------------------------------------------------------

def _dma_gather_raw(ns, out_ap, in_ap, idxs_ap, num_idxs, elem_size,
                    elem_step, queue_num=0):
    assert idxs_ap.dtype == mybir.dt.int16
    assert in_ap.dtype == out_ap.dtype
    assert in_ap.space == MemorySpace.DRAM
    assert ap_utils.ap_is_contiguous(in_ap.ap[1:])
    assert ap_utils.ap_is_contiguous(out_ap.ap[1:])
    assert ap_utils.ap_is_contiguous(idxs_ap.ap[1:])
    assert in_ap.ap[-1][1] == out_ap.ap[-1][1] == elem_size
    assert out_ap.ap[0][1] * out_ap.ap[1][1] == (num_idxs + 127) // 128 * 128
    assert in_ap.ap[0][0] == elem_step
    stride_bytes_256 = exact_div(elem_step * mybir.dt.size(in_ap.dtype), 256)
    assert 0 < stride_bytes_256 < 256
    _in_ap = ns.lower_ap_dma(in_ap, for_custom_bir_dma=True)
    return ns.add_instruction(
        mybir.InstDMAGatherAnt(
            name=ns.bass.get_next_instruction_name(),
            ins=[*_in_ap, ns.lower_ap(idxs_ap),
                 ns.lower_val_access(ns.to_reg(num_idxs))],
            outs=[ns.lower_ap(out_ap)],
            transpose=False, num_idxs=num_idxs, elem_size=elem_size,
            stride_bytes_256=stride_bytes_256, gen_mode=0,
            single_packet=False, queue_num=queue_num,
            sbuf_tokens_per_rank=0, sbuf_free_dim_per_rank=0,
            sbuf_free_dim_pad_per_rank=0, sbuf_byte_offset=0))


# --------------------------------------------------------------------------
# host-side graph preprocessing (pure index work, unchanged from baseline)
# --------------------------------------------------------------------------

def _preprocess(edge_index):
    src = np.concatenate([np.asarray(edge_index[0]), np.arange(N)])
    dst = np.concatenate([np.asarray(edge_index[1]), np.arange(N)])
    deg = np.bincount(dst, minlength=N)

    # permutation: per core, nodes sorted by degree desc
    pos = np.empty(N, np.int64)
    perm_nodes = np.empty(NROWS, np.int64)   # table row -> node id (or -1)
    perm_nodes.fill(-1)
    for c in range(NCORES):
        ids = np.arange(c * NPC, (c + 1) * NPC)
        order = np.argsort(-deg[ids], kind="stable")
        pos[ids[order]] = c * MPC + np.arange(NPC)
        perm_nodes[c * MPC:c * MPC + NPC] = ids[order]

    srcpos = pos[src]
    dstpos = pos[dst]

    cores = []
    counts = np.zeros((NCORES, NT, NWIN), np.int64)
    per_core = []
    for c in range(NCORES):
        m = (dst >= c * NPC) & (dst < (c + 1) * NPC)
        sp = srcpos[m]
        rank = dstpos[m] - c * MPC
        t = rank // 128
        w = sp // WINR
        per_core.append((sp, rank, t, w))
        np.add.at(counts[c], (t, w), 1)
    stc = counts.max(0)                              # [NT, NWIN]
    tile_load = stc.sum(1)
    bmap = np.zeros(NT, np.int64)
    acc = 0
    b = 0
    for t in range(NT):
        if acc and acc + tile_load[t] > BATCH_EDGES:
            b += 1
            acc = 0
        bmap[t] = b
        acc += tile_load[t]
    NBAT = int(bmap[-1]) + 1
    btiles = [list(np.where(bmap == bb)[0]) for bb in range(NBAT)]
    toff = np.zeros((NT, NWIN), np.int64)
    gsz = np.zeros((NBAT, NWIN), np.int64)
    for bb in range(NBAT):
        for w in range(NWIN):
            off = 0
            for t in btiles[bb]:
                toff[t, w] = off
                off += stc[t, w]
            gsz[bb, w] = off
    G = np.maximum((gsz + 127) // 128, 1)            # [NBAT, NWIN] slabs
    Q = G * 128
    qoff = np.zeros((NBAT, NWIN), np.int64)
    goff = np.zeros((NBAT, NWIN), np.int64)
    acc_q = 0
    for bb in range(NBAT):
        for w in range(NWIN):
            qoff[bb, w] = acc_q
            goff[bb, w] = acc_q // 128
            acc_q += Q[bb, w]
    TOTQ = acc_q
    TOTG = TOTQ // 128

    for c in range(NCORES):
        sp, rank, t, w = per_core[c]
        b = bmap[t]
        order = np.lexsort((rank, w, t))
        sp, rank, t, w, b = (sp[order], rank[order], t[order], w[order],
                             b[order])
        gid = t * NWIN + w
        gstart = np.searchsorted(gid, np.arange(NT * NWIN), side="left")
        within = np.arange(len(gid)) - gstart[gid]
        q = qoff[b, w] + toff[t, w] + within
        cores.append({"sp": sp, "rank": rank, "b": b, "w": w, "q": q})

    # union matmul schedule, merged per (b, t, w, j) with a band range.
    JMAX = TOTQ // 128 + 1
    keysets = []
    for c in range(NCORES):
        d = cores[c]
        j = (d["q"] - qoff[d["b"], d["w"]]) // 128
        t = d["rank"] // 128
        a = (d["rank"] % 128) // 32
        key = (t * NWIN + d["w"]) * JMAX + j
        keysets.append((key, a))
        d["j"] = j
        d["t"] = t
        d["key"] = key
    allk = np.concatenate([k for k, _ in keysets])
    alla = np.concatenate([a for _, a in keysets])
    ukeys, inv = np.unique(allk, return_inverse=True)
    TOTB = len(ukeys)
    amin = np.full(TOTB, 4, np.int64)
    amax = np.full(TOTB, -1, np.int64)
    np.minimum.at(amin, inv, alla)
    np.maximum.at(amax, inv, alla)
    ecol = np.where(amin == amax, amin,
                    np.where((amin == 0) & (amax == 1), 0,
                             np.where((amin == 2) & (amax == 3), 2, 0)))
    ewid = np.where(amin == amax, 1,
                    np.where((amin == 0) & (amax == 1), 2,
                             np.where((amin == 2) & (amax == 3), 2, 4)))
    soff = np.concatenate([[0], np.cumsum(ewid)])   # block col offsets (32u)
    uj = ukeys % JMAX
    r1 = ukeys // JMAX
    uw = r1 % NWIN
    ut = r1 // NWIN
    ub = bmap[ut]
    sched = {"b": ub, "t": ut, "w": uw, "j": uj, "col": ecol, "wid": ewid,
             "soff": soff, "n": TOTB, "totw": int(soff[-1])}

    for c in range(NCORES):
        d = cores[c]
        ent = np.searchsorted(ukeys, d["key"])
        d["ent"] = ent
        d["k"] = d["q"] % 128
        d["scol"] = d["rank"] % 128 - ecol[ent] * 32

    meta = {"G": G, "Q": Q, "qoff": qoff, "goff": goff, "TOTQ": TOTQ,
            "TOTG": TOTG, "sched": sched, "pos": pos, "NBAT": NBAT,
            "btiles": btiles, "perm_nodes": perm_nodes, "cores": cores}
    return meta


def _build_idx_and_s(meta):
    """Per-core gather index arrays (int16 wrapped) and fp8 S blocks."""
    TOTQ = meta["TOTQ"]
    idx_all, s_all, streams = [], [], []
    for c in range(NCORES):
        d = meta["cores"][c]
        flat = np.zeros(TOTQ, np.int16)
        loc = d["sp"] - d["w"] * WINR
        flat[d["q"]] = loc.astype(np.int16)
        resh = flat.reshape(TOTQ // 16, 16).T          # [16, TOTQ/16]
        idxw = np.tile(resh, (IDXR // 16, 1)).copy()   # [IDXR, TOTQ/16]
        idx_all.append(idxw)

        soff = meta["sched"]["soff"]
        totw = meta["sched"]["totw"]
        S = np.zeros((128, totw * 32), ml_dtypes.float8_e4m3)
        S[d["k"], soff[d["ent"]] * 32 + d["scol"]] = 1.0
        s_all.append(S)

        streams.append((d["q"] % 128, d["q"] // 128, d["rank"]))
    return idx_all, s_all, streams


def _expand_stream(stream, r_core, width, totg):
    """r_core [MPC, width] f32 -> per-position [128, totg, width] f16."""
    p, g, rank = stream
    out = np.zeros((128, int(totg), width), np.float16)
    out[p, g, :] = r_core[rank, :width].astype(np.float16)
    return out


# --------------------------------------------------------------------------
# launch builders
# --------------------------------------------------------------------------

def _new_nc():
    return bacc.Bacc("TRN2", target_bir_lowering=False, debug=False,
                     enable_asserts=False, num_devices=NCORES)


def _build_launch1():
    nc = _new_nc()
    xs_d = nc.dram_tensor("xs", [IN, MPC], F16, kind="ExternalInput")
    wc_d = nc.dram_tensor("wc", [IN, 80], F16, kind="ExternalInput")
    t1_d = nc.dram_tensor("t1s", [MPC, TB1], F8, kind="ExternalOutput")
    r1_d = nc.dram_tensor("r1", [MPC, 8], F32, kind="ExternalOutput")
    SLAB = SLAB1
    with tile.TileContext(nc) as tc:
        with tc.tile_pool(name="w", bufs=1) as wp, \
             tc.tile_pool(name="x", bufs=3) as xp, \
             tc.tile_pool(name="o", bufs=3) as op, \
             tc.tile_pool(name="ps", bufs=4, space="PSUM") as pp:
            wc_sb = wp.tile([128, 2, 80], F16)
            nc.sync.dma_start(wc_sb[:, 0, :], wc_d.ap()[0:128, :])
            nc.sync.dma_start(wc_sb[:, 1, :], wc_d.ap()[128:256, :])
            for s in range(NT // SLAB):
                cols = slice(s * SLAB * 128, (s + 1) * SLAB * 128)
                xt0 = xp.tile([128, SLAB * 128], F16, tag="xt0")
                xt1 = xp.tile([128, SLAB * 128], F16, tag="xt1")
                nc.sync.dma_start(xt0[:], xs_d.ap()[0:128, cols])
                nc.sync.dma_start(xt1[:], xs_d.ap()[128:256, cols])
                tout = op.tile([128, SLAB, TB1], F8, tag="tout")
                ex = op.tile([128, SLAB, 16], F32, tag="ex")
                rout = op.tile([128, SLAB, 8], F32, tag="rout")
                for i in range(SLAB):
                    ps = pp.tile([128, 80], F32)
                    nc.tensor.matmul(ps[:], lhsT=xt0[:, i * 128:(i + 1) * 128],
                                     rhs=wc_sb[:, 0, :], start=True, stop=False)
                    nc.tensor.matmul(ps[:], lhsT=xt1[:, i * 128:(i + 1) * 128],
                                     rhs=wc_sb[:, 1, :], start=False, stop=True)
                    nc.vector.tensor_copy(tout[:, i, 0:64], ps[:, 0:64])
                    nc.vector.tensor_copy(ex[:, i, :], ps[:, 64:80])
                tv = tout[:, :, 64:96].bitcast(F16)      # [128, SLAB, 16]
                nc.scalar.activation(out=tv[:, :, 0:8], in_=ex[:, :, 0:8],
                                     func=AF.Exp)
                nc.scalar.activation(out=tv[:, :, 8:16], in_=ex[:, :, 0:8],
                                     func=AF.Exp, scale=0.2)
                nc.scalar.activation(out=rout[:], in_=ex[:, :, 8:16],
                                     func=AF.Exp, scale=-0.8)
                rows = slice(s * SLAB * 128, (s + 1) * SLAB * 128)
                nc.scalar.dma_start(
                    t1_d.ap()[rows, :].rearrange("(i p) f -> p i f", p=128),
                    tout[:])
                nc.scalar.dma_start(
                    r1_d.ap()[rows, :].rearrange("(i p) f -> p i f", p=128),
                    rout[:])
    nc.compile()
    return nc


def _batch_geometry(meta):
    G, qoff = meta["G"], meta["qoff"]
    sched = meta["sched"]
    soff = sched["soff"]
    NBAT = meta["NBAT"]
    sb = sched["b"]
    blo = np.searchsorted(sb, np.arange(NBAT))
    bhi = np.searchsorted(sb, np.arange(NBAT), side="right")
    slo = [int(soff[blo[b]]) for b in range(NBAT)]
    shi = [int(soff[bhi[b]]) for b in range(NBAT)]
    nw32max = max(1, max(shi[b] - slo[b] for b in range(NBAT)))
    qb_lo = [int(qoff[b, 0]) for b in range(NBAT)]
    qb_hi = [int(qoff[b, NWIN - 1] + G[b, NWIN - 1] * 128)
             for b in range(NBAT)]
    qbmax = max(qb_hi[b] - qb_lo[b] for b in range(NBAT))
    ent_by_t = {}
    for i in range(sched["n"]):
        ent_by_t.setdefault(int(sched["t"][i]), []).append(i)
    return blo, bhi, slo, shi, nw32max, qb_lo, qb_hi, qbmax, ent_by_t


def _emit_batches(nc, meta, pools, tab_ap, idx_d, s_d, re_d, elem, estep,
                  rwidth, mwidth, edge_ops, tile_out, batch_out=None):
    """Shared batch loop: gathers, edge ops, banded S matmuls.

    edge_ops(Gs, rs, el, msg) fills msg [128, g, mwidth];
    tile_out(t, ps) consumes the per-tile PSUM accumulator;
    batch_out(t0, t1) runs after each batch's tiles [t0, t1) complete."""
    G, qoff, goff = meta["G"], meta["qoff"], meta["goff"]
    sched = meta["sched"]
    sw, sj = sched["w"], sched["j"]
    scol, swid, soff = sched["col"], sched["wid"], sched["soff"]
    NBAT = meta["NBAT"]
    btiles = meta["btiles"]
    blo, bhi, slo, shi, nw32max, qb_lo, qb_hi, qbmax, ent_by_t = \
        _batch_geometry(meta)
    mp, gp, wkp, ppA, zrow = pools

    border = sorted(range(NBAT), key=lambda b: qb_lo[b] - qb_hi[b])
    for b in border:
        nw32 = max(shi[b] - slo[b], 1)
        ssb = mp.tile([128, nw32max, 32], F8, tag="s", bufs=2)
        if shi[b] > slo[b]:
            nc.sync.dma_start(
                ssb[:, 0:nw32, :],
                s_d.ap()[:, slo[b] * 32:shi[b] * 32]
                .rearrange("p (n c) -> p n c", c=32))
        nq = qb_hi[b] - qb_lo[b]
        idx_sb = mp.tile([IDXR, qbmax // 16], I16, tag="idx", bufs=2)
        nc.sync.dma_start(idx_sb[:, 0:nq // 16],
                          idx_d.ap()[:, qb_lo[b] // 16:qb_hi[b] // 16])
        slabs = {}
        for w in range(NWIN):
            g = int(G[b, w])
            q0 = int(qoff[b, w]) - qb_lo[b]
            g0 = int(goff[b, w])
            Gs = gp.tile([128, g, elem], tab_ap.dtype, tag="G", bufs=6)
            win0 = w * WINR
            win1 = min(win0 + WINR, NROWS)
            for g1 in range(0, g, GSPLIT):
                g2 = min(g1 + GSPLIT, g)
                nn = (g2 - g1) * 128
                _dma_gather_raw(
                    nc.gpsimd, Gs[:, g1:g2, :],
                    tab_ap[win0:win1, 0:elem],
                    idx_sb[:, (q0 + g1 * 128) // 16:(q0 + g2 * 128) // 16],
                    nn, elem, estep)
            rs = gp.tile([128, g, rwidth], F16, tag="rs", bufs=5)
            nc.sync.dma_start(
                rs[:], re_d.ap()[:, g0 * rwidth:(g0 + g) * rwidth]
                .rearrange("p (g r) -> p g r", r=rwidth))
            msg = wkp.tile([128, g, mwidth], F16, tag="msg", bufs=5)
            el = wkp.tile([128, g, rwidth], F16, tag="el", bufs=5)
            edge_ops(Gs, rs, el, msg)
            slabs[w] = msg
        for t in btiles[b]:
            ents = ent_by_t.get(t, [])
            ps = ppA.tile([128, mwidth], F32, tag="ps")
            nc.tensor.matmul(ps[:], lhsT=zrow[:], rhs=zrow[:, 0:mwidth],
                             start=True, stop=False, skip_group_check=True)
            for n, i in enumerate(ents):
                w, j = int(sw[i]), int(sj[i])
                col, wid = int(scol[i]), int(swid[i])
                so = int(soff[i]) - slo[b]
                nc.tensor.matmul(
                    ps[col * 32:(col + wid) * 32, :],
                    lhsT=ssb[:, so:so + wid, :]
                    .rearrange("p n c -> p (n c)"),
                    rhs=slabs[w][:, j, :],
                    start=False, stop=(n == len(ents) - 1),
                    tile_position=(0, col * 32),
                    skip_group_check=True)
            tile_out(t, ps)
        if batch_out is not None:
            batch_out(btiles[b][0], btiles[b][-1] + 1)


def _build_launch2(meta):
    nc = _new_nc()
    t1_d = nc.dram_tensor("t1", [NROWS, 256], F8, kind="ExternalInput")
    idx_d = nc.dram_tensor("idx", [IDXR, meta["TOTQ"] // 16], I16,
                           kind="ExternalInput")
    s_d = nc.dram_tensor("sall", [128, meta["sched"]["totw"] * 32], F8,
                         kind="ExternalInput")
    re_d = nc.dram_tensor("re1", [128, meta["TOTG"] * 8], F16,
                          kind="ExternalInput")
    w2_d = nc.dram_tensor("w2e", [64, 18], F16, kind="ExternalInput")
    id_d = nc.dram_tensor("idm", [128, 128], F16, kind="ExternalInput")
    t2_d = nc.dram_tensor("t2s", [MPC, TW2], F16, kind="ExternalOutput")
    r2_d = nc.dram_tensor("r2", [MPC, 1], F32, kind="ExternalOutput")

    with tile.TileContext(nc) as tc:
        with tc.tile_pool(name="res", bufs=1) as rp, \
             tc.tile_pool(name="m", bufs=1) as mp, \
             tc.tile_pool(name="g", bufs=1) as gp, \
             tc.tile_pool(name="wk", bufs=1) as wkp, \
             tc.tile_pool(name="ep", bufs=1) as ep, \
             tc.tile_pool(name="zi", bufs=3) as zp, \
             tc.tile_pool(name="psA", bufs=4, space="PSUM") as ppA, \
             tc.tile_pool(name="psB", bufs=2, space="PSUM") as ppB:
            zrow = rp.tile([1, 128], F16)
            nc.vector.memset(zrow[:], 0.0)
            w2_sb = rp.tile([64, 18], F16)
            nc.sync.dma_start(w2_sb[:], w2_d.ap())
            idm = rp.tile([128, 128], F16)
            nc.sync.dma_start(idm[:], id_d.ap())
            ybuf = rp.tile([128, NT, 72], F16)
            t2t = rp.tile([128, NT, TW2], F16)
            pbuf = rp.tile([128, NT, 2], F32)
            r2sb = rp.tile([128, NT], F32)

            def edge_ops(Gs, rs, el, msg):
                g = Gs.shape[1]
                es_v = Gs[:, :, 64:80].bitcast(F16)
                e02_v = Gs[:, :, 80:96].bitcast(F16)
                nc.vector.tensor_tensor(out=el[:], in0=e02_v, in1=rs[:],
                                        op=ALU.mult)
                nc.vector.tensor_tensor(out=el[:], in0=es_v, in1=el[:],
                                        op=ALU.max)
                nc.vector.tensor_tensor(
                    out=msg[:, :, 0:64].rearrange("p g (h c) -> p g h c", h=8),
                    in0=Gs[:, :, 0:64].rearrange("p g (h c) -> p g h c", h=8),
                    in1=el[:].to_broadcast([128, g, 8, 8]), op=ALU.mult)
                nc.vector.tensor_copy(msg[:, :, 64:72], el[:])

            def tile_out(t, ps):
                nc.scalar.copy(ybuf[:, t, :], ps[:])

            def batch_out(t0, t1):
                nt = t1 - t0
                yb = ybuf[:, t0:t1, :]
                rec = ep.tile([128, NTB, 8], F32, tag="rec", bufs=2)
                nc.vector.tensor_scalar_add(rec[:, 0:nt, :],
                                            yb[:, :, 64:72], EPS)
                nc.vector.reciprocal(rec[:, 0:nt, :], rec[:, 0:nt, :])
                y16 = ep.tile([128, NTB, 64], F16, tag="y16", bufs=2)
                nc.vector.tensor_tensor(
                    out=y16[:, 0:nt, :]
                    .rearrange("p t (h c) -> p t h c", h=8),
                    in0=yb[:, :, 0:64].rearrange("p t (h c) -> p t h c", h=8),
                    in1=rec[:, 0:nt, :].to_broadcast([128, nt, 8, 8]),
                    op=ALU.mult)
                yn = ep.tile([128, NTB, 64], F16, tag="yn", bufs=2)
                nc.vector.tensor_scalar_min(yn[:, 0:nt, :], y16[:, 0:nt, :],
                                            0.0)
                nc.scalar.activation(out=yn[:, 0:nt, :], in_=yn[:, 0:nt, :],
                                     func=AF.Exp)
                nc.vector.tensor_scalar_add(yn[:, 0:nt, :], yn[:, 0:nt, :],
                                            -1.0)
                elu = ep.tile([128, NTB, 64], F16, tag="elu", bufs=2)
                nc.vector.tensor_tensor(out=elu[:, 0:nt, :],
                                        in0=y16[:, 0:nt, :],
                                        in1=yn[:, 0:nt, :], op=ALU.max)
                # z = elu @ [W2 | w2a | w2d] per tile via PE transpose
                for k0 in range(0, nt, 8):
                    nz = min(8, nt - k0)
                    zacc = ppB.tile([128, 8, 32], F32, tag="zacc", bufs=2)
                    for k in range(nz):
                        tp = ppB.tile([64, 128], F16, tag="tp", bufs=2)
                        nc.tensor.transpose(tp[:], elu[:, k0 + k, :], idm[:])
                        zin = zp.tile([64, 128], F16, tag="zin")
                        nc.scalar.copy(zin[:], tp[:])
                        nc.tensor.matmul(zacc[:, k, 0:18], lhsT=zin[:],
                                         rhs=w2_sb[:], start=True, stop=True)
                    tt = t0 + k0
                    nc.vector.tensor_copy(t2t[:, tt:tt + nz, 0:16],
                                          zacc[:, 0:nz, 0:16])
                    nc.vector.tensor_copy(pbuf[:, tt:tt + nz, :],
                                          zacc[:, 0:nz, 16:18])
                nc.scalar.activation(out=t2t[:, t0:t1, 16:17],
                                     in_=pbuf[:, t0:t1, 0:1], func=AF.Exp)
                nc.scalar.activation(out=t2t[:, t0:t1, 17:18],
                                     in_=pbuf[:, t0:t1, 0:1], func=AF.Exp,
                                     scale=0.2)
                nc.scalar.activation(out=r2sb[:, t0:t1],
                                     in_=pbuf[:, t0:t1, 1:2], func=AF.Exp,
                                     scale=-0.8)
                nc.scalar.dma_start(
                    t2_d.ap()[t0 * 128:t1 * 128, :]
                    .rearrange("(t p) f -> p t f", p=128), t2t[:, t0:t1, :])
                nc.scalar.dma_start(
                    r2_d.ap()[t0 * 128:t1 * 128, :]
                    .rearrange("(t p) o -> p (t o)", p=128), r2sb[:, t0:t1])

            NTB = max(len(bt) for bt in meta["btiles"])
            _emit_batches(nc, meta, (mp, gp, wkp, ppA, zrow), t1_d.ap(),
                          idx_d, s_d, re_d, TB1, 256, 8, 72,
                          edge_ops, tile_out, batch_out)
    nc.compile()
    return nc


def _build_launch3(meta):
    nc = _new_nc()
    t2_d = nc.dram_tensor("t2", [NROWS, 128], F16, kind="ExternalInput")
    idx_d = nc.dram_tensor("idx", [IDXR, meta["TOTQ"] // 16], I16,
                           kind="ExternalInput")
    s_d = nc.dram_tensor("sall", [128, meta["sched"]["totw"] * 32], F8,
                         kind="ExternalInput")
    re_d = nc.dram_tensor("re2", [128, meta["TOTG"]], F16,
                          kind="ExternalInput")
    o_d = nc.dram_tensor("o", [MPC, 16], F32, kind="ExternalOutput")

    with tile.TileContext(nc) as tc:
        with tc.tile_pool(name="res", bufs=1) as rp, \
             tc.tile_pool(name="m", bufs=1) as mp, \
             tc.tile_pool(name="g", bufs=1) as gp, \
             tc.tile_pool(name="wk", bufs=1) as wkp, \
             tc.tile_pool(name="ep", bufs=1) as ep, \
             tc.tile_pool(name="psA", bufs=4, space="PSUM") as ppA:
            zrow = rp.tile([1, 128], F16)
            nc.vector.memset(zrow[:], 0.0)
            obuf = rp.tile([128, NT, 17], F32)

            def edge_ops(Gs, rs, el, msg):
                g = Gs.shape[1]
                nc.vector.tensor_tensor(out=el[:], in0=Gs[:, :, 17:18],
                                        in1=rs[:], op=ALU.mult)
                nc.vector.tensor_tensor(out=el[:], in0=Gs[:, :, 16:17],
                                        in1=el[:], op=ALU.max)
                nc.vector.tensor_tensor(
                    out=msg[:, :, 0:16], in0=Gs[:, :, 0:16],
                    in1=el[:].rearrange("p g o -> p (g o)")
                    .to_broadcast([128, g, 16]), op=ALU.mult)
                nc.vector.tensor_copy(msg[:, :, 16:17], el[:])

            o16 = rp.tile([128, NT, 16], F32)
            ssum = rp.tile([128, NT, 1], F32)

            def tile_out(t, ps):
                nc.scalar.copy(obuf[:, t, :], ps[:])

            def batch_out(t0, t1):
                nt = t1 - t0
                ob = obuf[:, t0:t1, :]
                o1 = o16[:, t0:t1, :]
                rec = ep.tile([128, NTB, 1], F32, tag="rec", bufs=2)
                nc.vector.tensor_scalar_add(rec[:, 0:nt, :],
                                            ob[:, :, 16:17], EPS)
                nc.vector.reciprocal(rec[:, 0:nt, :], rec[:, 0:nt, :])
                nc.vector.tensor_tensor(
                    out=o1[:], in0=ob[:, :, 0:16],
                    in1=rec[:, 0:nt, :].rearrange("p t o -> p (t o)")
                    .to_broadcast([128, nt, 16]), op=ALU.mult)
                mx = ep.tile([128, NTB, 1], F32, tag="mx", bufs=2)
                nc.vector.tensor_reduce(out=mx[:, 0:nt, :], in_=o1[:],
                                        axis=AX.X, op=ALU.max)
                nc.vector.tensor_tensor(
                    out=o1[:], in0=o1[:],
                    in1=mx[:, 0:nt, :].rearrange("p t o -> p (t o)")
                    .to_broadcast([128, nt, 16]), op=ALU.subtract)
                es = ep.tile([128, NTB, 16], F16, tag="es", bufs=2)
                nc.scalar.activation(out=es[:, 0:nt, :], in_=o1[:],
                                     func=AF.Exp)
                nc.vector.tensor_reduce(out=ssum[:, t0:t1, :],
                                        in_=es[:, 0:nt, :], axis=AX.X,
                                        op=ALU.add)

            NTB = max(len(bt) for bt in meta["btiles"])
            _emit_batches(nc, meta, (mp, gp, wkp, ppA, zrow), t2_d.ap(),
                          idx_d, s_d, re_d, TW2, 128, 1, 17,
                          edge_ops, tile_out, batch_out)

            lns = ep.tile([128, NT, 1], F32, tag="lns")
            nc.scalar.activation(out=lns[:], in_=ssum[:], func=AF.Ln)
            nc.vector.tensor_tensor(
                out=o16[:], in0=o16[:],
                in1=lns[:].rearrange("p t o -> p (t o)")
                .to_broadcast([128, NT, 16]), op=ALU.subtract)
            nc.scalar.dma_start(
                o_d.ap().rearrange("(t p) f -> p t f", p=128), o16[:])
    nc.compile()
    return nc


# --------------------------------------------------------------------------
# the kernel
# --------------------------------------------------------------------------

def kernel(x, edge_index, W1, a_src1, a_dst1, b1, W2, a_src2, a_dst2, b2):
    x = np.asarray(x, np.float32)
    edge_index = np.asarray(edge_index)
    W1 = np.asarray(W1, np.float32)
    W2 = np.asarray(W2, np.float32)
    a_src1 = np.asarray(a_src1, np.float32)
    a_dst1 = np.asarray(a_dst1, np.float32)
    a_src2 = np.asarray(a_src2, np.float32)
    a_dst2 = np.asarray(a_dst2, np.float32)

    key = edge_index.tobytes()[:4096]
    if _CACHE.get("key") != key:
        meta = _preprocess(edge_index)
        idx_all, s_all, streams = _build_idx_and_s(meta)
        _CACHE.update(key=key, meta=meta, idx_all=idx_all, s_all=s_all,
                      streams=streams,
                      nc1=_build_launch1(), nc2=_build_launch2(meta),
                      nc3=_build_launch3(meta))
    meta = _CACHE["meta"]
    idx_all, s_all, streams = (_CACHE["idx_all"], _CACHE["s_all"],
                               _CACHE["streams"])

    # weight packing
    W1r = W1.reshape(IN, HEADS, HID)
    B1 = np.einsum("khc,hc->kh", W1r, a_src1)        # [256, 8]
    C1 = np.einsum("khc,hc->kh", W1r, a_dst1)
    wc = np.concatenate([W1, B1, C1], 1).astype(np.float16)   # [256, 80]
    w2a = W2 @ a_src2[0]                              # [64]
    w2d = W2 @ a_dst2[0]
    w2e = np.concatenate([W2, w2a[:, None], w2d[:, None]],
                         1).astype(np.float16)        # [64, 18]
    idm = np.eye(128, dtype=np.float16)

    # launch 1: build T1 slices
    perm = meta["perm_nodes"]
    xT = np.zeros((IN, NROWS), np.float16)
    real = perm >= 0
    xT[:, real] = x[perm[real]].astype(np.float16).T
    in1 = [{"xs": np.ascontiguousarray(xT[:, c * MPC:(c + 1) * MPC]),
            "wc": wc} for c in range(NCORES)]
    r1_res = bass_utils.run_bass_kernel_spmd(
        _CACHE["nc1"], in1, core_ids=list(range(NCORES)), trace=TRACE)
    T1 = np.zeros((NROWS, 256), np.uint8)
    for c in range(NCORES):
        T1[c * MPC:(c + 1) * MPC, 0:TB1] = \
            np.asarray(r1_res.results[c]["t1s"]).view(np.uint8)
    T1 = T1.view(ml_dtypes.float8_e4m3)

    # launch 2: layer-1 message passing -> T2 slices
    in2 = []
    for c in range(NCORES):
        re1 = _expand_stream(streams[c], np.asarray(r1_res.results[c]["r1"]),
                             8, meta["TOTG"])
        in2.append({"t1": T1, "idx": idx_all[c], "sall": s_all[c],
                    "re1": re1.reshape(128, -1), "w2e": w2e, "idm": idm})
    r2_res = bass_utils.run_bass_kernel_spmd(
        _CACHE["nc2"], in2, core_ids=list(range(NCORES)), trace=TRACE)
    T2 = np.zeros((NROWS, 128), np.float16)
    for c in range(NCORES):
        T2[c * MPC:(c + 1) * MPC, 0:TW2] = \
            np.asarray(r2_res.results[c]["t2s"])

    # launch 3: layer-2 + log_softmax
    in3 = []
    for c in range(NCORES):
        re2 = _expand_stream(streams[c], np.asarray(r2_res.results[c]["r2"]),
                             1, meta["TOTG"])
        in3.append({"t2": T2, "idx": idx_all[c], "sall": s_all[c],
                    "re2": re2.reshape(128, -1)})
    r3_res = bass_utils.run_bass_kernel_spmd(
        _CACHE["nc3"], in3, core_ids=list(range(NCORES)), trace=TRACE)
    o_all = np.concatenate([np.asarray(r3_res.results[c]["o"])
                            for c in range(NCORES)], 0)

    out = o_all[meta["pos"][np.arange(N)]].astype(np.float32)
    _CACHE["exec_ns"] = [r.exec_time_ns for r in (r1_res, r2_res, r3_res)]
    return out


def predict_ns():
    """Cost-model (TimelineSim) per-launch predictions for cached programs."""
    from concourse.timeline_sim import TimelineSim
    out = []
    for k in ("nc1", "nc2", "nc3"):
        out.append(TimelineSim(_CACHE[k]).simulate())
    return out


# revision 33
# speedup vs baseline: 2.3322x; 1.0756x over previous
"""2-layer GAT on 8 trn2 NeuronCores (Bass/Tile).

Node-partitioned (12500/core, padded 12544), edges assigned by destination,
per-edge dma_gather of source-node table rows, segment softmax via the
factorization  exp(leaky(s+a)) = A * max(exp(s), exp(0.2 s) * exp(-0.8 a))
(per-dst factor A cancels), segment sums via banded one-hot S matmuls on
the PE.  Three SPMD launches with host halo exchange between them:

  1. "build":  h1 = x @ W1 + attention projections -> per-node table T1
     rows of 96B: [h fp8e4 x64 | exp(s) fp16 x8 | exp(0.2 s) fp16 x8],
     256B row stride in DRAM; r1 = exp(-0.8 a) per node.
  2. "layer1": per-edge 96B gathers from T1 (cost-model: 8.5 ns/descriptor
     vs 22.8 at 256B), edge softmax, banded S matmuls -> per-node epilogue
     (batched: softmax-normalize, ELU, z = elu @ [W2|w2a|w2d] via PE
     transpose) -> T2 rows of 36B: [z fp16 x16 | exp(s2) | exp(0.2 s2)].
  3. "layer2": 36B gathers from T2, 17-wide messages [el*z | el], banded
     S matmuls, batched log_softmax epilogue (single Ln table load).

Folding W2 into the T2 table (z instead of the 64-wide hidden vector) cuts
layer-2 gather/message/matmul width 4x and removes the output-head matmul.
"""

import numpy as np
import ml_dtypes

import concourse.bacc as bacc
import concourse.tile as tile
import concourse.mybir as mybir
from concourse import bass_utils
from concourse.bass import ap_utils, exact_div, MemorySpace

F32 = mybir.dt.float32
F16 = mybir.dt.float16
F8 = mybir.dt.float8e4
I16 = mybir.dt.int16
AF = mybir.ActivationFunctionType
ALU = mybir.AluOpType
AX = mybir.AxisListType

# problem constants (hardcoded per the task statement)
NCORES = 8
N = 100000
IN = 256
HID = 8
HEADS = 8
OUT = 16
NEG = 0.2
NPC = 12500            # real nodes per core
MPC = 12544            # padded nodes per core (98 * 128)
NT = MPC // 128        # 98 dst tiles per core
BATCH_EDGES = 24576    # shared edge budget per batch
NROWS = NCORES * MPC   # 100352 table rows
WINR = 25088           # gather window rows (2 cores; int16-safe, and a
                       # node's window is then fixed by its core alone)
NWIN = (NROWS + WINR - 1) // WINR  # 4
EPS = 1e-16
SLAB1 = 14             # launch-1 chunks per slab (must divide NT)
TB1 = 96               # T1 gathered bytes: 64 fp8 h + 16 fp16 exps
TW2 = 18               # T2 row width in fp16: 16 z + 2 exps

_CACHE = {}
TRACE = False
GSPLIT = 96            # max slabs (x128 idxs) per dma_gather call
IDXR = 32              # idx tile partition replication (ucode reads <=32)


# --------------------------------------------------------------------------
# raw gather: InstDMAGatherAnt without the elem%256B assert (the non-
# transpose ucode path supports any elem size; only the row STRIDE must be
# a multiple of 256B)
# --------------------------------------------------------------------------

def _dma_gather_raw(ns, out_ap, in_ap, idxs_ap, num_idxs, elem_size,
                    elem_step, queue_num=0):
    assert idxs_ap.dtype == mybir.dt.int16
    assert in_ap.dtype == out_ap.dtype
    assert in_ap.space == MemorySpace.DRAM
    assert ap_utils.ap_is_contiguous(in_ap.ap[1:])
    assert ap_utils.ap_is_contiguous(out_ap.ap[1:])
    assert ap_utils.ap_is_contiguous(idxs_ap.ap[1:])
    assert in_ap.ap[-1][1] == out_ap.ap[-1][1] == elem_size
    assert out_ap.ap[0][1] * out_ap.ap[1][1] == (num_idxs + 127) // 128 * 128
    assert in_ap.ap[0][0] == elem_step
    stride_bytes_256 = exact_div(elem_step * mybir.dt.size(in_ap.dtype), 256)
    assert 0 < stride_bytes_256 < 256
    _in_ap = ns.lower_ap_dma(in_ap, for_custom_bir_dma=True)
    return ns.add_instruction(
        mybir.InstDMAGatherAnt(
            name=ns.bass.get_next_instruction_name(),
            ins=[*_in_ap, ns.lower_ap(idxs_ap),
                 ns.lower_val_access(ns.to_reg(num_idxs))],
            outs=[ns.lower_ap(out_ap)],
            transpose=False, num_idxs=num_idxs, elem_size=elem_size,
            stride_bytes_256=stride_bytes_256, gen_mode=0,
            single_packet=False, queue_num=queue_num,
            sbuf_tokens_per_rank=0, sbuf_free_dim_per_rank=0,
            sbuf_free_dim_pad_per_rank=0, sbuf_byte_offset=0))


# --------------------------------------------------------------------------
# host-side graph preprocessing (pure index work, unchanged from baseline)
# --------------------------------------------------------------------------

def _preprocess(edge_index):
    # self-loops are handled locally per core (no gather), so the edge
    # machinery only sees the real edges
    src = np.asarray(edge_index[0])
    dst = np.asarray(edge_index[1])

    # per-window in-degree of each dst node (window of an edge = source
    # core pair, independent of the permutation since WINR = 2*MPC)
    wsrc = (src // NPC) // 2                         # [E+N] source window
    degw = np.zeros((N, NWIN), np.int64)
    np.add.at(degw, (dst, wsrc), 1)

    # permutation: per core, bin-pack nodes into the 98 tiles so that each
    # tile's per-window edge counts match a shared target profile -> the
    # cross-core union schedule (stc = max over cores) has minimal slack
    tgt = degw.sum(0).astype(np.float64) / NCORES / NT         # [NWIN]
    pos = np.empty(N, np.int64)
    perm_nodes = np.empty(NROWS, np.int64)   # table row -> node id (or -1)
    perm_nodes.fill(-1)
    for c in range(NCORES):
        ids = np.arange(c * NPC, (c + 1) * NPC)
        dw = degw[ids]                               # [NPC, NWIN]
        order = np.argsort(-dw.sum(1), kind="stable")
        cur = np.zeros((NT, NWIN), np.int64)
        slots = np.full(NT, 128, np.int64)
        assign = np.empty(NPC, np.int64)
        for v in order:
            score = ((cur + dw[v]) - tgt).max(1)
            score[slots == 0] = np.inf
            b = int(np.argmin(score))
            assign[v] = b
            cur[b] += dw[v]
            slots[b] -= 1
        order2 = np.argsort(assign, kind="stable")
        cpt = np.bincount(assign, minlength=NT)
        starts = np.concatenate([[0], np.cumsum(cpt)])[:-1]
        within = np.arange(NPC) - starts[assign[order2]]
        rank = np.empty(NPC, np.int64)
        rank[order2] = assign[order2] * 128 + within
        pos[ids] = c * MPC + rank
        perm_nodes[c * MPC + rank] = ids

    srcpos = pos[src]
    dstpos = pos[dst]

    cores = []
    counts = np.zeros((NCORES, NT, NWIN), np.int64)
    per_core = []
    for c in range(NCORES):
        m = (dst >= c * NPC) & (dst < (c + 1) * NPC)
        sp = srcpos[m]
        rank = dstpos[m] - c * MPC
        t = rank // 128
        w = sp // WINR
        per_core.append((sp, rank, t, w))
        np.add.at(counts[c], (t, w), 1)
    stc = counts.max(0)                              # [NT, NWIN]
    tile_load = stc.sum(1)
    bmap = np.zeros(NT, np.int64)
    acc = 0
    b = 0
    for t in range(NT):
        if acc and acc + tile_load[t] > BATCH_EDGES:
            b += 1
            acc = 0
        bmap[t] = b
        acc += tile_load[t]
    NBAT = int(bmap[-1]) + 1
    btiles = [list(np.where(bmap == bb)[0]) for bb in range(NBAT)]
    toff = np.zeros((NT, NWIN), np.int64)
    gsz = np.zeros((NBAT, NWIN), np.int64)
    for bb in range(NBAT):
        for w in range(NWIN):
            off = 0
            for t in btiles[bb]:
                toff[t, w] = off
                off += stc[t, w]
            gsz[bb, w] = off
    G = np.maximum((gsz + 127) // 128, 1)            # [NBAT, NWIN] slabs
    Q = G * 128
    qoff = np.zeros((NBAT, NWIN), np.int64)
    goff = np.zeros((NBAT, NWIN), np.int64)
    acc_q = 0
    for bb in range(NBAT):
        for w in range(NWIN):
            qoff[bb, w] = acc_q
            goff[bb, w] = acc_q // 128
            acc_q += Q[bb, w]
    TOTQ = acc_q
    TOTG = TOTQ // 128

    for c in range(NCORES):
        sp, rank, t, w = per_core[c]
        b = bmap[t]
        order = np.lexsort((rank, w, t))
        sp, rank, t, w, b = (sp[order], rank[order], t[order], w[order],
                             b[order])
        gid = t * NWIN + w
        gstart = np.searchsorted(gid, np.arange(NT * NWIN), side="left")
        within = np.arange(len(gid)) - gstart[gid]
        q = qoff[b, w] + toff[t, w] + within
        cores.append({"sp": sp, "rank": rank, "b": b, "w": w, "q": q})

    # union matmul schedule, merged per (b, t, w, j) with a band range.
    JMAX = TOTQ // 128 + 1
    keysets = []
    for c in range(NCORES):
        d = cores[c]
        j = (d["q"] - qoff[d["b"], d["w"]]) // 128
        t = d["rank"] // 128
        a = (d["rank"] % 128) // 32
        key = (t * NWIN + d["w"]) * JMAX + j
        keysets.append((key, a))
        d["j"] = j
        d["t"] = t
        d["key"] = key
    allk = np.concatenate([k for k, _ in keysets])
    alla = np.concatenate([a for _, a in keysets])
    ukeys, inv = np.unique(allk, return_inverse=True)
    TOTB = len(ukeys)
    amin = np.full(TOTB, 4, np.int64)
    amax = np.full(TOTB, -1, np.int64)
    np.minimum.at(amin, inv, alla)
    np.maximum.at(amax, inv, alla)
    ecol = np.where(amin == amax, amin,
                    np.where((amin == 0) & (amax == 1), 0,
                             np.where((amin == 2) & (amax == 3), 2, 0)))
    ewid = np.where(amin == amax, 1,
                    np.where((amin == 0) & (amax == 1), 2,
                             np.where((amin == 2) & (amax == 3), 2, 4)))
    soff = np.concatenate([[0], np.cumsum(ewid)])   # block col offsets (32u)
    uj = ukeys % JMAX
    r1 = ukeys // JMAX
    uw = r1 % NWIN
    ut = r1 // NWIN
    ub = bmap[ut]
    sched = {"b": ub, "t": ut, "w": uw, "j": uj, "col": ecol, "wid": ewid,
             "soff": soff, "n": TOTB, "totw": int(soff[-1])}

    for c in range(NCORES):
        d = cores[c]
        ent = np.searchsorted(ukeys, d["key"])
        d["ent"] = ent
        d["k"] = d["q"] % 128
        d["scol"] = d["rank"] % 128 - ecol[ent] * 32

    meta = {"G": G, "Q": Q, "qoff": qoff, "goff": goff, "TOTQ": TOTQ,
            "TOTG": TOTG, "sched": sched, "pos": pos, "NBAT": NBAT,
            "btiles": btiles, "perm_nodes": perm_nodes, "cores": cores}
    return meta


def _build_idx_and_s(meta):
    """Per-core gather index arrays (int16 wrapped) and fp8 S blocks."""
    TOTQ = meta["TOTQ"]
    idx_all, s_all, streams = [], [], []
    for c in range(NCORES):
        d = meta["cores"][c]
        flat = np.zeros(TOTQ, np.int16)
        loc = d["sp"] - d["w"] * WINR
        flat[d["q"]] = loc.astype(np.int16)
        resh = flat.reshape(TOTQ // 16, 16).T          # [16, TOTQ/16]
        idxw = np.tile(resh, (IDXR // 16, 1)).copy()   # [IDXR, TOTQ/16]
        idx_all.append(idxw)

        soff = meta["sched"]["soff"]
        totw = meta["sched"]["totw"]
        S = np.zeros((128, totw * 32), ml_dtypes.float8_e4m3)
        S[d["k"], soff[d["ent"]] * 32 + d["scol"]] = 1.0
        s_all.append(S)

        streams.append((d["q"] % 128, d["q"] // 128, d["rank"]))
    return idx_all, s_all, streams


def _expand_stream(stream, r_core, width, totg):
    """r_core [MPC, width] f32 -> per-position [128, totg, width] f16."""
    p, g, rank = stream
    out = np.zeros((128, int(totg), width), np.float16)
    out[p, g, :] = r_core[rank, :width].astype(np.float16)
    return out


# --------------------------------------------------------------------------
# launch builders
# --------------------------------------------------------------------------

def _new_nc():
    return bacc.Bacc("TRN2", target_bir_lowering=False, debug=False,
                     enable_asserts=False, num_devices=NCORES)


def _build_launch1():
    nc = _new_nc()
    xs_d = nc.dram_tensor("xs", [IN, MPC], F16, kind="ExternalInput")
    wc_d = nc.dram_tensor("wc", [IN, 80], F16, kind="ExternalInput")
    t1_d = nc.dram_tensor("t1s", [MPC, TB1], F8, kind="ExternalOutput")
    r1_d = nc.dram_tensor("r1", [MPC, 8], F32, kind="ExternalOutput")
    SLAB = SLAB1
    with tile.TileContext(nc) as tc:
        with tc.tile_pool(name="w", bufs=1) as wp, \
             tc.tile_pool(name="x", bufs=3) as xp, \
             tc.tile_pool(name="o", bufs=3) as op, \
             tc.tile_pool(name="ps", bufs=4, space="PSUM") as pp:
            wc_sb = wp.tile([128, 2, 80], F16)
            nc.sync.dma_start(wc_sb[:, 0, :], wc_d.ap()[0:128, :])
            nc.sync.dma_start(wc_sb[:, 1, :], wc_d.ap()[128:256, :])
            for s in range(NT // SLAB):
                cols = slice(s * SLAB * 128, (s + 1) * SLAB * 128)
                xt0 = xp.tile([128, SLAB * 128], F16, tag="xt0")
                xt1 = xp.tile([128, SLAB * 128], F16, tag="xt1")
                nc.sync.dma_start(xt0[:], xs_d.ap()[0:128, cols])
                nc.sync.dma_start(xt1[:], xs_d.ap()[128:256, cols])
                tout = op.tile([128, SLAB, TB1], F8, tag="tout")
                ex = op.tile([128, SLAB, 16], F32, tag="ex")
                rout = op.tile([128, SLAB, 8], F32, tag="rout")
                for i in range(SLAB):
                    ps = pp.tile([128, 80], F32)
                    nc.tensor.matmul(ps[:], lhsT=xt0[:, i * 128:(i + 1) * 128],
                                     rhs=wc_sb[:, 0, :], start=True, stop=False)
                    nc.tensor.matmul(ps[:], lhsT=xt1[:, i * 128:(i + 1) * 128],
                                     rhs=wc_sb[:, 1, :], start=False, stop=True)
                    nc.vector.tensor_copy(tout[:, i, 0:64], ps[:, 0:64])
                    nc.vector.tensor_copy(ex[:, i, :], ps[:, 64:80])
                tv = tout[:, :, 64:96].bitcast(F16)      # [128, SLAB, 16]
                nc.scalar.activation(out=tv[:, :, 0:8], in_=ex[:, :, 0:8],
                                     func=AF.Exp)
                nc.scalar.activation(out=tv[:, :, 8:16], in_=ex[:, :, 0:8],
                                     func=AF.Exp, scale=0.2)
                nc.scalar.activation(out=rout[:], in_=ex[:, :, 8:16],
                                     func=AF.Exp, scale=-0.8)
                rows = slice(s * SLAB * 128, (s + 1) * SLAB * 128)
                nc.scalar.dma_start(
                    t1_d.ap()[rows, :].rearrange("(i p) f -> p i f", p=128),
                    tout[:])
                nc.scalar.dma_start(
                    r1_d.ap()[rows, :].rearrange("(i p) f -> p i f", p=128),
                    rout[:])
    nc.compile()
    return nc


def _batch_geometry(meta):
    G, qoff = meta["G"], meta["qoff"]
    sched = meta["sched"]
    soff = sched["soff"]
    NBAT = meta["NBAT"]
    sb = sched["b"]
    blo = np.searchsorted(sb, np.arange(NBAT))
    bhi = np.searchsorted(sb, np.arange(NBAT), side="right")
    slo = [int(soff[blo[b]]) for b in range(NBAT)]
    shi = [int(soff[bhi[b]]) for b in range(NBAT)]
    nw32max = max(1, max(shi[b] - slo[b] for b in range(NBAT)))
    qb_lo = [int(qoff[b, 0]) for b in range(NBAT)]
    qb_hi = [int(qoff[b, NWIN - 1] + G[b, NWIN - 1] * 128)
             for b in range(NBAT)]
    qbmax = max(qb_hi[b] - qb_lo[b] for b in range(NBAT))
    ent_by_t = {}
    for i in range(sched["n"]):
        ent_by_t.setdefault(int(sched["t"][i]), []).append(i)
    return blo, bhi, slo, shi, nw32max, qb_lo, qb_hi, qbmax, ent_by_t


def _emit_batches(nc, meta, pools, tab_ap, idx_d, s_d, re_d, elem, estep,
                  rwidth, mwidth, edge_ops, tile_out, batch_out=None):
    """Shared batch loop: gathers, edge ops, banded S matmuls.

    edge_ops(Gs, rs, el, msg) fills msg [128, g, mwidth];
    tile_out(t, ps) consumes the per-tile PSUM accumulator;
    batch_out(t0, t1) runs after each batch's tiles [t0, t1) complete."""
    G, qoff, goff = meta["G"], meta["qoff"], meta["goff"]
    sched = meta["sched"]
    sw, sj = sched["w"], sched["j"]
    scol, swid, soff = sched["col"], sched["wid"], sched["soff"]
    NBAT = meta["NBAT"]
    btiles = meta["btiles"]
    blo, bhi, slo, shi, nw32max, qb_lo, qb_hi, qbmax, ent_by_t = \
        _batch_geometry(meta)
    mp, gp, wkp, ppA, zrow = pools

    border = sorted(range(NBAT), key=lambda b: qb_lo[b] - qb_hi[b])
    for b in border:
        nw32 = max(shi[b] - slo[b], 1)
        ssb = mp.tile([128, nw32max, 32], F8, tag="s", bufs=2)
        if shi[b] > slo[b]:
            nc.sync.dma_start(
                ssb[:, 0:nw32, :],
                s_d.ap()[:, slo[b] * 32:shi[b] * 32]
                .rearrange("p (n c) -> p n c", c=32))
        nq = qb_hi[b] - qb_lo[b]
        idx_sb = mp.tile([IDXR, qbmax // 16], I16, tag="idx", bufs=2)
        nc.sync.dma_start(idx_sb[:, 0:nq // 16],
                          idx_d.ap()[:, qb_lo[b] // 16:qb_hi[b] // 16])
        slabs = {}
        for w in range(NWIN):
            g = int(G[b, w])
            q0 = int(qoff[b, w]) - qb_lo[b]
            g0 = int(goff[b, w])
            Gs = gp.tile([128, g, elem], tab_ap.dtype, tag="G", bufs=6)
            win0 = w * WINR
            win1 = min(win0 + WINR, NROWS)
            for g1 in range(0, g, GSPLIT):
                g2 = min(g1 + GSPLIT, g)
                nn = (g2 - g1) * 128
                _dma_gather_raw(
                    nc.gpsimd, Gs[:, g1:g2, :],
                    tab_ap[win0:win1, 0:elem],
                    idx_sb[:, (q0 + g1 * 128) // 16:(q0 + g2 * 128) // 16],
                    nn, elem, estep)
            rs = gp.tile([128, g, rwidth], F16, tag="rs", bufs=5)
            nc.sync.dma_start(
                rs[:], re_d.ap()[:, g0 * rwidth:(g0 + g) * rwidth]
                .rearrange("p (g r) -> p g r", r=rwidth))
            msg = wkp.tile([128, g, mwidth], F16, tag="msg", bufs=5)
            el = wkp.tile([128, g, rwidth], F16, tag="el", bufs=5)
            edge_ops(Gs, rs, el, msg)
            slabs[w] = msg
        for t in btiles[b]:
            ents = ent_by_t.get(t, [])
            ps = ppA.tile([128, mwidth], F32, tag="ps")
            nc.tensor.matmul(ps[:], lhsT=zrow[:], rhs=zrow[:, 0:mwidth],
                             start=True, stop=False, skip_group_check=True)
            for n, i in enumerate(ents):
                w, j = int(sw[i]), int(sj[i])
                col, wid = int(scol[i]), int(swid[i])
                so = int(soff[i]) - slo[b]
                nc.tensor.matmul(
                    ps[col * 32:(col + wid) * 32, :],
                    lhsT=ssb[:, so:so + wid, :]
                    .rearrange("p n c -> p (n c)"),
                    rhs=slabs[w][:, j, :],
                    start=False, stop=(n == len(ents) - 1),
                    tile_position=(0, col * 32),
                    skip_group_check=True)
            tile_out(t, ps)
        if batch_out is not None:
            batch_out(btiles[b][0], btiles[b][-1] + 1)


def _build_launch2(meta):
    nc = _new_nc()
    t1_d = nc.dram_tensor("t1", [NROWS, 256], F8, kind="ExternalInput")
    idx_d = nc.dram_tensor("idx", [IDXR, meta["TOTQ"] // 16], I16,
                           kind="ExternalInput")
    s_d = nc.dram_tensor("sall", [128, meta["sched"]["totw"] * 32], F8,
                         kind="ExternalInput")
    re_d = nc.dram_tensor("re1", [128, meta["TOTG"] * 8], F16,
                          kind="ExternalInput")
    to_d = nc.dram_tensor("t1own", [MPC, TB1], F8, kind="ExternalInput")
    ro_d = nc.dram_tensor("r1own", [MPC, 8], F16, kind="ExternalInput")
    w2_d = nc.dram_tensor("w2e", [64, 18], F16, kind="ExternalInput")
    id_d = nc.dram_tensor("idm", [128, 128], F16, kind="ExternalInput")
    t2_d = nc.dram_tensor("t2s", [MPC, TW2], F16, kind="ExternalOutput")
    r2_d = nc.dram_tensor("r2", [MPC, 1], F32, kind="ExternalOutput")

    with tile.TileContext(nc) as tc:
        with tc.tile_pool(name="res", bufs=1) as rp, \
             tc.tile_pool(name="m", bufs=1) as mp, \
             tc.tile_pool(name="g", bufs=1) as gp, \
             tc.tile_pool(name="wk", bufs=1) as wkp, \
             tc.tile_pool(name="ep", bufs=1) as ep, \
             tc.tile_pool(name="zi", bufs=3) as zp, \
             tc.tile_pool(name="psA", bufs=4, space="PSUM") as ppA, \
             tc.tile_pool(name="psB", bufs=2, space="PSUM") as ppB:
            zrow = rp.tile([1, 128], F16)
            nc.vector.memset(zrow[:], 0.0)
            w2_sb = rp.tile([64, 18], F16)
            nc.sync.dma_start(w2_sb[:], w2_d.ap())
            idm = rp.tile([128, 128], F16)
            nc.sync.dma_start(idm[:], id_d.ap())
            ybuf = rp.tile([128, NT, 72], F16)
            t2t = rp.tile([128, NT, TW2], F16)
            pbuf = rp.tile([128, NT, 2], F32)
            r2sb = rp.tile([128, NT], F32)

            def edge_ops(Gs, rs, el, msg):
                g = Gs.shape[1]
                es_v = Gs[:, :, 64:80].bitcast(F16)
                e02_v = Gs[:, :, 80:96].bitcast(F16)
                nc.vector.tensor_tensor(out=el[:], in0=e02_v, in1=rs[:],
                                        op=ALU.mult)
                nc.vector.tensor_tensor(out=el[:], in0=es_v, in1=el[:],
                                        op=ALU.max)
                nc.vector.tensor_tensor(
                    out=msg[:, :, 0:64].rearrange("p g (h c) -> p g h c", h=8),
                    in0=Gs[:, :, 0:64].rearrange("p g (h c) -> p g h c", h=8),
                    in1=el[:].to_broadcast([128, g, 8, 8]), op=ALU.mult)
                nc.vector.tensor_copy(msg[:, :, 64:72], el[:])

            def tile_out(t, ps):
                nc.scalar.copy(ybuf[:, t, :], ps[:])

            def batch_out(t0, t1):
                nt = t1 - t0
                yb = ybuf[:, t0:t1, :]
                # self-loop contribution from the core's own table slice
                ow = ep.tile([128, NTB, TB1], F8, tag="own", bufs=2)
                nc.sync.dma_start(
                    ow[:, 0:nt, :], to_d.ap()[t0 * 128:t1 * 128, :]
                    .rearrange("(t p) f -> p t f", p=128))
                rw = ep.tile([128, NTB, 8], F16, tag="rown", bufs=2)
                nc.sync.dma_start(
                    rw[:, 0:nt, :], ro_d.ap()[t0 * 128:t1 * 128, :]
                    .rearrange("(t p) f -> p t f", p=128))
                els = ep.tile([128, NTB, 8], F16, tag="els", bufs=2)
                nc.vector.tensor_tensor(
                    out=els[:, 0:nt, :], in0=ow[:, 0:nt, 80:96].bitcast(F16),
                    in1=rw[:, 0:nt, :], op=ALU.mult)
                nc.vector.tensor_tensor(
                    out=els[:, 0:nt, :], in0=ow[:, 0:nt, 64:80].bitcast(F16),
                    in1=els[:, 0:nt, :], op=ALU.max)
                msf = ep.tile([128, NTB, 64], F16, tag="msf", bufs=2)
                nc.vector.tensor_tensor(
                    out=msf[:, 0:nt, :]
                    .rearrange("p t (h c) -> p t h c", h=8),
                    in0=ow[:, 0:nt, 0:64]
                    .rearrange("p t (h c) -> p t h c", h=8),
                    in1=els[:, 0:nt, :].to_broadcast([128, nt, 8, 8]),
                    op=ALU.mult)
                nc.vector.tensor_tensor(out=yb[:, :, 0:64],
                                        in0=yb[:, :, 0:64],
                                        in1=msf[:, 0:nt, :], op=ALU.add)
                nc.vector.tensor_tensor(out=yb[:, :, 64:72],
                                        in0=yb[:, :, 64:72],
                                        in1=els[:, 0:nt, :], op=ALU.add)
                rec = ep.tile([128, NTB, 8], F32, tag="rec", bufs=2)
                nc.vector.tensor_scalar_add(rec[:, 0:nt, :],
                                            yb[:, :, 64:72], EPS)
                nc.vector.reciprocal(rec[:, 0:nt, :], rec[:, 0:nt, :])
                y16 = ep.tile([128, NTB, 64], F16, tag="y16", bufs=2)
                nc.vector.tensor_tensor(
                    out=y16[:, 0:nt, :]
                    .rearrange("p t (h c) -> p t h c", h=8),
                    in0=yb[:, :, 0:64].rearrange("p t (h c) -> p t h c", h=8),
                    in1=rec[:, 0:nt, :].to_broadcast([128, nt, 8, 8]),
                    op=ALU.mult)
                yn = ep.tile([128, NTB, 64], F16, tag="yn", bufs=2)
                nc.vector.tensor_scalar_min(yn[:, 0:nt, :], y16[:, 0:nt, :],
                                            0.0)
                nc.scalar.activation(out=yn[:, 0:nt, :], in_=yn[:, 0:nt, :],
                                     func=AF.Exp)
                nc.vector.tensor_scalar_add(yn[:, 0:nt, :], yn[:, 0:nt, :],
                                            -1.0)
                elu = ep.tile([128, NTB, 64], F16, tag="elu", bufs=2)
                nc.vector.tensor_tensor(out=elu[:, 0:nt, :],
                                        in0=y16[:, 0:nt, :],
                                        in1=yn[:, 0:nt, :], op=ALU.max)
                # z = elu @ [W2 | w2a | w2d] per tile via PE transpose
                for k0 in range(0, nt, 8):
                    nz = min(8, nt - k0)
                    zacc = ppB.tile([128, 8, 32], F32, tag="zacc", bufs=2)
                    for k in range(nz):
                        tp = ppB.tile([64, 128], F16, tag="tp", bufs=2)
                        nc.tensor.transpose(tp[:], elu[:, k0 + k, :], idm[:])
                        zin = zp.tile([64, 128], F16, tag="zin")
                        nc.scalar.copy(zin[:], tp[:])
                        nc.tensor.matmul(zacc[:, k, 0:18], lhsT=zin[:],
                                         rhs=w2_sb[:], start=True, stop=True)
                    tt = t0 + k0
                    nc.vector.tensor_copy(t2t[:, tt:tt + nz, 0:16],
                                          zacc[:, 0:nz, 0:16])
                    nc.vector.tensor_copy(pbuf[:, tt:tt + nz, :],
                                          zacc[:, 0:nz, 16:18])
                nc.scalar.activation(out=t2t[:, t0:t1, 16:17],
                                     in_=pbuf[:, t0:t1, 0:1], func=AF.Exp)
                nc.scalar.activation(out=t2t[:, t0:t1, 17:18],
                                     in_=pbuf[:, t0:t1, 0:1], func=AF.Exp,
                                     scale=0.2)
                nc.scalar.activation(out=r2sb[:, t0:t1],
                                     in_=pbuf[:, t0:t1, 1:2], func=AF.Exp,
                                     scale=-0.8)
                nc.scalar.dma_start(
                    t2_d.ap()[t0 * 128:t1 * 128, :]
                    .rearrange("(t p) f -> p t f", p=128), t2t[:, t0:t1, :])
                nc.scalar.dma_start(
                    r2_d.ap()[t0 * 128:t1 * 128, :]
                    .rearrange("(t p) o -> p (t o)", p=128), r2sb[:, t0:t1])

            NTB = max(len(bt) for bt in meta["btiles"])
            _emit_batches(nc, meta, (mp, gp, wkp, ppA, zrow), t1_d.ap(),
                          idx_d, s_d, re_d, TB1, 256, 8, 72,
                          edge_ops, tile_out, batch_out)
    nc.compile()
    return nc


def _build_launch3(meta):
    nc = _new_nc()
    t2_d = nc.dram_tensor("t2", [NROWS, 128], F16, kind="ExternalInput")
    idx_d = nc.dram_tensor("idx", [IDXR, meta["TOTQ"] // 16], I16,
                           kind="ExternalInput")
    s_d = nc.dram_tensor("sall", [128, meta["sched"]["totw"] * 32], F8,
                         kind="ExternalInput")
    re_d = nc.dram_tensor("re2", [128, meta["TOTG"]], F16,
                          kind="ExternalInput")
    to_d = nc.dram_tensor("t2own", [MPC, TW2], F16, kind="ExternalInput")
    ro_d = nc.dram_tensor("r2own", [MPC, 1], F16, kind="ExternalInput")
    o_d = nc.dram_tensor("o", [MPC, 16], F32, kind="ExternalOutput")

    with tile.TileContext(nc) as tc:
        with tc.tile_pool(name="res", bufs=1) as rp, \
             tc.tile_pool(name="m", bufs=1) as mp, \
             tc.tile_pool(name="g", bufs=1) as gp, \
             tc.tile_pool(name="wk", bufs=1) as wkp, \
             tc.tile_pool(name="ep", bufs=1) as ep, \
             tc.tile_pool(name="psA", bufs=4, space="PSUM") as ppA:
            zrow = rp.tile([1, 128], F16)
            nc.vector.memset(zrow[:], 0.0)
            obuf = rp.tile([128, NT, 17], F32)

            def edge_ops(Gs, rs, el, msg):
                g = Gs.shape[1]
                nc.vector.tensor_tensor(out=el[:], in0=Gs[:, :, 17:18],
                                        in1=rs[:], op=ALU.mult)
                nc.vector.tensor_tensor(out=el[:], in0=Gs[:, :, 16:17],
                                        in1=el[:], op=ALU.max)
                nc.vector.tensor_tensor(
                    out=msg[:, :, 0:16], in0=Gs[:, :, 0:16],
                    in1=el[:].rearrange("p g o -> p (g o)")
                    .to_broadcast([128, g, 16]), op=ALU.mult)
                nc.vector.tensor_copy(msg[:, :, 16:17], el[:])

            o16 = rp.tile([128, NT, 16], F32)
            ssum = rp.tile([128, NT, 1], F32)

            def tile_out(t, ps):
                nc.scalar.copy(obuf[:, t, :], ps[:])

            def batch_out(t0, t1):
                nt = t1 - t0
                ob = obuf[:, t0:t1, :]
                o1 = o16[:, t0:t1, :]
                ow = ep.tile([128, NTB, TW2], F16, tag="own", bufs=2)
                nc.sync.dma_start(
                    ow[:, 0:nt, :], to_d.ap()[t0 * 128:t1 * 128, :]
                    .rearrange("(t p) f -> p t f", p=128))
                rw = ep.tile([128, NTB, 1], F16, tag="rown", bufs=2)
                nc.sync.dma_start(
                    rw[:, 0:nt, :], ro_d.ap()[t0 * 128:t1 * 128, :]
                    .rearrange("(t p) f -> p t f", p=128))
                els = ep.tile([128, NTB, 1], F16, tag="els", bufs=2)
                nc.vector.tensor_tensor(out=els[:, 0:nt, :],
                                        in0=ow[:, 0:nt, 17:18],
                                        in1=rw[:, 0:nt, :], op=ALU.mult)
                nc.vector.tensor_tensor(out=els[:, 0:nt, :],
                                        in0=ow[:, 0:nt, 16:17],
                                        in1=els[:, 0:nt, :], op=ALU.max)
                msf = ep.tile([128, NTB, 16], F16, tag="msf", bufs=2)
                nc.vector.tensor_tensor(
                    out=msf[:, 0:nt, :], in0=ow[:, 0:nt, 0:16],
                    in1=els[:, 0:nt, :].rearrange("p t o -> p (t o)")
                    .to_broadcast([128, nt, 16]), op=ALU.mult)
                nc.vector.tensor_tensor(out=ob[:, :, 0:16],
                                        in0=ob[:, :, 0:16],
                                        in1=msf[:, 0:nt, :], op=ALU.add)
                nc.vector.tensor_tensor(out=ob[:, :, 16:17],
                                        in0=ob[:, :, 16:17],
                                        in1=els[:, 0:nt, :], op=ALU.add)
                rec = ep.tile([128, NTB, 1], F32, tag="rec", bufs=2)
                nc.vector.tensor_scalar_add(rec[:, 0:nt, :],
                                            ob[:, :, 16:17], EPS)
                nc.vector.reciprocal(rec[:, 0:nt, :], rec[:, 0:nt, :])
                nc.vector.tensor_tensor(
                    out=o1[:], in0=ob[:, :, 0:16],
                    in1=rec[:, 0:nt, :].rearrange("p t o -> p (t o)")
                    .to_broadcast([128, nt, 16]), op=ALU.mult)
                mx = ep.tile([128, NTB, 1], F32, tag="mx", bufs=2)
                nc.vector.tensor_reduce(out=mx[:, 0:nt, :], in_=o1[:],
                                        axis=AX.X, op=ALU.max)
                nc.vector.tensor_tensor(
                    out=o1[:], in0=o1[:],
                    in1=mx[:, 0:nt, :].rearrange("p t o -> p (t o)")
                    .to_broadcast([128, nt, 16]), op=ALU.subtract)
                es = ep.tile([128, NTB, 16], F16, tag="es", bufs=2)
                nc.scalar.activation(out=es[:, 0:nt, :], in_=o1[:],
                                     func=AF.Exp)
                nc.vector.tensor_reduce(out=ssum[:, t0:t1, :],
                                        in_=es[:, 0:nt, :], axis=AX.X,
                                        op=ALU.add)

            NTB = max(len(bt) for bt in meta["btiles"])
            _emit_batches(nc, meta, (mp, gp, wkp, ppA, zrow), t2_d.ap(),
                          idx_d, s_d, re_d, TW2, 128, 1, 17,
                          edge_ops, tile_out, batch_out)

            lns = ep.tile([128, NT, 1], F32, tag="lns")
            nc.scalar.activation(out=lns[:], in_=ssum[:], func=AF.Ln)
            nc.vector.tensor_tensor(
                out=o16[:], in0=o16[:],
                in1=lns[:].rearrange("p t o -> p (t o)")
                .to_broadcast([128, NT, 16]), op=ALU.subtract)
            nc.scalar.dma_start(
                o_d.ap().rearrange("(t p) f -> p t f", p=128), o16[:])
    nc.compile()
    return nc


# --------------------------------------------------------------------------
# the kernel
# --------------------------------------------------------------------------

def kernel(x, edge_index, W1, a_src1, a_dst1, b1, W2, a_src2, a_dst2, b2):
    x = np.asarray(x, np.float32)
    edge_index = np.asarray(edge_index)
    W1 = np.asarray(W1, np.float32)
    W2 = np.asarray(W2, np.float32)
    a_src1 = np.asarray(a_src1, np.float32)
    a_dst1 = np.asarray(a_dst1, np.float32)
    a_src2 = np.asarray(a_src2, np.float32)
    a_dst2 = np.asarray(a_dst2, np.float32)

    key = edge_index.tobytes()[:4096]
    if _CACHE.get("key") != key:
        meta = _preprocess(edge_index)
        idx_all, s_all, streams = _build_idx_and_s(meta)
        _CACHE.update(key=key, meta=meta, idx_all=idx_all, s_all=s_all,
                      streams=streams,
                      nc1=_build_launch1(), nc2=_build_launch2(meta),
                      nc3=_build_launch3(meta))
    meta = _CACHE["meta"]
    idx_all, s_all, streams = (_CACHE["idx_all"], _CACHE["s_all"],
                               _CACHE["streams"])

    # weight packing
    W1r = W1.reshape(IN, HEADS, HID)
    B1 = np.einsum("khc,hc->kh", W1r, a_src1)        # [256, 8]
    C1 = np.einsum("khc,hc->kh", W1r, a_dst1)
    wc = np.concatenate([W1, B1, C1], 1).astype(np.float16)   # [256, 80]
    w2a = W2 @ a_src2[0]                              # [64]
    w2d = W2 @ a_dst2[0]
    w2e = np.concatenate([W2, w2a[:, None], w2d[:, None]],
                         1).astype(np.float16)        # [64, 18]
    idm = np.eye(128, dtype=np.float16)

    # launch 1: build T1 slices
    perm = meta["perm_nodes"]
    xT = np.zeros((IN, NROWS), np.float16)
    real = perm >= 0
    xT[:, real] = x[perm[real]].astype(np.float16).T
    in1 = [{"xs": np.ascontiguousarray(xT[:, c * MPC:(c + 1) * MPC]),
            "wc": wc} for c in range(NCORES)]
    r1_res = bass_utils.run_bass_kernel_spmd(
        _CACHE["nc1"], in1, core_ids=list(range(NCORES)), trace=TRACE)
    T1 = np.zeros((NROWS, 256), np.uint8)
    for c in range(NCORES):
        T1[c * MPC:(c + 1) * MPC, 0:TB1] = \
            np.asarray(r1_res.results[c]["t1s"]).view(np.uint8)
    T1 = T1.view(ml_dtypes.float8_e4m3)

    # launch 2: layer-1 message passing -> T2 slices
    in2 = []
    for c in range(NCORES):
        re1 = _expand_stream(streams[c], np.asarray(r1_res.results[c]["r1"]),
                             8, meta["TOTG"])
        in2.append({"t1": T1, "idx": idx_all[c], "sall": s_all[c],
                    "re1": re1.reshape(128, -1), "w2e": w2e, "idm": idm,
                    "t1own": np.asarray(r1_res.results[c]["t1s"]),
                    "r1own": np.asarray(r1_res.results[c]["r1"])
                    .astype(np.float16)})
    r2_res = bass_utils.run_bass_kernel_spmd(
        _CACHE["nc2"], in2, core_ids=list(range(NCORES)), trace=TRACE)
    T2 = np.zeros((NROWS, 128), np.float16)
    for c in range(NCORES):
        T2[c * MPC:(c + 1) * MPC, 0:TW2] = \
            np.asarray(r2_res.results[c]["t2s"])

    # launch 3: layer-2 + log_softmax
    in3 = []
    for c in range(NCORES):
        re2 = _expand_stream(streams[c], np.asarray(r2_res.results[c]["r2"]),
                             1, meta["TOTG"])
        in3.append({"t2": T2, "idx": idx_all[c], "sall": s_all[c],
                    "re2": re2.reshape(128, -1),
                    "t2own": np.asarray(r2_res.results[c]["t2s"]),
                    "r2own": np.asarray(r2_res.results[c]["r2"])
                    .astype(np.float16)})
    r3_res = bass_utils.run_bass_kernel_spmd(
        _CACHE["nc3"], in3, core_ids=list(range(NCORES)), trace=TRACE)
    o_all = np.concatenate([np.asarray(r3_res.results[c]["o"])
                            for c in range(NCORES)], 0)

    out = o_all[meta["pos"][np.arange(N)]].astype(np.float32)
    _CACHE["exec_ns"] = [r.exec_time_ns for r in (r1_res, r2_res, r3_res)]
    return out


def predict_ns():
    """Cost-model (TimelineSim) per-launch predictions for cached programs."""
    from concourse.timeline_sim import TimelineSim
    out = []
    for k in ("nc1", "nc2", "nc3"):
        out.append(TimelineSim(_CACHE[k]).simulate())
    return out


# revision 34
# speedup vs baseline: 2.4200x; 1.0376x over previous
"""2-layer GAT on 8 trn2 NeuronCores (Bass/Tile).

Node-partitioned (12500/core, padded 12544), edges assigned by destination,
per-edge dma_gather of source-node table rows, segment softmax via the
factorization  exp(leaky(s+a)) = A * max(exp(s), exp(0.2 s) * exp(-0.8 a))
(per-dst factor A cancels), segment sums via banded one-hot S matmuls on
the PE.  Three SPMD launches with host halo exchange between them:

  1. "build":  h1 = x @ W1 + attention projections -> per-node table T1
     rows of 96B: [h fp8e4 x64 | exp(s) fp16 x8 | exp(0.2 s) fp16 x8],
     256B row stride in DRAM; r1 = exp(-0.8 a) per node.
  2. "layer1": per-edge 96B gathers from T1 (cost-model: 8.5 ns/descriptor
     vs 22.8 at 256B), edge softmax, banded S matmuls -> per-node epilogue
     (batched: softmax-normalize, ELU, z = elu @ [W2|w2a|w2d] via PE
     transpose) -> T2 rows of 36B: [z fp16 x16 | exp(s2) | exp(0.2 s2)].
  3. "layer2": 36B gathers from T2, 17-wide messages [el*z | el], banded
     S matmuls, batched log_softmax epilogue (single Ln table load).

Folding W2 into the T2 table (z instead of the 64-wide hidden vector) cuts
layer-2 gather/message/matmul width 4x and removes the output-head matmul.
"""

import numpy as np
import ml_dtypes

import concourse.bacc as bacc
import concourse.tile as tile
import concourse.mybir as mybir
from concourse import bass_utils
from concourse.bass import ap_utils, exact_div, MemorySpace

F32 = mybir.dt.float32
F16 = mybir.dt.float16
F8 = mybir.dt.float8e4
I16 = mybir.dt.int16
AF = mybir.ActivationFunctionType
ALU = mybir.AluOpType
AX = mybir.AxisListType

# problem constants (hardcoded per the task statement)
NCORES = 8
N = 100000
IN = 256
HID = 8
HEADS = 8
OUT = 16
NEG = 0.2
NPC = 12500            # real nodes per core
MPC = 12544            # padded nodes per core (98 * 128)
NT = MPC // 128        # 98 dst tiles per core
BATCH_EDGES = 24576    # shared edge budget per batch
NROWS = NCORES * MPC   # 100352 table rows
WINR = 25088           # gather window rows (2 cores; int16-safe, and a
                       # node's window is then fixed by its core alone)
NWIN = (NROWS + WINR - 1) // WINR  # 4
EPS = 1e-16
SLAB1 = 14             # launch-1 chunks per slab (must divide NT)
TB1 = 96               # T1 gathered bytes: 64 fp8 h + 16 fp16 exps
TW2 = 18               # T2 row width in fp16: 16 z + 2 exps

_CACHE = {}
TRACE = False
GSPLIT = 96            # max slabs (x128 idxs) per dma_gather call
IDXR = 32              # idx tile partition replication (ucode reads <=32)


# --------------------------------------------------------------------------
# raw gather: InstDMAGatherAnt without the elem%256B assert (the non-
# transpose ucode path supports any elem size; only the row STRIDE must be
# a multiple of 256B)
# --------------------------------------------------------------------------

def _dma_gather_raw(ns, out_ap, in_ap, idxs_ap, num_idxs, elem_size,
                    elem_step, queue_num=0):
    assert idxs_ap.dtype == mybir.dt.int16
    assert in_ap.dtype == out_ap.dtype
    assert in_ap.space == MemorySpace.DRAM
    assert ap_utils.ap_is_contiguous(in_ap.ap[1:])
    assert ap_utils.ap_is_contiguous(out_ap.ap[1:])
    assert ap_utils.ap_is_contiguous(idxs_ap.ap[1:])
    assert in_ap.ap[-1][1] == out_ap.ap[-1][1] == elem_size
    assert out_ap.ap[0][1] * out_ap.ap[1][1] == (num_idxs + 127) // 128 * 128
    assert in_ap.ap[0][0] == elem_step
    stride_bytes_256 = exact_div(elem_step * mybir.dt.size(in_ap.dtype), 256)
    assert 0 < stride_bytes_256 < 256
    _in_ap = ns.lower_ap_dma(in_ap, for_custom_bir_dma=True)
    return ns.add_instruction(
        mybir.InstDMAGatherAnt(
            name=ns.bass.get_next_instruction_name(),
            ins=[*_in_ap, ns.lower_ap(idxs_ap),
                 ns.lower_val_access(ns.to_reg(num_idxs))],
            outs=[ns.lower_ap(out_ap)],
            transpose=False, num_idxs=num_idxs, elem_size=elem_size,
            stride_bytes_256=stride_bytes_256, gen_mode=0,
            single_packet=False, queue_num=queue_num,
            sbuf_tokens_per_rank=0, sbuf_free_dim_per_rank=0,
            sbuf_free_dim_pad_per_rank=0, sbuf_byte_offset=0))


# --------------------------------------------------------------------------
# host-side graph preprocessing (pure index work, unchanged from baseline)
# --------------------------------------------------------------------------

def _preprocess(edge_index):
    # self-loops are handled locally per core (no gather), so the edge
    # machinery only sees the real edges
    src = np.asarray(edge_index[0])
    dst = np.asarray(edge_index[1])

    # per-window in-degree of each dst node (window of an edge = source
    # core pair, independent of the permutation since WINR = 2*MPC)
    wsrc = (src // NPC) // 2                         # [E+N] source window
    degw = np.zeros((N, NWIN), np.int64)
    np.add.at(degw, (dst, wsrc), 1)

    # permutation: per core, bin-pack nodes into the 98 tiles so that each
    # tile's per-window edge counts match a shared target profile -> the
    # cross-core union schedule (stc = max over cores) has minimal slack
    tgt = degw.sum(0).astype(np.float64) / NCORES / NT         # [NWIN]
    pos = np.empty(N, np.int64)
    perm_nodes = np.empty(NROWS, np.int64)   # table row -> node id (or -1)
    perm_nodes.fill(-1)
    for c in range(NCORES):
        ids = np.arange(c * NPC, (c + 1) * NPC)
        dw = degw[ids]                               # [NPC, NWIN]
        order = np.argsort(-dw.sum(1), kind="stable")
        cur = np.zeros((NT, NWIN), np.int64)
        slots = np.full(NT, 128, np.int64)
        assign = np.empty(NPC, np.int64)
        for v in order:
            score = ((cur + dw[v]) - tgt).max(1)
            score[slots == 0] = np.inf
            b = int(np.argmin(score))
            assign[v] = b
            cur[b] += dw[v]
            slots[b] -= 1
        order2 = np.argsort(assign, kind="stable")
        cpt = np.bincount(assign, minlength=NT)
        starts = np.concatenate([[0], np.cumsum(cpt)])[:-1]
        within = np.arange(NPC) - starts[assign[order2]]
        rank = np.empty(NPC, np.int64)
        rank[order2] = assign[order2] * 128 + within
        pos[ids] = c * MPC + rank
        perm_nodes[c * MPC + rank] = ids

    srcpos = pos[src]
    dstpos = pos[dst]

    cores = []
    counts = np.zeros((NCORES, NT, NWIN), np.int64)
    per_core = []
    for c in range(NCORES):
        m = (dst >= c * NPC) & (dst < (c + 1) * NPC)
        sp = srcpos[m]
        rank = dstpos[m] - c * MPC
        t = rank // 128
        w = sp // WINR
        per_core.append((sp, rank, t, w))
        np.add.at(counts[c], (t, w), 1)
    stc = counts.max(0)                              # [NT, NWIN]
    tile_load = stc.sum(1)
    bmap = np.zeros(NT, np.int64)
    acc = 0
    b = 0
    for t in range(NT):
        if acc and acc + tile_load[t] > BATCH_EDGES:
            b += 1
            acc = 0
        bmap[t] = b
        acc += tile_load[t]
    NBAT = int(bmap[-1]) + 1
    btiles = [list(np.where(bmap == bb)[0]) for bb in range(NBAT)]
    toff = np.zeros((NT, NWIN), np.int64)
    gsz = np.zeros((NBAT, NWIN), np.int64)
    for bb in range(NBAT):
        for w in range(NWIN):
            off = 0
            for t in btiles[bb]:
                toff[t, w] = off
                off += stc[t, w]
            gsz[bb, w] = off
    G = np.maximum((gsz + 127) // 128, 1)            # [NBAT, NWIN] slabs
    Q = G * 128
    qoff = np.zeros((NBAT, NWIN), np.int64)
    goff = np.zeros((NBAT, NWIN), np.int64)
    acc_q = 0
    for bb in range(NBAT):
        for w in range(NWIN):
            qoff[bb, w] = acc_q
            goff[bb, w] = acc_q // 128
            acc_q += Q[bb, w]
    TOTQ = acc_q
    TOTG = TOTQ // 128

    for c in range(NCORES):
        sp, rank, t, w = per_core[c]
        b = bmap[t]
        order = np.lexsort((rank, w, t))
        sp, rank, t, w, b = (sp[order], rank[order], t[order], w[order],
                             b[order])
        gid = t * NWIN + w
        gstart = np.searchsorted(gid, np.arange(NT * NWIN), side="left")
        within = np.arange(len(gid)) - gstart[gid]
        q = qoff[b, w] + toff[t, w] + within
        cores.append({"sp": sp, "rank": rank, "b": b, "w": w, "q": q})

    # union matmul schedule, merged per (b, t, w, j) with a band range.
    JMAX = TOTQ // 128 + 1
    keysets = []
    for c in range(NCORES):
        d = cores[c]
        j = (d["q"] - qoff[d["b"], d["w"]]) // 128
        t = d["rank"] // 128
        a = (d["rank"] % 128) // 32
        key = (t * NWIN + d["w"]) * JMAX + j
        keysets.append((key, a))
        d["j"] = j
        d["t"] = t
        d["key"] = key
    allk = np.concatenate([k for k, _ in keysets])
    alla = np.concatenate([a for _, a in keysets])
    ukeys, inv = np.unique(allk, return_inverse=True)
    TOTB = len(ukeys)
    amin = np.full(TOTB, 4, np.int64)
    amax = np.full(TOTB, -1, np.int64)
    np.minimum.at(amin, inv, alla)
    np.maximum.at(amax, inv, alla)
    ecol = np.where(amin == amax, amin,
                    np.where((amin == 0) & (amax == 1), 0,
                             np.where((amin == 2) & (amax == 3), 2, 0)))
    ewid = np.where(amin == amax, 1,
                    np.where((amin == 0) & (amax == 1), 2,
                             np.where((amin == 2) & (amax == 3), 2, 4)))
    soff = np.concatenate([[0], np.cumsum(ewid)])   # block col offsets (32u)
    uj = ukeys % JMAX
    r1 = ukeys // JMAX
    uw = r1 % NWIN
    ut = r1 // NWIN
    ub = bmap[ut]
    sched = {"b": ub, "t": ut, "w": uw, "j": uj, "col": ecol, "wid": ewid,
             "soff": soff, "n": TOTB, "totw": int(soff[-1])}

    for c in range(NCORES):
        d = cores[c]
        ent = np.searchsorted(ukeys, d["key"])
        d["ent"] = ent
        d["k"] = d["q"] % 128
        d["scol"] = d["rank"] % 128 - ecol[ent] * 32

    meta = {"G": G, "Q": Q, "qoff": qoff, "goff": goff, "TOTQ": TOTQ,
            "TOTG": TOTG, "sched": sched, "pos": pos, "NBAT": NBAT,
            "btiles": btiles, "perm_nodes": perm_nodes, "cores": cores}
    return meta


def _build_idx_and_s(meta):
    """Per-core gather index arrays (int16 wrapped) and fp8 S blocks."""
    TOTQ = meta["TOTQ"]
    idx_all, s_all, streams = [], [], []
    for c in range(NCORES):
        d = meta["cores"][c]
        flat = np.zeros(TOTQ, np.int16)
        loc = d["sp"] - d["w"] * WINR
        flat[d["q"]] = loc.astype(np.int16)
        resh = flat.reshape(TOTQ // 16, 16).T          # [16, TOTQ/16]
        idxw = np.tile(resh, (IDXR // 16, 1)).copy()   # [IDXR, TOTQ/16]
        idx_all.append(idxw)

        soff = meta["sched"]["soff"]
        totw = meta["sched"]["totw"]
        S = np.zeros((128, totw * 32), ml_dtypes.float8_e4m3)
        S[d["k"], soff[d["ent"]] * 32 + d["scol"]] = 1.0
        s_all.append(S)

        streams.append((d["q"] % 128, d["q"] // 128, d["rank"]))
    return idx_all, s_all, streams


def _expand_stream(stream, r_core, width, totg):
    """r_core [MPC, width] f32 -> per-position [128, totg, width] f16."""
    p, g, rank = stream
    out = np.zeros((128, int(totg), width), np.float16)
    out[p, g, :] = r_core[rank, :width].astype(np.float16)
    return out


# --------------------------------------------------------------------------
# launch builders
# --------------------------------------------------------------------------

def _new_nc():
    return bacc.Bacc("TRN2", target_bir_lowering=False, debug=False,
                     enable_asserts=False, num_devices=NCORES)


def _build_launch1():
    nc = _new_nc()
    xs_d = nc.dram_tensor("xs", [IN, MPC], F16, kind="ExternalInput")
    wc_d = nc.dram_tensor("wc", [IN, 80], F16, kind="ExternalInput")
    t1_d = nc.dram_tensor("t1s", [MPC, TB1], F8, kind="ExternalOutput")
    r1_d = nc.dram_tensor("r1", [MPC, 8], F32, kind="ExternalOutput")
    SLAB = SLAB1
    with tile.TileContext(nc) as tc:
        with tc.tile_pool(name="w", bufs=1) as wp, \
             tc.tile_pool(name="x", bufs=3) as xp, \
             tc.tile_pool(name="o", bufs=3) as op, \
             tc.tile_pool(name="ps", bufs=4, space="PSUM") as pp:
            wc_sb = wp.tile([128, 2, 80], F16)
            nc.sync.dma_start(wc_sb[:, 0, :], wc_d.ap()[0:128, :])
            nc.sync.dma_start(wc_sb[:, 1, :], wc_d.ap()[128:256, :])
            for s in range(NT // SLAB):
                cols = slice(s * SLAB * 128, (s + 1) * SLAB * 128)
                xt0 = xp.tile([128, SLAB * 128], F16, tag="xt0")
                xt1 = xp.tile([128, SLAB * 128], F16, tag="xt1")
                nc.sync.dma_start(xt0[:], xs_d.ap()[0:128, cols])
                nc.sync.dma_start(xt1[:], xs_d.ap()[128:256, cols])
                tout = op.tile([128, SLAB, TB1], F8, tag="tout")
                ex = op.tile([128, SLAB, 16], F32, tag="ex")
                rout = op.tile([128, SLAB, 8], F32, tag="rout")
                for i in range(SLAB):
                    ps = pp.tile([128, 80], F32)
                    nc.tensor.matmul(ps[:], lhsT=xt0[:, i * 128:(i + 1) * 128],
                                     rhs=wc_sb[:, 0, :], start=True, stop=False)
                    nc.tensor.matmul(ps[:], lhsT=xt1[:, i * 128:(i + 1) * 128],
                                     rhs=wc_sb[:, 1, :], start=False, stop=True)
                    nc.vector.tensor_copy(tout[:, i, 0:64], ps[:, 0:64])
                    nc.vector.tensor_copy(ex[:, i, :], ps[:, 64:80])
                tv = tout[:, :, 64:96].bitcast(F16)      # [128, SLAB, 16]
                nc.scalar.activation(out=tv[:, :, 0:8], in_=ex[:, :, 0:8],
                                     func=AF.Exp)
                nc.scalar.activation(out=tv[:, :, 8:16], in_=ex[:, :, 0:8],
                                     func=AF.Exp, scale=0.2)
                nc.scalar.activation(out=rout[:], in_=ex[:, :, 8:16],
                                     func=AF.Exp, scale=-0.8)
                rows = slice(s * SLAB * 128, (s + 1) * SLAB * 128)
                nc.scalar.dma_start(
                    t1_d.ap()[rows, :].rearrange("(i p) f -> p i f", p=128),
                    tout[:])
                nc.scalar.dma_start(
                    r1_d.ap()[rows, :].rearrange("(i p) f -> p i f", p=128),
                    rout[:])
    nc.compile()
    return nc


def _batch_geometry(meta):
    G, qoff = meta["G"], meta["qoff"]
    sched = meta["sched"]
    soff = sched["soff"]
    NBAT = meta["NBAT"]
    sb = sched["b"]
    blo = np.searchsorted(sb, np.arange(NBAT))
    bhi = np.searchsorted(sb, np.arange(NBAT), side="right")
    slo = [int(soff[blo[b]]) for b in range(NBAT)]
    shi = [int(soff[bhi[b]]) for b in range(NBAT)]
    nw32max = max(1, max(shi[b] - slo[b] for b in range(NBAT)))
    qb_lo = [int(qoff[b, 0]) for b in range(NBAT)]
    qb_hi = [int(qoff[b, NWIN - 1] + G[b, NWIN - 1] * 128)
             for b in range(NBAT)]
    qbmax = max(qb_hi[b] - qb_lo[b] for b in range(NBAT))
    ent_by_t = {}
    for i in range(sched["n"]):
        ent_by_t.setdefault(int(sched["t"][i]), []).append(i)
    return blo, bhi, slo, shi, nw32max, qb_lo, qb_hi, qbmax, ent_by_t


def _emit_batches(nc, meta, pools, tab_ap, idx_d, s_d, re_d, elem, estep,
                  rwidth, mwidth, edge_ops, tile_out, batch_out=None):
    """Shared batch loop: gathers, edge ops, banded S matmuls.

    edge_ops(Gs, rs, el, msg) fills msg [128, g, mwidth];
    tile_out(t, ps) consumes the per-tile PSUM accumulator;
    batch_out(t0, t1) runs after each batch's tiles [t0, t1) complete."""
    G, qoff, goff = meta["G"], meta["qoff"], meta["goff"]
    sched = meta["sched"]
    sw, sj = sched["w"], sched["j"]
    scol, swid, soff = sched["col"], sched["wid"], sched["soff"]
    NBAT = meta["NBAT"]
    btiles = meta["btiles"]
    blo, bhi, slo, shi, nw32max, qb_lo, qb_hi, qbmax, ent_by_t = \
        _batch_geometry(meta)
    mp, gp, wkp, ppA, zrow = pools

    border = sorted(range(NBAT), key=lambda b: qb_lo[b] - qb_hi[b])
    for b in border:
        nw32 = max(shi[b] - slo[b], 1)
        ssb = mp.tile([128, nw32max, 32], F8, tag="s", bufs=2)
        if shi[b] > slo[b]:
            nc.sync.dma_start(
                ssb[:, 0:nw32, :],
                s_d.ap()[:, slo[b] * 32:shi[b] * 32]
                .rearrange("p (n c) -> p n c", c=32))
        nq = qb_hi[b] - qb_lo[b]
        idx_sb = mp.tile([IDXR, qbmax // 16], I16, tag="idx", bufs=2)
        nc.sync.dma_start(idx_sb[:, 0:nq // 16],
                          idx_d.ap()[:, qb_lo[b] // 16:qb_hi[b] // 16])
        slabs = {}
        for w in range(NWIN):
            g = int(G[b, w])
            q0 = int(qoff[b, w]) - qb_lo[b]
            g0 = int(goff[b, w])
            Gs = gp.tile([128, g, elem], tab_ap.dtype, tag="G", bufs=6)
            win0 = w * WINR
            win1 = min(win0 + WINR, NROWS)
            rs = gp.tile([128, g, rwidth], F16, tag="rs", bufs=5)
            nc.sync.dma_start(
                rs[:], re_d.ap()[:, g0 * rwidth:(g0 + g) * rwidth]
                .rearrange("p (g r) -> p g r", r=rwidth))
            msg = wkp.tile([128, g, mwidth], F16, tag="msg", bufs=5)
            el = wkp.tile([128, g, rwidth], F16, tag="el", bufs=5)
            half = max((g + 1) // 2, 1)
            for g1 in range(0, g, half):
                g2 = min(g1 + half, g)
                nn = (g2 - g1) * 128
                _dma_gather_raw(
                    nc.gpsimd, Gs[:, g1:g2, :],
                    tab_ap[win0:win1, 0:elem],
                    idx_sb[:, (q0 + g1 * 128) // 16:(q0 + g2 * 128) // 16],
                    nn, elem, estep)
                edge_ops(Gs, rs, el, msg, g1, g2)
            slabs[w] = msg
        for t in btiles[b]:
            ents = ent_by_t.get(t, [])
            ps = ppA.tile([128, mwidth], F32, tag="ps")
            nc.tensor.matmul(ps[:], lhsT=zrow[:], rhs=zrow[:, 0:mwidth],
                             start=True, stop=False, skip_group_check=True)
            for n, i in enumerate(ents):
                w, j = int(sw[i]), int(sj[i])
                col, wid = int(scol[i]), int(swid[i])
                so = int(soff[i]) - slo[b]
                nc.tensor.matmul(
                    ps[col * 32:(col + wid) * 32, :],
                    lhsT=ssb[:, so:so + wid, :]
                    .rearrange("p n c -> p (n c)"),
                    rhs=slabs[w][:, j, :],
                    start=False, stop=(n == len(ents) - 1),
                    tile_position=(0, col * 32),
                    skip_group_check=True)
            tile_out(t, ps)
        if batch_out is not None:
            batch_out(btiles[b][0], btiles[b][-1] + 1)


def _build_launch2(meta):
    nc = _new_nc()
    t1_d = nc.dram_tensor("t1", [NROWS, 256], F8, kind="ExternalInput")
    idx_d = nc.dram_tensor("idx", [IDXR, meta["TOTQ"] // 16], I16,
                           kind="ExternalInput")
    s_d = nc.dram_tensor("sall", [128, meta["sched"]["totw"] * 32], F8,
                         kind="ExternalInput")
    re_d = nc.dram_tensor("re1", [128, meta["TOTG"] * 8], F16,
                          kind="ExternalInput")
    to_d = nc.dram_tensor("t1own", [MPC, TB1], F8, kind="ExternalInput")
    ro_d = nc.dram_tensor("r1own", [MPC, 8], F16, kind="ExternalInput")
    w2_d = nc.dram_tensor("w2e", [64, 18], F16, kind="ExternalInput")
    id_d = nc.dram_tensor("idm", [128, 128], F16, kind="ExternalInput")
    t2_d = nc.dram_tensor("t2s", [MPC, TW2], F16, kind="ExternalOutput")
    r2_d = nc.dram_tensor("r2", [MPC, 1], F32, kind="ExternalOutput")

    with tile.TileContext(nc) as tc:
        with tc.tile_pool(name="res", bufs=1) as rp, \
             tc.tile_pool(name="m", bufs=1) as mp, \
             tc.tile_pool(name="g", bufs=1) as gp, \
             tc.tile_pool(name="wk", bufs=1) as wkp, \
             tc.tile_pool(name="ep", bufs=1) as ep, \
             tc.tile_pool(name="zi", bufs=3) as zp, \
             tc.tile_pool(name="psA", bufs=4, space="PSUM") as ppA, \
             tc.tile_pool(name="psB", bufs=2, space="PSUM") as ppB:
            zrow = rp.tile([1, 128], F16)
            nc.vector.memset(zrow[:], 0.0)
            w2_sb = rp.tile([64, 18], F16)
            nc.sync.dma_start(w2_sb[:], w2_d.ap())
            idm = rp.tile([128, 128], F16)
            nc.sync.dma_start(idm[:], id_d.ap())
            ybuf = rp.tile([128, NT, 72], F16)
            t2t = rp.tile([128, NT, TW2], F16)
            pbuf = rp.tile([128, NT, 2], F32)
            r2sb = rp.tile([128, NT], F32)

            def edge_ops(Gs, rs, el, msg, g1, g2):
                sl = slice(g1, g2)
                ng = g2 - g1
                es_v = Gs[:, sl, 64:80].bitcast(F16)
                e02_v = Gs[:, sl, 80:96].bitcast(F16)
                nc.vector.tensor_tensor(out=el[:, sl, :], in0=e02_v,
                                        in1=rs[:, sl, :], op=ALU.mult)
                nc.vector.tensor_tensor(out=el[:, sl, :], in0=es_v,
                                        in1=el[:, sl, :], op=ALU.max)
                nc.vector.tensor_tensor(
                    out=msg[:, sl, 0:64]
                    .rearrange("p g (h c) -> p g h c", h=8),
                    in0=Gs[:, sl, 0:64]
                    .rearrange("p g (h c) -> p g h c", h=8),
                    in1=el[:, sl, :].to_broadcast([128, ng, 8, 8]),
                    op=ALU.mult)
                nc.vector.tensor_copy(msg[:, sl, 64:72], el[:, sl, :])

            def tile_out(t, ps):
                nc.scalar.copy(ybuf[:, t, :], ps[:])

            def batch_out(t0, t1):
                nt = t1 - t0
                yb = ybuf[:, t0:t1, :]
                # self-loop contribution from the core's own table slice
                ow = ep.tile([128, NTB, TB1], F8, tag="own", bufs=2)
                nc.sync.dma_start(
                    ow[:, 0:nt, :], to_d.ap()[t0 * 128:t1 * 128, :]
                    .rearrange("(t p) f -> p t f", p=128))
                rw = ep.tile([128, NTB, 8], F16, tag="rown", bufs=2)
                nc.sync.dma_start(
                    rw[:, 0:nt, :], ro_d.ap()[t0 * 128:t1 * 128, :]
                    .rearrange("(t p) f -> p t f", p=128))
                els = ep.tile([128, NTB, 8], F16, tag="els", bufs=2)
                nc.vector.tensor_tensor(
                    out=els[:, 0:nt, :], in0=ow[:, 0:nt, 80:96].bitcast(F16),
                    in1=rw[:, 0:nt, :], op=ALU.mult)
                nc.vector.tensor_tensor(
                    out=els[:, 0:nt, :], in0=ow[:, 0:nt, 64:80].bitcast(F16),
                    in1=els[:, 0:nt, :], op=ALU.max)
                msf = ep.tile([128, NTB, 64], F16, tag="msf", bufs=2)
                nc.vector.tensor_tensor(
                    out=msf[:, 0:nt, :]
                    .rearrange("p t (h c) -> p t h c", h=8),
                    in0=ow[:, 0:nt, 0:64]
                    .rearrange("p t (h c) -> p t h c", h=8),
                    in1=els[:, 0:nt, :].to_broadcast([128, nt, 8, 8]),
                    op=ALU.mult)
                nc.vector.tensor_tensor(out=yb[:, :, 0:64],
                                        in0=yb[:, :, 0:64],
                                        in1=msf[:, 0:nt, :], op=ALU.add)
                nc.vector.tensor_tensor(out=yb[:, :, 64:72],
                                        in0=yb[:, :, 64:72],
                                        in1=els[:, 0:nt, :], op=ALU.add)
                rec = ep.tile([128, NTB, 8], F32, tag="rec", bufs=2)
                nc.vector.tensor_scalar_add(rec[:, 0:nt, :],
                                            yb[:, :, 64:72], EPS)
                nc.vector.reciprocal(rec[:, 0:nt, :], rec[:, 0:nt, :])
                y16 = ep.tile([128, NTB, 64], F16, tag="y16", bufs=2)
                nc.vector.tensor_tensor(
                    out=y16[:, 0:nt, :]
                    .rearrange("p t (h c) -> p t h c", h=8),
                    in0=yb[:, :, 0:64].rearrange("p t (h c) -> p t h c", h=8),
                    in1=rec[:, 0:nt, :].to_broadcast([128, nt, 8, 8]),
                    op=ALU.mult)
                yn = ep.tile([128, NTB, 64], F16, tag="yn", bufs=2)
                nc.vector.tensor_scalar_min(yn[:, 0:nt, :], y16[:, 0:nt, :],
                                            0.0)
                nc.scalar.activation(out=yn[:, 0:nt, :], in_=yn[:, 0:nt, :],
                                     func=AF.Exp)
                nc.vector.tensor_scalar_add(yn[:, 0:nt, :], yn[:, 0:nt, :],
                                            -1.0)
                elu = ep.tile([128, NTB, 64], F16, tag="elu", bufs=2)
                nc.vector.tensor_tensor(out=elu[:, 0:nt, :],
                                        in0=y16[:, 0:nt, :],
                                        in1=yn[:, 0:nt, :], op=ALU.max)
                # z = elu @ [W2 | w2a | w2d] per tile via PE transpose
                for k0 in range(0, nt, 8):
                    nz = min(8, nt - k0)
                    zacc = ppB.tile([128, 8, 32], F32, tag="zacc", bufs=2)
                    for k in range(nz):
                        tp = ppB.tile([64, 128], F16, tag="tp", bufs=2)
                        nc.tensor.transpose(tp[:], elu[:, k0 + k, :], idm[:])
                        zin = zp.tile([64, 128], F16, tag="zin")
                        nc.scalar.copy(zin[:], tp[:])
                        nc.tensor.matmul(zacc[:, k, 0:18], lhsT=zin[:],
                                         rhs=w2_sb[:], start=True, stop=True)
                    tt = t0 + k0
                    nc.vector.tensor_copy(t2t[:, tt:tt + nz, 0:16],
                                          zacc[:, 0:nz, 0:16])
                    nc.vector.tensor_copy(pbuf[:, tt:tt + nz, :],
                                          zacc[:, 0:nz, 16:18])
                nc.scalar.activation(out=t2t[:, t0:t1, 16:17],
                                     in_=pbuf[:, t0:t1, 0:1], func=AF.Exp)
                nc.scalar.activation(out=t2t[:, t0:t1, 17:18],
                                     in_=pbuf[:, t0:t1, 0:1], func=AF.Exp,
                                     scale=0.2)
                nc.scalar.activation(out=r2sb[:, t0:t1],
                                     in_=pbuf[:, t0:t1, 1:2], func=AF.Exp,
                                     scale=-0.8)
                nc.scalar.dma_start(
                    t2_d.ap()[t0 * 128:t1 * 128, :]
                    .rearrange("(t p) f -> p t f", p=128), t2t[:, t0:t1, :])
                nc.scalar.dma_start(
                    r2_d.ap()[t0 * 128:t1 * 128, :]
                    .rearrange("(t p) o -> p (t o)", p=128), r2sb[:, t0:t1])

            NTB = max(len(bt) for bt in meta["btiles"])
            _emit_batches(nc, meta, (mp, gp, wkp, ppA, zrow), t1_d.ap(),
                          idx_d, s_d, re_d, TB1, 256, 8, 72,
                          edge_ops, tile_out, batch_out)
    nc.compile()
    return nc


def _build_launch3(meta):
    nc = _new_nc()
    t2_d = nc.dram_tensor("t2", [NROWS, 128], F16, kind="ExternalInput")
    idx_d = nc.dram_tensor("idx", [IDXR, meta["TOTQ"] // 16], I16,
                           kind="ExternalInput")
    s_d = nc.dram_tensor("sall", [128, meta["sched"]["totw"] * 32], F8,
                         kind="ExternalInput")
    re_d = nc.dram_tensor("re2", [128, meta["TOTG"]], F16,
                          kind="ExternalInput")
    to_d = nc.dram_tensor("t2own", [MPC, TW2], F16, kind="ExternalInput")
    ro_d = nc.dram_tensor("r2own", [MPC, 1], F16, kind="ExternalInput")
    o_d = nc.dram_tensor("o", [MPC, 16], F32, kind="ExternalOutput")

    with tile.TileContext(nc) as tc:
        with tc.tile_pool(name="res", bufs=1) as rp, \
             tc.tile_pool(name="m", bufs=1) as mp, \
             tc.tile_pool(name="g", bufs=1) as gp, \
             tc.tile_pool(name="wk", bufs=1) as wkp, \
             tc.tile_pool(name="ep", bufs=1) as ep, \
             tc.tile_pool(name="psA", bufs=4, space="PSUM") as ppA:
            zrow = rp.tile([1, 128], F16)
            nc.vector.memset(zrow[:], 0.0)
            obuf = rp.tile([128, NT, 17], F32)

            def edge_ops(Gs, rs, el, msg, g1, g2):
                sl = slice(g1, g2)
                ng = g2 - g1
                nc.vector.tensor_tensor(out=el[:, sl, :],
                                        in0=Gs[:, sl, 17:18],
                                        in1=rs[:, sl, :], op=ALU.mult)
                nc.vector.tensor_tensor(out=el[:, sl, :],
                                        in0=Gs[:, sl, 16:17],
                                        in1=el[:, sl, :], op=ALU.max)
                nc.vector.tensor_tensor(
                    out=msg[:, sl, 0:16], in0=Gs[:, sl, 0:16],
                    in1=el[:, sl, :].rearrange("p g o -> p (g o)")
                    .to_broadcast([128, ng, 16]), op=ALU.mult)
                nc.vector.tensor_copy(msg[:, sl, 16:17], el[:, sl, :])

            o16 = rp.tile([128, NT, 16], F32)
            ssum = rp.tile([128, NT, 1], F32)

            def tile_out(t, ps):
                nc.scalar.copy(obuf[:, t, :], ps[:])

            def batch_out(t0, t1):
                nt = t1 - t0
                ob = obuf[:, t0:t1, :]
                o1 = o16[:, t0:t1, :]
                ow = ep.tile([128, NTB, TW2], F16, tag="own", bufs=2)
                nc.sync.dma_start(
                    ow[:, 0:nt, :], to_d.ap()[t0 * 128:t1 * 128, :]
                    .rearrange("(t p) f -> p t f", p=128))
                rw = ep.tile([128, NTB, 1], F16, tag="rown", bufs=2)
                nc.sync.dma_start(
                    rw[:, 0:nt, :], ro_d.ap()[t0 * 128:t1 * 128, :]
                    .rearrange("(t p) f -> p t f", p=128))
                els = ep.tile([128, NTB, 1], F16, tag="els", bufs=2)
                nc.vector.tensor_tensor(out=els[:, 0:nt, :],
                                        in0=ow[:, 0:nt, 17:18],
                                        in1=rw[:, 0:nt, :], op=ALU.mult)
                nc.vector.tensor_tensor(out=els[:, 0:nt, :],
                                        in0=ow[:, 0:nt, 16:17],
                                        in1=els[:, 0:nt, :], op=ALU.max)
                msf = ep.tile([128, NTB, 16], F16, tag="msf", bufs=2)
                nc.vector.tensor_tensor(
                    out=msf[:, 0:nt, :], in0=ow[:, 0:nt, 0:16],
                    in1=els[:, 0:nt, :].rearrange("p t o -> p (t o)")
                    .to_broadcast([128, nt, 16]), op=ALU.mult)
                nc.vector.tensor_tensor(out=ob[:, :, 0:16],
                                        in0=ob[:, :, 0:16],
                                        in1=msf[:, 0:nt, :], op=ALU.add)
                nc.vector.tensor_tensor(out=ob[:, :, 16:17],
                                        in0=ob[:, :, 16:17],
                                        in1=els[:, 0:nt, :], op=ALU.add)
                rec = ep.tile([128, NTB, 1], F32, tag="rec", bufs=2)
                nc.vector.tensor_scalar_add(rec[:, 0:nt, :],
                                            ob[:, :, 16:17], EPS)
                nc.vector.reciprocal(rec[:, 0:nt, :], rec[:, 0:nt, :])
                nc.vector.tensor_tensor(
                    out=o1[:], in0=ob[:, :, 0:16],
                    in1=rec[:, 0:nt, :].rearrange("p t o -> p (t o)")
                    .to_broadcast([128, nt, 16]), op=ALU.mult)
                mx = ep.tile([128, NTB, 1], F32, tag="mx", bufs=2)
                nc.vector.tensor_reduce(out=mx[:, 0:nt, :], in_=o1[:],
                                        axis=AX.X, op=ALU.max)
                nc.vector.tensor_tensor(
                    out=o1[:], in0=o1[:],
                    in1=mx[:, 0:nt, :].rearrange("p t o -> p (t o)")
                    .to_broadcast([128, nt, 16]), op=ALU.subtract)
                es = ep.tile([128, NTB, 16], F16, tag="es", bufs=2)
                nc.scalar.activation(out=es[:, 0:nt, :], in_=o1[:],
                                     func=AF.Exp)
                nc.vector.tensor_reduce(out=ssum[:, t0:t1, :],
                                        in_=es[:, 0:nt, :], axis=AX.X,
                                        op=ALU.add)

            NTB = max(len(bt) for bt in meta["btiles"])
            _emit_batches(nc, meta, (mp, gp, wkp, ppA, zrow), t2_d.ap(),
                          idx_d, s_d, re_d, TW2, 128, 1, 17,
                          edge_ops, tile_out, batch_out)

            lns = ep.tile([128, NT, 1], F32, tag="lns")
            nc.scalar.activation(out=lns[:], in_=ssum[:], func=AF.Ln)
            nc.vector.tensor_tensor(
                out=o16[:], in0=o16[:],
                in1=lns[:].rearrange("p t o -> p (t o)")
                .to_broadcast([128, NT, 16]), op=ALU.subtract)
            nc.scalar.dma_start(
                o_d.ap().rearrange("(t p) f -> p t f", p=128), o16[:])
    nc.compile()
    return nc


# --------------------------------------------------------------------------
# the kernel
# --------------------------------------------------------------------------

def kernel(x, edge_index, W1, a_src1, a_dst1, b1, W2, a_src2, a_dst2, b2):
    x = np.asarray(x, np.float32)
    edge_index = np.asarray(edge_index)
    W1 = np.asarray(W1, np.float32)
    W2 = np.asarray(W2, np.float32)
    a_src1 = np.asarray(a_src1, np.float32)
    a_dst1 = np.asarray(a_dst1, np.float32)
    a_src2 = np.asarray(a_src2, np.float32)
    a_dst2 = np.asarray(a_dst2, np.float32)

    key = edge_index.tobytes()[:4096]
    if _CACHE.get("key") != key:
        meta = _preprocess(edge_index)
        idx_all, s_all, streams = _build_idx_and_s(meta)
        _CACHE.update(key=key, meta=meta, idx_all=idx_all, s_all=s_all,
                      streams=streams,
                      nc1=_build_launch1(), nc2=_build_launch2(meta),
                      nc3=_build_launch3(meta))
    meta = _CACHE["meta"]
    idx_all, s_all, streams = (_CACHE["idx_all"], _CACHE["s_all"],
                               _CACHE["streams"])

    # weight packing
    W1r = W1.reshape(IN, HEADS, HID)
    B1 = np.einsum("khc,hc->kh", W1r, a_src1)        # [256, 8]
    C1 = np.einsum("khc,hc->kh", W1r, a_dst1)
    wc = np.concatenate([W1, B1, C1], 1).astype(np.float16)   # [256, 80]
    w2a = W2 @ a_src2[0]                              # [64]
    w2d = W2 @ a_dst2[0]
    w2e = np.concatenate([W2, w2a[:, None], w2d[:, None]],
                         1).astype(np.float16)        # [64, 18]
    idm = np.eye(128, dtype=np.float16)

    # launch 1: build T1 slices
    perm = meta["perm_nodes"]
    xT = np.zeros((IN, NROWS), np.float16)
    real = perm >= 0
    xT[:, real] = x[perm[real]].astype(np.float16).T
    in1 = [{"xs": np.ascontiguousarray(xT[:, c * MPC:(c + 1) * MPC]),
            "wc": wc} for c in range(NCORES)]
    r1_res = bass_utils.run_bass_kernel_spmd(
        _CACHE["nc1"], in1, core_ids=list(range(NCORES)), trace=TRACE)
    T1 = np.zeros((NROWS, 256), np.uint8)
    for c in range(NCORES):
        T1[c * MPC:(c + 1) * MPC, 0:TB1] = \
            np.asarray(r1_res.results[c]["t1s"]).view(np.uint8)
    T1 = T1.view(ml_dtypes.float8_e4m3)

    # launch 2: layer-1 message passing -> T2 slices
    in2 = []
    for c in range(NCORES):
        re1 = _expand_stream(streams[c], np.asarray(r1_res.results[c]["r1"]),
                             8, meta["TOTG"])
        in2.append({"t1": T1, "idx": idx_all[c], "sall": s_all[c],
                    "re1": re1.reshape(128, -1), "w2e": w2e, "idm": idm,
                    "t1own": np.asarray(r1_res.results[c]["t1s"]),
                    "r1own": np.asarray(r1_res.results[c]["r1"])
                    .astype(np.float16)})
    r2_res = bass_utils.run_bass_kernel_spmd(
        _CACHE["nc2"], in2, core_ids=list(range(NCORES)), trace=TRACE)
    T2 = np.zeros((NROWS, 128), np.float16)
    for c in range(NCORES):
        T2[c * MPC:(c + 1) * MPC, 0:TW2] = \
            np.asarray(r2_res.results[c]["t2s"])

    # launch 3: layer-2 + log_softmax
    in3 = []
    for c in range(NCORES):
        re2 = _expand_stream(streams[c], np.asarray(r2_res.results[c]["r2"]),
                             1, meta["TOTG"])
        in3.append({"t2": T2, "idx": idx_all[c], "sall": s_all[c],
                    "re2": re2.reshape(128, -1),
                    "t2own": np.asarray(r2_res.results[c]["t2s"]),
                    "r2own": np.asarray(r2_res.results[c]["r2"])
                    .astype(np.float16)})
    r3_res = bass_utils.run_bass_kernel_spmd(
        _CACHE["nc3"], in3, core_ids=list(range(NCORES)), trace=TRACE)
    o_all = np.concatenate([np.asarray(r3_res.results[c]["o"])
                            for c in range(NCORES)], 0)

    out = o_all[meta["pos"][np.arange(N)]].astype(np.float32)
    _CACHE["exec_ns"] = [r.exec_time_ns for r in (r1_res, r2_res, r3_res)]
    return out


def predict_ns():
    """Cost-model (TimelineSim) per-launch predictions for cached programs."""
    from concourse.timeline_sim import TimelineSim
    out = []
    for k in ("nc1", "nc2", "nc3"):
        out.append(TimelineSim(_CACHE[k]).simulate())
    return out


# revision 35
# speedup vs baseline: 2.4303x; 1.0043x over previous
"""2-layer GAT on 8 trn2 NeuronCores (Bass/Tile).

Node-partitioned (12500/core, padded 12544), edges assigned by destination,
per-edge dma_gather of source-node table rows, segment softmax via the
factorization  exp(leaky(s+a)) = A * max(exp(s), exp(0.2 s) * exp(-0.8 a))
(per-dst factor A cancels), segment sums via banded one-hot S matmuls on
the PE.  Three SPMD launches with host halo exchange between them:

  1. "build":  h1 = x @ W1 + attention projections -> per-node table T1
     rows of 96B: [h fp8e4 x64 | exp(s) fp16 x8 | exp(0.2 s) fp16 x8],
     256B row stride in DRAM; r1 = exp(-0.8 a) per node.
  2. "layer1": per-edge 96B gathers from T1 (cost-model: 8.5 ns/descriptor
     vs 22.8 at 256B), edge softmax, banded S matmuls -> per-node epilogue
     (batched: softmax-normalize, ELU, z = elu @ [W2|w2a|w2d] via PE
     transpose) -> T2 rows of 36B: [z fp16 x16 | exp(s2) | exp(0.2 s2)].
  3. "layer2": 36B gathers from T2, 17-wide messages [el*z | el], banded
     S matmuls, batched log_softmax epilogue (single Ln table load).

Folding W2 into the T2 table (z instead of the 64-wide hidden vector) cuts
layer-2 gather/message/matmul width 4x and removes the output-head matmul.
"""

import numpy as np
import ml_dtypes

import concourse.bacc as bacc
import concourse.tile as tile
import concourse.mybir as mybir
from concourse import bass_utils
from concourse.bass import ap_utils, exact_div, MemorySpace

F32 = mybir.dt.float32
F16 = mybir.dt.float16
F8 = mybir.dt.float8e4
I16 = mybir.dt.int16
AF = mybir.ActivationFunctionType
ALU = mybir.AluOpType
AX = mybir.AxisListType

# problem constants (hardcoded per the task statement)
NCORES = 8
N = 100000
IN = 256
HID = 8
HEADS = 8
OUT = 16
NEG = 0.2
NPC = 12500            # real nodes per core
MPC = 12544            # padded nodes per core (98 * 128)
NT = MPC // 128        # 98 dst tiles per core
BATCH_EDGES = 24576    # shared edge budget per batch
NROWS = NCORES * MPC   # 100352 table rows
WINR = 25088           # gather window rows (2 cores; int16-safe, and a
                       # node's window is then fixed by its core alone)
NWIN = (NROWS + WINR - 1) // WINR  # 4
EPS = 1e-16
SLAB1 = 14             # launch-1 chunks per slab (must divide NT)
TB1 = 96               # T1 gathered bytes: 64 fp8 h + 16 fp16 exps
TW2 = 18               # T2 row width in fp16: 16 z + 2 exps

_CACHE = {}
TRACE = False
GSPLIT = 96            # max slabs (x128 idxs) per dma_gather call
IDXR = 32              # idx tile partition replication (ucode reads <=32)


# --------------------------------------------------------------------------
# raw gather: InstDMAGatherAnt without the elem%256B assert (the non-
# transpose ucode path supports any elem size; only the row STRIDE must be
# a multiple of 256B)
# --------------------------------------------------------------------------

def _dma_gather_raw(ns, out_ap, in_ap, idxs_ap, num_idxs, elem_size,
                    elem_step, queue_num=0):
    assert idxs_ap.dtype == mybir.dt.int16
    assert in_ap.dtype == out_ap.dtype
    assert in_ap.space == MemorySpace.DRAM
    assert ap_utils.ap_is_contiguous(in_ap.ap[1:])
    assert ap_utils.ap_is_contiguous(out_ap.ap[1:])
    assert ap_utils.ap_is_contiguous(idxs_ap.ap[1:])
    assert in_ap.ap[-1][1] == out_ap.ap[-1][1] == elem_size
    assert out_ap.ap[0][1] * out_ap.ap[1][1] == (num_idxs + 127) // 128 * 128
    assert in_ap.ap[0][0] == elem_step
    stride_bytes_256 = exact_div(elem_step * mybir.dt.size(in_ap.dtype), 256)
    assert 0 < stride_bytes_256 < 256
    _in_ap = ns.lower_ap_dma(in_ap, for_custom_bir_dma=True)
    return ns.add_instruction(
        mybir.InstDMAGatherAnt(
            name=ns.bass.get_next_instruction_name(),
            ins=[*_in_ap, ns.lower_ap(idxs_ap),
                 ns.lower_val_access(ns.to_reg(num_idxs))],
            outs=[ns.lower_ap(out_ap)],
            transpose=False, num_idxs=num_idxs, elem_size=elem_size,
            stride_bytes_256=stride_bytes_256, gen_mode=0,
            single_packet=False, queue_num=queue_num,
            sbuf_tokens_per_rank=0, sbuf_free_dim_per_rank=0,
            sbuf_free_dim_pad_per_rank=0, sbuf_byte_offset=0))


# --------------------------------------------------------------------------
# host-side graph preprocessing (pure index work, unchanged from baseline)
# --------------------------------------------------------------------------

def _preprocess(edge_index):
    # self-loops are handled locally per core (no gather), so the edge
    # machinery only sees the real edges
    src = np.asarray(edge_index[0])
    dst = np.asarray(edge_index[1])

    # per-window in-degree of each dst node (window of an edge = source
    # core pair, independent of the permutation since WINR = 2*MPC)
    wsrc = (src // NPC) // 2                         # [E+N] source window
    degw = np.zeros((N, NWIN), np.int64)
    np.add.at(degw, (dst, wsrc), 1)

    # permutation: per core, bin-pack nodes into the 98 tiles so that each
    # tile's per-window edge counts match a shared target profile -> the
    # cross-core union schedule (stc = max over cores) has minimal slack
    tgt = degw.sum(0).astype(np.float64) / NCORES / NT         # [NWIN]
    pos = np.empty(N, np.int64)
    perm_nodes = np.empty(NROWS, np.int64)   # table row -> node id (or -1)
    perm_nodes.fill(-1)
    for c in range(NCORES):
        ids = np.arange(c * NPC, (c + 1) * NPC)
        dw = degw[ids]                               # [NPC, NWIN]
        order = np.argsort(-dw.sum(1), kind="stable")
        cur = np.zeros((NT, NWIN), np.int64)
        slots = np.full(NT, 128, np.int64)
        assign = np.empty(NPC, np.int64)
        for v in order:
            score = ((cur + dw[v]) - tgt).max(1)
            score[slots == 0] = np.inf
            b = int(np.argmin(score))
            assign[v] = b
            cur[b] += dw[v]
            slots[b] -= 1
        order2 = np.argsort(assign, kind="stable")
        cpt = np.bincount(assign, minlength=NT)
        starts = np.concatenate([[0], np.cumsum(cpt)])[:-1]
        within = np.arange(NPC) - starts[assign[order2]]
        rank = np.empty(NPC, np.int64)
        rank[order2] = assign[order2] * 128 + within
        pos[ids] = c * MPC + rank
        perm_nodes[c * MPC + rank] = ids

    srcpos = pos[src]
    dstpos = pos[dst]

    cores = []
    counts = np.zeros((NCORES, NT, NWIN), np.int64)
    per_core = []
    for c in range(NCORES):
        m = (dst >= c * NPC) & (dst < (c + 1) * NPC)
        sp = srcpos[m]
        rank = dstpos[m] - c * MPC
        t = rank // 128
        w = sp // WINR
        per_core.append((sp, rank, t, w))
        np.add.at(counts[c], (t, w), 1)
    stc = counts.max(0)                              # [NT, NWIN]
    tile_load = stc.sum(1)
    bmap = np.zeros(NT, np.int64)
    acc = 0
    b = 0
    for t in range(NT):
        if acc and acc + tile_load[t] > BATCH_EDGES:
            b += 1
            acc = 0
        bmap[t] = b
        acc += tile_load[t]
    NBAT = int(bmap[-1]) + 1
    btiles = [list(np.where(bmap == bb)[0]) for bb in range(NBAT)]
    toff = np.zeros((NT, NWIN), np.int64)
    gsz = np.zeros((NBAT, NWIN), np.int64)
    for bb in range(NBAT):
        for w in range(NWIN):
            off = 0
            for t in btiles[bb]:
                toff[t, w] = off
                off += stc[t, w]
            gsz[bb, w] = off
    G = np.maximum((gsz + 127) // 128, 1)            # [NBAT, NWIN] slabs
    Q = G * 128
    qoff = np.zeros((NBAT, NWIN), np.int64)
    goff = np.zeros((NBAT, NWIN), np.int64)
    acc_q = 0
    for bb in range(NBAT):
        for w in range(NWIN):
            qoff[bb, w] = acc_q
            goff[bb, w] = acc_q // 128
            acc_q += Q[bb, w]
    TOTQ = acc_q
    TOTG = TOTQ // 128

    for c in range(NCORES):
        sp, rank, t, w = per_core[c]
        b = bmap[t]
        order = np.lexsort((rank, w, t))
        sp, rank, t, w, b = (sp[order], rank[order], t[order], w[order],
                             b[order])
        gid = t * NWIN + w
        gstart = np.searchsorted(gid, np.arange(NT * NWIN), side="left")
        within = np.arange(len(gid)) - gstart[gid]
        q = qoff[b, w] + toff[t, w] + within
        cores.append({"sp": sp, "rank": rank, "b": b, "w": w, "q": q})

    # union matmul schedule, merged per (b, t, w, j) with a band range.
    JMAX = TOTQ // 128 + 1
    keysets = []
    for c in range(NCORES):
        d = cores[c]
        j = (d["q"] - qoff[d["b"], d["w"]]) // 128
        t = d["rank"] // 128
        a = (d["rank"] % 128) // 32
        key = (t * NWIN + d["w"]) * JMAX + j
        keysets.append((key, a))
        d["j"] = j
        d["t"] = t
        d["key"] = key
    allk = np.concatenate([k for k, _ in keysets])
    alla = np.concatenate([a for _, a in keysets])
    ukeys, inv = np.unique(allk, return_inverse=True)
    TOTB = len(ukeys)
    amin = np.full(TOTB, 4, np.int64)
    amax = np.full(TOTB, -1, np.int64)
    np.minimum.at(amin, inv, alla)
    np.maximum.at(amax, inv, alla)
    ecol = np.where(amin == amax, amin,
                    np.where((amin == 0) & (amax == 1), 0,
                             np.where((amin == 2) & (amax == 3), 2, 0)))
    ewid = np.where(amin == amax, 1,
                    np.where((amin == 0) & (amax == 1), 2,
                             np.where((amin == 2) & (amax == 3), 2, 4)))
    soff = np.concatenate([[0], np.cumsum(ewid)])   # block col offsets (32u)
    uj = ukeys % JMAX
    r1 = ukeys // JMAX
    uw = r1 % NWIN
    ut = r1 // NWIN
    ub = bmap[ut]
    sched = {"b": ub, "t": ut, "w": uw, "j": uj, "col": ecol, "wid": ewid,
             "soff": soff, "n": TOTB, "totw": int(soff[-1])}

    for c in range(NCORES):
        d = cores[c]
        ent = np.searchsorted(ukeys, d["key"])
        d["ent"] = ent
        d["k"] = d["q"] % 128
        d["scol"] = d["rank"] % 128 - ecol[ent] * 32

    meta = {"G": G, "Q": Q, "qoff": qoff, "goff": goff, "TOTQ": TOTQ,
            "TOTG": TOTG, "sched": sched, "pos": pos, "NBAT": NBAT,
            "btiles": btiles, "perm_nodes": perm_nodes, "cores": cores}
    return meta


def _build_idx_and_s(meta):
    """Per-core gather index arrays (int16 wrapped) and fp8 S blocks."""
    TOTQ = meta["TOTQ"]
    idx_all, s_all, streams = [], [], []
    for c in range(NCORES):
        d = meta["cores"][c]
        flat = np.zeros(TOTQ, np.int16)
        loc = d["sp"] - d["w"] * WINR
        flat[d["q"]] = loc.astype(np.int16)
        resh = flat.reshape(TOTQ // 16, 16).T          # [16, TOTQ/16]
        idxw = np.tile(resh, (IDXR // 16, 1)).copy()   # [IDXR, TOTQ/16]
        idx_all.append(idxw)

        soff = meta["sched"]["soff"]
        totw = meta["sched"]["totw"]
        S = np.zeros((128, totw * 32), ml_dtypes.float8_e4m3)
        S[d["k"], soff[d["ent"]] * 32 + d["scol"]] = 1.0
        s_all.append(S)

        streams.append((d["q"] % 128, d["q"] // 128, d["rank"]))
    return idx_all, s_all, streams


def _expand_stream(stream, r_core, width, totg):
    """r_core [MPC, width] f32 -> per-position [128, totg, width] f16."""
    p, g, rank = stream
    out = np.zeros((128, int(totg), width), np.float16)
    out[p, g, :] = r_core[rank, :width].astype(np.float16)
    return out


# --------------------------------------------------------------------------
# launch builders
# --------------------------------------------------------------------------

def _new_nc():
    return bacc.Bacc("TRN2", target_bir_lowering=False, debug=False,
                     enable_asserts=False, num_devices=NCORES)


def _build_launch1():
    nc = _new_nc()
    xs_d = nc.dram_tensor("xs", [IN, MPC], F16, kind="ExternalInput")
    wc_d = nc.dram_tensor("wc", [IN, 80], F16, kind="ExternalInput")
    t1_d = nc.dram_tensor("t1s", [MPC, TB1], F8, kind="ExternalOutput")
    r1_d = nc.dram_tensor("r1", [MPC, 8], F32, kind="ExternalOutput")
    SLAB = SLAB1
    with tile.TileContext(nc) as tc:
        with tc.tile_pool(name="w", bufs=1) as wp, \
             tc.tile_pool(name="x", bufs=3) as xp, \
             tc.tile_pool(name="o", bufs=3) as op, \
             tc.tile_pool(name="ps", bufs=4, space="PSUM") as pp:
            wc_sb = wp.tile([128, 2, 80], F16)
            nc.sync.dma_start(wc_sb[:, 0, :], wc_d.ap()[0:128, :])
            nc.sync.dma_start(wc_sb[:, 1, :], wc_d.ap()[128:256, :])
            for s in range(NT // SLAB):
                cols = slice(s * SLAB * 128, (s + 1) * SLAB * 128)
                xt0 = xp.tile([128, SLAB * 128], F16, tag="xt0")
                xt1 = xp.tile([128, SLAB * 128], F16, tag="xt1")
                nc.sync.dma_start(xt0[:], xs_d.ap()[0:128, cols])
                nc.sync.dma_start(xt1[:], xs_d.ap()[128:256, cols])
                tout = op.tile([128, SLAB, TB1], F8, tag="tout")
                ex = op.tile([128, SLAB, 16], F32, tag="ex")
                rout = op.tile([128, SLAB, 8], F32, tag="rout")
                for i in range(SLAB):
                    ps = pp.tile([128, 80], F32)
                    nc.tensor.matmul(ps[:], lhsT=xt0[:, i * 128:(i + 1) * 128],
                                     rhs=wc_sb[:, 0, :], start=True, stop=False)
                    nc.tensor.matmul(ps[:], lhsT=xt1[:, i * 128:(i + 1) * 128],
                                     rhs=wc_sb[:, 1, :], start=False, stop=True)
                    nc.vector.tensor_copy(tout[:, i, 0:64], ps[:, 0:64])
                    nc.vector.tensor_copy(ex[:, i, :], ps[:, 64:80])
                tv = tout[:, :, 64:96].bitcast(F16)      # [128, SLAB, 16]
                nc.scalar.activation(out=tv[:, :, 0:8], in_=ex[:, :, 0:8],
                                     func=AF.Exp)
                nc.scalar.activation(out=tv[:, :, 8:16], in_=ex[:, :, 0:8],
                                     func=AF.Exp, scale=0.2)
                nc.scalar.activation(out=rout[:], in_=ex[:, :, 8:16],
                                     func=AF.Exp, scale=-0.8)
                rows = slice(s * SLAB * 128, (s + 1) * SLAB * 128)
                nc.scalar.dma_start(
                    t1_d.ap()[rows, :].rearrange("(i p) f -> p i f", p=128),
                    tout[:])
                nc.scalar.dma_start(
                    r1_d.ap()[rows, :].rearrange("(i p) f -> p i f", p=128),
                    rout[:])
    nc.compile()
    return nc


def _batch_geometry(meta):
    G, qoff = meta["G"], meta["qoff"]
    sched = meta["sched"]
    soff = sched["soff"]
    NBAT = meta["NBAT"]
    sb = sched["b"]
    blo = np.searchsorted(sb, np.arange(NBAT))
    bhi = np.searchsorted(sb, np.arange(NBAT), side="right")
    slo = [int(soff[blo[b]]) for b in range(NBAT)]
    shi = [int(soff[bhi[b]]) for b in range(NBAT)]
    nw32max = max(1, max(shi[b] - slo[b] for b in range(NBAT)))
    qb_lo = [int(qoff[b, 0]) for b in range(NBAT)]
    qb_hi = [int(qoff[b, NWIN - 1] + G[b, NWIN - 1] * 128)
             for b in range(NBAT)]
    qbmax = max(qb_hi[b] - qb_lo[b] for b in range(NBAT))
    ent_by_t = {}
    for i in range(sched["n"]):
        ent_by_t.setdefault(int(sched["t"][i]), []).append(i)
    return blo, bhi, slo, shi, nw32max, qb_lo, qb_hi, qbmax, ent_by_t


def _emit_batches(nc, meta, pools, tab_ap, idx_d, s_d, re_d, elem, estep,
                  rwidth, mwidth, edge_ops, tile_out, batch_out=None):
    """Shared batch loop: gathers, edge ops, banded S matmuls.

    edge_ops(Gs, rs, el, msg) fills msg [128, g, mwidth];
    tile_out(t, ps) consumes the per-tile PSUM accumulator;
    batch_out(t0, t1) runs after each batch's tiles [t0, t1) complete."""
    G, qoff, goff = meta["G"], meta["qoff"], meta["goff"]
    sched = meta["sched"]
    sw, sj = sched["w"], sched["j"]
    scol, swid, soff = sched["col"], sched["wid"], sched["soff"]
    NBAT = meta["NBAT"]
    btiles = meta["btiles"]
    blo, bhi, slo, shi, nw32max, qb_lo, qb_hi, qbmax, ent_by_t = \
        _batch_geometry(meta)
    mp, gp, wkp, ppA, zrow = pools

    border = sorted(range(NBAT), key=lambda b: qb_lo[b] - qb_hi[b])
    for b in border:
        nw32 = max(shi[b] - slo[b], 1)
        ssb = mp.tile([128, nw32max, 32], F8, tag="s", bufs=2)
        if shi[b] > slo[b]:
            nc.sync.dma_start(
                ssb[:, 0:nw32, :],
                s_d.ap()[:, slo[b] * 32:shi[b] * 32]
                .rearrange("p (n c) -> p n c", c=32))
        nq = qb_hi[b] - qb_lo[b]
        idx_sb = mp.tile([IDXR, qbmax // 16], I16, tag="idx", bufs=2)
        nc.sync.dma_start(idx_sb[:, 0:nq // 16],
                          idx_d.ap()[:, qb_lo[b] // 16:qb_hi[b] // 16])
        slabs = {}
        for w in range(NWIN):
            g = int(G[b, w])
            q0 = int(qoff[b, w]) - qb_lo[b]
            g0 = int(goff[b, w])
            Gs = gp.tile([128, g, elem], tab_ap.dtype, tag="G", bufs=6)
            win0 = w * WINR
            win1 = min(win0 + WINR, NROWS)
            rs = gp.tile([128, g, rwidth], F16, tag="rs", bufs=5)
            nc.sync.dma_start(
                rs[:], re_d.ap()[:, g0 * rwidth:(g0 + g) * rwidth]
                .rearrange("p (g r) -> p g r", r=rwidth))
            msg = wkp.tile([128, g, mwidth], F16, tag="msg", bufs=5)
            el = wkp.tile([128, g, rwidth], F16, tag="el", bufs=5)
            half = max((g + 1) // 2, 1)
            for g1 in range(0, g, half):
                g2 = min(g1 + half, g)
                nn = (g2 - g1) * 128
                _dma_gather_raw(
                    nc.gpsimd, Gs[:, g1:g2, :],
                    tab_ap[win0:win1, 0:elem],
                    idx_sb[:, (q0 + g1 * 128) // 16:(q0 + g2 * 128) // 16],
                    nn, elem, estep)
                edge_ops(Gs, rs, el, msg, g1, g2)
            slabs[w] = msg
        for t in btiles[b]:
            ents = ent_by_t.get(t, [])
            ps = ppA.tile([128, mwidth], F32, tag="ps")
            nc.tensor.matmul(ps[:], lhsT=zrow[:], rhs=zrow[:, 0:mwidth],
                             start=True, stop=False, skip_group_check=True)
            for n, i in enumerate(ents):
                w, j = int(sw[i]), int(sj[i])
                col, wid = int(scol[i]), int(swid[i])
                so = int(soff[i]) - slo[b]
                nc.tensor.matmul(
                    ps[col * 32:(col + wid) * 32, :],
                    lhsT=ssb[:, so:so + wid, :]
                    .rearrange("p n c -> p (n c)"),
                    rhs=slabs[w][:, j, :],
                    start=False, stop=(n == len(ents) - 1),
                    tile_position=(0, col * 32),
                    skip_group_check=True)
            tile_out(t, ps)
        if batch_out is not None:
            batch_out(btiles[b][0], btiles[b][-1] + 1)


def _build_launch2(meta):
    nc = _new_nc()
    t1_d = nc.dram_tensor("t1", [NROWS, 256], F8, kind="ExternalInput")
    idx_d = nc.dram_tensor("idx", [IDXR, meta["TOTQ"] // 16], I16,
                           kind="ExternalInput")
    s_d = nc.dram_tensor("sall", [128, meta["sched"]["totw"] * 32], F8,
                         kind="ExternalInput")
    re_d = nc.dram_tensor("re1", [128, meta["TOTG"] * 8], F16,
                          kind="ExternalInput")
    to_d = nc.dram_tensor("t1own", [MPC, TB1], F8, kind="ExternalInput")
    ro_d = nc.dram_tensor("r1own", [MPC, 8], F16, kind="ExternalInput")
    w2_d = nc.dram_tensor("w2e", [64, 18], F16, kind="ExternalInput")
    id_d = nc.dram_tensor("idm", [128, 128], F16, kind="ExternalInput")
    t2_d = nc.dram_tensor("t2s", [MPC, TW2], F16, kind="ExternalOutput")
    r2_d = nc.dram_tensor("r2", [MPC, 1], F32, kind="ExternalOutput")

    with tile.TileContext(nc) as tc:
        with tc.tile_pool(name="res", bufs=1) as rp, \
             tc.tile_pool(name="m", bufs=1) as mp, \
             tc.tile_pool(name="g", bufs=1) as gp, \
             tc.tile_pool(name="wk", bufs=1) as wkp, \
             tc.tile_pool(name="ep", bufs=1) as ep, \
             tc.tile_pool(name="zi", bufs=3) as zp, \
             tc.tile_pool(name="psA", bufs=4, space="PSUM") as ppA, \
             tc.tile_pool(name="psB", bufs=2, space="PSUM") as ppB:
            zrow = rp.tile([1, 128], F16)
            nc.vector.memset(zrow[:], 0.0)
            w2_sb = rp.tile([64, 18], F16)
            nc.sync.dma_start(w2_sb[:], w2_d.ap())
            idm = rp.tile([128, 128], F16)
            nc.sync.dma_start(idm[:], id_d.ap())
            ybuf = rp.tile([128, NT, 72], F16)
            t2t = rp.tile([128, NT, TW2], F16)
            pbuf = rp.tile([128, NT, 2], F32)
            r2sb = rp.tile([128, NT], F32)

            def edge_ops(Gs, rs, el, msg, g1, g2):
                sl = slice(g1, g2)
                ng = g2 - g1
                es_v = Gs[:, sl, 64:80].bitcast(F16)
                e02_v = Gs[:, sl, 80:96].bitcast(F16)
                eld = msg[:, sl, 64:72]
                nc.vector.tensor_tensor(out=eld, in0=e02_v,
                                        in1=rs[:, sl, :], op=ALU.mult)
                nc.vector.tensor_tensor(out=eld, in0=es_v, in1=eld,
                                        op=ALU.max)
                nc.vector.tensor_tensor(
                    out=msg[:, sl, 0:64]
                    .rearrange("p g (h c) -> p g h c", h=8),
                    in0=Gs[:, sl, 0:64]
                    .rearrange("p g (h c) -> p g h c", h=8),
                    in1=msg[:, sl, 64:72].to_broadcast([128, ng, 8, 8]),
                    op=ALU.mult)

            def tile_out(t, ps):
                nc.scalar.copy(ybuf[:, t, :], ps[:])

            def batch_out(t0, t1):
                nt = t1 - t0
                yb = ybuf[:, t0:t1, :]
                # self-loop contribution from the core's own table slice
                ow = ep.tile([128, NTB, TB1], F8, tag="own", bufs=2)
                nc.sync.dma_start(
                    ow[:, 0:nt, :], to_d.ap()[t0 * 128:t1 * 128, :]
                    .rearrange("(t p) f -> p t f", p=128))
                rw = ep.tile([128, NTB, 8], F16, tag="rown", bufs=2)
                nc.sync.dma_start(
                    rw[:, 0:nt, :], ro_d.ap()[t0 * 128:t1 * 128, :]
                    .rearrange("(t p) f -> p t f", p=128))
                els = ep.tile([128, NTB, 8], F16, tag="els", bufs=2)
                nc.vector.tensor_tensor(
                    out=els[:, 0:nt, :], in0=ow[:, 0:nt, 80:96].bitcast(F16),
                    in1=rw[:, 0:nt, :], op=ALU.mult)
                nc.vector.tensor_tensor(
                    out=els[:, 0:nt, :], in0=ow[:, 0:nt, 64:80].bitcast(F16),
                    in1=els[:, 0:nt, :], op=ALU.max)
                msf = ep.tile([128, NTB, 64], F16, tag="msf", bufs=2)
                nc.vector.tensor_tensor(
                    out=msf[:, 0:nt, :]
                    .rearrange("p t (h c) -> p t h c", h=8),
                    in0=ow[:, 0:nt, 0:64]
                    .rearrange("p t (h c) -> p t h c", h=8),
                    in1=els[:, 0:nt, :].to_broadcast([128, nt, 8, 8]),
                    op=ALU.mult)
                nc.vector.tensor_tensor(out=yb[:, :, 0:64],
                                        in0=yb[:, :, 0:64],
                                        in1=msf[:, 0:nt, :], op=ALU.add)
                nc.vector.tensor_tensor(out=yb[:, :, 64:72],
                                        in0=yb[:, :, 64:72],
                                        in1=els[:, 0:nt, :], op=ALU.add)
                rec = ep.tile([128, NTB, 8], F32, tag="rec", bufs=2)
                nc.vector.tensor_scalar_add(rec[:, 0:nt, :],
                                            yb[:, :, 64:72], EPS)
                nc.vector.reciprocal(rec[:, 0:nt, :], rec[:, 0:nt, :])
                y16 = ep.tile([128, NTB, 64], F16, tag="y16", bufs=2)
                nc.vector.tensor_tensor(
                    out=y16[:, 0:nt, :]
                    .rearrange("p t (h c) -> p t h c", h=8),
                    in0=yb[:, :, 0:64].rearrange("p t (h c) -> p t h c", h=8),
                    in1=rec[:, 0:nt, :].to_broadcast([128, nt, 8, 8]),
                    op=ALU.mult)
                yn = ep.tile([128, NTB, 64], F16, tag="yn", bufs=2)
                nc.vector.tensor_scalar_min(yn[:, 0:nt, :], y16[:, 0:nt, :],
                                            0.0)
                nc.scalar.activation(out=yn[:, 0:nt, :], in_=yn[:, 0:nt, :],
                                     func=AF.Exp)
                nc.vector.tensor_scalar_add(yn[:, 0:nt, :], yn[:, 0:nt, :],
                                            -1.0)
                elu = ep.tile([128, NTB, 64], F16, tag="elu", bufs=2)
                nc.vector.tensor_tensor(out=elu[:, 0:nt, :],
                                        in0=y16[:, 0:nt, :],
                                        in1=yn[:, 0:nt, :], op=ALU.max)
                # z = elu @ [W2 | w2a | w2d] per tile via PE transpose
                for k0 in range(0, nt, 8):
                    nz = min(8, nt - k0)
                    zacc = ppB.tile([128, 8, 32], F32, tag="zacc", bufs=2)
                    for k in range(nz):
                        tp = ppB.tile([64, 128], F16, tag="tp", bufs=2)
                        nc.tensor.transpose(tp[:], elu[:, k0 + k, :], idm[:])
                        zin = zp.tile([64, 128], F16, tag="zin")
                        nc.scalar.copy(zin[:], tp[:])
                        nc.tensor.matmul(zacc[:, k, 0:18], lhsT=zin[:],
                                         rhs=w2_sb[:], start=True, stop=True)
                    tt = t0 + k0
                    nc.vector.tensor_copy(t2t[:, tt:tt + nz, 0:16],
                                          zacc[:, 0:nz, 0:16])
                    nc.vector.tensor_copy(pbuf[:, tt:tt + nz, :],
                                          zacc[:, 0:nz, 16:18])
                nc.scalar.activation(out=t2t[:, t0:t1, 16:17],
                                     in_=pbuf[:, t0:t1, 0:1], func=AF.Exp)
                nc.scalar.activation(out=t2t[:, t0:t1, 17:18],
                                     in_=pbuf[:, t0:t1, 0:1], func=AF.Exp,
                                     scale=0.2)
                nc.scalar.activation(out=r2sb[:, t0:t1],
                                     in_=pbuf[:, t0:t1, 1:2], func=AF.Exp,
                                     scale=-0.8)
                nc.scalar.dma_start(
                    t2_d.ap()[t0 * 128:t1 * 128, :]
                    .rearrange("(t p) f -> p t f", p=128), t2t[:, t0:t1, :])
                nc.scalar.dma_start(
                    r2_d.ap()[t0 * 128:t1 * 128, :]
                    .rearrange("(t p) o -> p (t o)", p=128), r2sb[:, t0:t1])

            NTB = max(len(bt) for bt in meta["btiles"])
            _emit_batches(nc, meta, (mp, gp, wkp, ppA, zrow), t1_d.ap(),
                          idx_d, s_d, re_d, TB1, 256, 8, 72,
                          edge_ops, tile_out, batch_out)
    nc.compile()
    return nc


def _build_launch3(meta):
    nc = _new_nc()
    t2_d = nc.dram_tensor("t2", [NROWS, 128], F16, kind="ExternalInput")
    idx_d = nc.dram_tensor("idx", [IDXR, meta["TOTQ"] // 16], I16,
                           kind="ExternalInput")
    s_d = nc.dram_tensor("sall", [128, meta["sched"]["totw"] * 32], F8,
                         kind="ExternalInput")
    re_d = nc.dram_tensor("re2", [128, meta["TOTG"]], F16,
                          kind="ExternalInput")
    to_d = nc.dram_tensor("t2own", [MPC, TW2], F16, kind="ExternalInput")
    ro_d = nc.dram_tensor("r2own", [MPC, 1], F16, kind="ExternalInput")
    o_d = nc.dram_tensor("o", [MPC, 16], F32, kind="ExternalOutput")

    with tile.TileContext(nc) as tc:
        with tc.tile_pool(name="res", bufs=1) as rp, \
             tc.tile_pool(name="m", bufs=1) as mp, \
             tc.tile_pool(name="g", bufs=1) as gp, \
             tc.tile_pool(name="wk", bufs=1) as wkp, \
             tc.tile_pool(name="ep", bufs=1) as ep, \
             tc.tile_pool(name="psA", bufs=6, space="PSUM") as ppA:
            zrow = rp.tile([1, 128], F16)
            nc.vector.memset(zrow[:], 0.0)
            obuf = rp.tile([128, NT, 17], F32)

            def edge_ops(Gs, rs, el, msg, g1, g2):
                sl = slice(g1, g2)
                ng = g2 - g1
                eld = msg[:, sl, 16:17]
                nc.vector.tensor_tensor(out=eld, in0=Gs[:, sl, 17:18],
                                        in1=rs[:, sl, :], op=ALU.mult)
                nc.vector.tensor_tensor(out=eld, in0=Gs[:, sl, 16:17],
                                        in1=eld, op=ALU.max)
                nc.vector.tensor_tensor(
                    out=msg[:, sl, 0:16], in0=Gs[:, sl, 0:16],
                    in1=msg[:, sl, 16:17].rearrange("p g o -> p (g o)")
                    .to_broadcast([128, ng, 16]), op=ALU.mult)

            o16 = rp.tile([128, NT, 16], F32)
            ssum = rp.tile([128, NT, 1], F32)

            def tile_out(t, ps):
                nc.scalar.copy(obuf[:, t, :], ps[:])

            def batch_out(t0, t1):
                nt = t1 - t0
                ob = obuf[:, t0:t1, :]
                o1 = o16[:, t0:t1, :]
                ow = ep.tile([128, NTB, TW2], F16, tag="own", bufs=2)
                nc.sync.dma_start(
                    ow[:, 0:nt, :], to_d.ap()[t0 * 128:t1 * 128, :]
                    .rearrange("(t p) f -> p t f", p=128))
                rw = ep.tile([128, NTB, 1], F16, tag="rown", bufs=2)
                nc.sync.dma_start(
                    rw[:, 0:nt, :], ro_d.ap()[t0 * 128:t1 * 128, :]
                    .rearrange("(t p) f -> p t f", p=128))
                els = ep.tile([128, NTB, 1], F16, tag="els", bufs=2)
                nc.vector.tensor_tensor(out=els[:, 0:nt, :],
                                        in0=ow[:, 0:nt, 17:18],
                                        in1=rw[:, 0:nt, :], op=ALU.mult)
                nc.vector.tensor_tensor(out=els[:, 0:nt, :],
                                        in0=ow[:, 0:nt, 16:17],
                                        in1=els[:, 0:nt, :], op=ALU.max)
                msf = ep.tile([128, NTB, 16], F16, tag="msf", bufs=2)
                nc.vector.tensor_tensor(
                    out=msf[:, 0:nt, :], in0=ow[:, 0:nt, 0:16],
                    in1=els[:, 0:nt, :].rearrange("p t o -> p (t o)")
                    .to_broadcast([128, nt, 16]), op=ALU.mult)
                nc.vector.tensor_tensor(out=ob[:, :, 0:16],
                                        in0=ob[:, :, 0:16],
                                        in1=msf[:, 0:nt, :], op=ALU.add)
                nc.vector.tensor_tensor(out=ob[:, :, 16:17],
                                        in0=ob[:, :, 16:17],
                                        in1=els[:, 0:nt, :], op=ALU.add)
                rec = ep.tile([128, NTB, 1], F32, tag="rec", bufs=2)
                nc.vector.tensor_scalar_add(rec[:, 0:nt, :],
                                            ob[:, :, 16:17], EPS)
                nc.vector.reciprocal(rec[:, 0:nt, :], rec[:, 0:nt, :])
                nc.vector.tensor_tensor(
                    out=o1[:], in0=ob[:, :, 0:16],
                    in1=rec[:, 0:nt, :].rearrange("p t o -> p (t o)")
                    .to_broadcast([128, nt, 16]), op=ALU.mult)
                mx = ep.tile([128, NTB, 1], F32, tag="mx", bufs=2)
                nc.vector.tensor_reduce(out=mx[:, 0:nt, :], in_=o1[:],
                                        axis=AX.X, op=ALU.max)
                nc.vector.tensor_tensor(
                    out=o1[:], in0=o1[:],
                    in1=mx[:, 0:nt, :].rearrange("p t o -> p (t o)")
                    .to_broadcast([128, nt, 16]), op=ALU.subtract)
                es = ep.tile([128, NTB, 16], F16, tag="es", bufs=2)
                nc.scalar.activation(out=es[:, 0:nt, :], in_=o1[:],
                                     func=AF.Exp)
                nc.vector.tensor_reduce(out=ssum[:, t0:t1, :],
                                        in_=es[:, 0:nt, :], axis=AX.X,
                                        op=ALU.add)

            NTB = max(len(bt) for bt in meta["btiles"])
            _emit_batches(nc, meta, (mp, gp, wkp, ppA, zrow), t2_d.ap(),
                          idx_d, s_d, re_d, TW2, 128, 1, 17,
                          edge_ops, tile_out, batch_out)

            lns = ep.tile([128, NT, 1], F32, tag="lns")
            nc.scalar.activation(out=lns[:], in_=ssum[:], func=AF.Ln)
            nc.vector.tensor_tensor(
                out=o16[:], in0=o16[:],
                in1=lns[:].rearrange("p t o -> p (t o)")
                .to_broadcast([128, NT, 16]), op=ALU.subtract)
            nc.scalar.dma_start(
                o_d.ap().rearrange("(t p) f -> p t f", p=128), o16[:])
    nc.compile()
    return nc


# --------------------------------------------------------------------------
# the kernel
# --------------------------------------------------------------------------

def kernel(x, edge_index, W1, a_src1, a_dst1, b1, W2, a_src2, a_dst2, b2):
    x = np.asarray(x, np.float32)
    edge_index = np.asarray(edge_index)
    W1 = np.asarray(W1, np.float32)
    W2 = np.asarray(W2, np.float32)
    a_src1 = np.asarray(a_src1, np.float32)
    a_dst1 = np.asarray(a_dst1, np.float32)
    a_src2 = np.asarray(a_src2, np.float32)
    a_dst2 = np.asarray(a_dst2, np.float32)

    key = edge_index.tobytes()[:4096]
    if _CACHE.get("key") != key:
        meta = _preprocess(edge_index)
        idx_all, s_all, streams = _build_idx_and_s(meta)
        _CACHE.update(key=key, meta=meta, idx_all=idx_all, s_all=s_all,
                      streams=streams,
                      nc1=_build_launch1(), nc2=_build_launch2(meta),
                      nc3=_build_launch3(meta))
    meta = _CACHE["meta"]
    idx_all, s_all, streams = (_CACHE["idx_all"], _CACHE["s_all"],
                               _CACHE["streams"])

    # weight packing
    W1r = W1.reshape(IN, HEADS, HID)
    B1 = np.einsum("khc,hc->kh", W1r, a_src1)        # [256, 8]
    C1 = np.einsum("khc,hc->kh", W1r, a_dst1)
    wc = np.concatenate([W1, B1, C1], 1).astype(np.float16)   # [256, 80]
    w2a = W2 @ a_src2[0]                              # [64]
    w2d = W2 @ a_dst2[0]
    w2e = np.concatenate([W2, w2a[:, None], w2d[:, None]],
                         1).astype(np.float16)        # [64, 18]
    idm = np.eye(128, dtype=np.float16)

    # launch 1: build T1 slices
    perm = meta["perm_nodes"]
    xT = np.zeros((IN, NROWS), np.float16)
    real = perm >= 0
    xT[:, real] = x[perm[real]].astype(np.float16).T
    in1 = [{"xs": np.ascontiguousarray(xT[:, c * MPC:(c + 1) * MPC]),
            "wc": wc} for c in range(NCORES)]
    r1_res = bass_utils.run_bass_kernel_spmd(
        _CACHE["nc1"], in1, core_ids=list(range(NCORES)), trace=TRACE)
    T1 = np.zeros((NROWS, 256), np.uint8)
    for c in range(NCORES):
        T1[c * MPC:(c + 1) * MPC, 0:TB1] = \
            np.asarray(r1_res.results[c]["t1s"]).view(np.uint8)
    T1 = T1.view(ml_dtypes.float8_e4m3)

    # launch 2: layer-1 message passing -> T2 slices
    in2 = []
    for c in range(NCORES):
        re1 = _expand_stream(streams[c], np.asarray(r1_res.results[c]["r1"]),
                             8, meta["TOTG"])
        in2.append({"t1": T1, "idx": idx_all[c], "sall": s_all[c],
                    "re1": re1.reshape(128, -1), "w2e": w2e, "idm": idm,
                    "t1own": np.asarray(r1_res.results[c]["t1s"]),
                    "r1own": np.asarray(r1_res.results[c]["r1"])
                    .astype(np.float16)})
    r2_res = bass_utils.run_bass_kernel_spmd(
        _CACHE["nc2"], in2, core_ids=list(range(NCORES)), trace=TRACE)
    T2 = np.zeros((NROWS, 128), np.float16)
    for c in range(NCORES):
        T2[c * MPC:(c + 1) * MPC, 0:TW2] = \
            np.asarray(r2_res.results[c]["t2s"])

    # launch 3: layer-2 + log_softmax
    in3 = []
    for c in range(NCORES):
        re2 = _expand_stream(streams[c], np.asarray(r2_res.results[c]["r2"]),
                             1, meta["TOTG"])
        in3.append({"t2": T2, "idx": idx_all[c], "sall": s_all[c],
                    "re2": re2.reshape(128, -1),
                    "t2own": np.asarray(r2_res.results[c]["t2s"]),
                    "r2own": np.asarray(r2_res.results[c]["r2"])
                    .astype(np.float16)})
    r3_res = bass_utils.run_bass_kernel_spmd(
        _CACHE["nc3"], in3, core_ids=list(range(NCORES)), trace=TRACE)
    o_all = np.concatenate([np.asarray(r3_res.results[c]["o"])
                            for c in range(NCORES)], 0)

    out = o_all[meta["pos"][np.arange(N)]].astype(np.float32)
    _CACHE["exec_ns"] = [r.exec_time_ns for r in (r1_res, r2_res, r3_res)]
    return out


def predict_ns():
    """Cost-model (TimelineSim) per-launch predictions for cached programs."""
    from concourse.timeline_sim import TimelineSim
    out = []
    for k in ("nc1", "nc2", "nc3"):
        out.append(TimelineSim(_CACHE[k]).simulate())
    return out
